# revision 67
# baseline (speedup 1.0000x reference)
"""DualCrossAttention Trainium2 kernel (bf16).

Data-parallel: batch=8 across 8 NeuronCores, one batch element per core.
Per core: two cross-attentions + FFN + 3 LayerNorms on [768, 512] activations.

Layout: feature-major activations (x.T: [feature(part), seq(free)]); weights
host-pre-transposed and cast to bf16 so every projection is a full-rate PE
matmul (bf16 streams 1 cycle/row vs 1.5 for fp32-HIGH, and enables FWL).
Attention: S.T = k_h @ q_h.T with the two heads of a pair emitted as adjacent
matmuls into PE row-groups 0/64 (concurrent execution) writing one shared
PSUM tile, so a single wide ACT exp call covers both heads; the wm scale is
folded into the exp affine. O.T accumulates with a ones-augmented V column so
the softmax denominator lands in PSUM row 64. All per-position normalizers
(attn 1/denominator, LN rstd/mean*rstd) are broadcast across partitions with
tiny K=1 PE matmuls into PSUM — no DRAM bounce. All weights are prefetched
at kernel start (bf16 halves the SBUF/DMA footprint).
"""
import contextlib

import numpy as np
import ml_dtypes

import concourse.bacc as bacc
import concourse.bass as bass
import concourse.tile as tile
from concourse import mybir
from concourse.bass_utils import run_bass_kernel_spmd
class _Bacc(bacc.Bacc):
    """Bacc with Exp/Ln pinned to the natural_log_exp_and_others ACT table
    set: the default chooser alternates between exp_and_others (attention
    softmax) and natural_log (LayerNorm rstd), paying a ~2.7us table load at
    every switch. Removing Exp/Ln from the single-function sets makes both
    resolve to the combined set, leaving only the Gelu switches."""

    def insert_act_table_loads(self):
        from concourse.hw_specs import get_activation_tables
        import bass_rust as _bass_rust
        has_activation = any(
            isinstance(i, mybir.InstActivation)
            for b in self.main_func.blocks
            for i in b.instructions
        )
        if not has_activation:
            return
        AFt = mybir.ActivationFunctionType
        tables = []
        for name, fns in get_activation_tables(self.m.arch).items():
            if name == "exp_and_others":
                fns = fns - {AFt.Exp}
            elif name == "natural_log":
                fns = fns - {AFt.Ln}
            tables.append((name, fns))
        _bass_rust.insert_act_table_loads(self, tables)


F32 = mybir.dt.float32
F32R = mybir.dt.float32r
BF = mybir.dt.bfloat16
AF = mybir.ActivationFunctionType
ALU = mybir.AluOpType

H, KD, VD = 8, 64, 64
D, DF = 512, 2048
S1, S2, S3 = 768, 1024, 768
P = 128
NCH = D // P            # 4 feature chunks of the 512-dim residual stream
W = 1.25                # wm weight scale
INV_SQRT = 0.125        # 1/sqrt(64)
EPS = 1e-5

_PROGRAM_CACHE = {}
_USE_BASE_ATTN = False
_USE_ACT_COPY = True
_KO_MAJOR = True
_DEBUG_TAPS = False


def _regions(n):
    """Split free dim n into <=512 column regions (PSUM-bank aligned)."""
    out = []
    s = 0
    while s < n:
        e = min(s + 512, n)
        out.append((s, e))
        s = e
    return out


class _Ctx:
    """Shared handles for the emit helpers."""
    pass


def _emit_proj(nc, psum_pool, wT_sb, xT_sb, n_out, n_seq, k_chunks, consume,
               ko_major=False, m_group=None, tag="proj"):
    """out.T[o, i] = sum_d wT[d, o] * xT[d, i]; calls consume(m, psum_ap).

    ko_major=True holds m-tiles live and loops ko outer / m inner, so the
    first matmuls only need xT chunk 0 (pipelines into a producer of xT).
    m_group limits how many m-tiles are live at once (PSUM pressure): the
    m-range is processed in groups, each group ko-major.
    """
    if ko_major:
        n_m = n_out // P
        if m_group is None:
            m_group = n_m
        for m0 in range(0, n_m, m_group):
            ms = range(m0, min(m0 + m_group, n_m))
            tiles = {m: psum_pool.tile([P, 1024], F32, tag=tag,
                                       name=f"pp{tag}{m}") for m in ms}
            for ko in range(k_chunks):
                for m in ms:
                    for (a, b) in _regions(n_seq):
                        nc.tensor.matmul(
                            tiles[m][:, a:b],
                            wT_sb[:, ko, m * P:(m + 1) * P],
                            xT_sb[:, ko, a:b],
                            start=(ko == 0), stop=(ko == k_chunks - 1),
                        )
            for m in ms:
                consume(m, tiles[m][:, :n_seq])
        return
    for m in range(n_out // P):
        ps = psum_pool.tile([P, 1024], F32, tag="proj", name=f"pp{m}")
        for ko in range(k_chunks):
            for (a, b) in _regions(n_seq):
                nc.tensor.matmul(
                    ps[:, a:b],
                    wT_sb[:, ko, m * P:(m + 1) * P],
                    xT_sb[:, ko, a:b],
                    start=(ko == 0), stop=(ko == k_chunks - 1),
                )
        consume(m, ps[:, :n_seq])


def _emit_ln(nc, cx, psum_pool, z_src, resid, bias_pm, z_sb, y_sb, n_seq,
             scale_gb, out_cb=None, bc_tags=("proj", "proj")):
    """LayerNorm over the feature axis (partitions x NCH chunks).

    z_src(m) -> psum AP [P, n_seq] (projection output chunk m);
    z = psum + bias + resid is built in z_sb (bf16); stats via ones-matmuls
    (partition reduction on PE); rstd via exp(-0.5 ln(var+eps)); rstd and
    mean*rstd are broadcast to [P, n_seq] with K=1 PE matmuls into recycled
    proj-tag PSUM slots; y_sb = z*rstd_bc - mr_bc (may alias z_sb).
    """
    sb = cx.sb
    stat_z = psum_pool.tile([1, 768], F32, tag="ln_stat_z", bufs=1)
    stat_zsq = psum_pool.tile([1, 768], F32, tag="ln_stat_zsq", bufs=1)
    for m in range(NCH):
        ps = z_src(m)
        if bias_pm is not None:
            nc.vector.scalar_tensor_tensor(
                z_sb[:, m, :], ps, bias_pm[:, m:m + 1], resid[:, m, :],
                op0=ALU.add, op1=ALU.add)
        else:
            nc.vector.tensor_tensor(z_sb[:, m, :], ps, resid[:, m, :], ALU.add)
        zsq = cx.zsq_pool.tile([P, 768], BF, tag="ln_zsq", name=f"zsq{m}")
        # last chunk's square on DVE (shorter critical path); others on the
        # otherwise-idle gpsimd
        sq_eng = nc.vector if m == NCH - 1 else nc.gpsimd
        sq_eng.tensor_tensor(zsq[:, :n_seq], z_sb[:, m, :], z_sb[:, m, :],
                             ALU.mult)
        for (a, b) in _regions(n_seq):
            nc.tensor.matmul(stat_z[0:1, a:b], cx.ones_bf[:, 0:1],
                             z_sb[:, m, a:b],
                             start=(m == 0), stop=(m == NCH - 1))
            nc.tensor.matmul(stat_zsq[0:1, a:b], cx.ones_bf[:, 0:1],
                             zsq[:, a:b],
                             start=(m == 0), stop=(m == NCH - 1))
    # small per-position vectors, all on partition 0
    # msq = (sum_z/D)^2 on ACT (Square is in the pinned table set);
    # var = sum(z^2)/D - msq -> ln(var+eps) -> rstd = exp(-0.5*ln);
    # mr = (sum_z/D)*rstd fused in one scalar_tensor_tensor
    msq = sb.tile([1, n_seq], F32, tag="ln_msq")
    nc.scalar.activation(msq[:], stat_z[0:1, :n_seq], AF.Square, bias=0.0,
                         scale=1.0 / D)
    rstd_t = sb.tile([1, n_seq], F32, tag="ln_rstd")
    nc.vector.scalar_tensor_tensor(rstd_t[:], stat_zsq[0:1, :n_seq], 1.0 / D,
                                   msq[:], op0=ALU.mult, op1=ALU.subtract)
    nc.scalar.activation(rstd_t[:], rstd_t[:], AF.Ln, bias=cx.eps_sb[0:1, :],
                         scale=1.0)
    lnv = sb.tile([1, 2 * n_seq], BF, tag="lnv")
    nc.scalar.activation(lnv[:, 0:n_seq], rstd_t[:], AF.Exp, bias=0.0,
                         scale=-0.5)
    nc.vector.scalar_tensor_tensor(lnv[:, n_seq:2 * n_seq],
                                   stat_z[0:1, :n_seq], 1.0 / D,
                                   lnv[:, 0:n_seq], op0=ALU.mult,
                                   op1=ALU.mult)
    # broadcast rstd / mean*rstd across partitions with K=1 matmuls into
    # recycled proj-tag PSUM slots (no DRAM bounce)
    rstd_bc = psum_pool.tile([P, 1024], F32, tag=bc_tags[0], name="rstdbc")
    mr_bc = psum_pool.tile([P, 1024], F32, tag=bc_tags[1], name="mrbc")
    for (a, b) in _regions(n_seq):
        nc.tensor.matmul(rstd_bc[:, a:b], cx.ones128[:, :], lnv[0:1, a:b],
                         start=True, stop=True)
        nc.tensor.matmul(mr_bc[:, a:b], cx.ones128[:, :],
                         lnv[0:1, n_seq + a:n_seq + b], start=True, stop=True)
    for m in range(NCH):
        nc.vector.tensor_tensor(y_sb[:, m, :], z_sb[:, m, :],
                                rstd_bc[:, :n_seq], ALU.mult)
        nc.vector.tensor_tensor(y_sb[:, m, :], y_sb[:, m, :],
                                mr_bc[:, :n_seq], ALU.subtract)
        if scale_gb is not None:
            g_sb, b_sb = scale_gb
            nc.vector.tensor_scalar(
                y_sb[:, m, :], y_sb[:, m, :],
                g_sb[:, m:m + 1], b_sb[:, m:m + 1], op0=ALU.mult, op1=ALU.add)
        if out_cb is not None:
            out_cb(m)


def _attn_units(layer, n_kv):
    """Unit list: (j_list, qa, qb, exp_scale). One exp call per unit covers
    both heads of the pair (and both j's of a pair for layer 2)."""
    J = n_kv // P
    units = []
    if layer == 1:
        # wm1: (q<512, kv<512) and (q>=512, kv>=512) get W
        for j in range(J):
            lo = W * INV_SQRT if j * P < 512 else INV_SQRT
            hi = INV_SQRT if j * P < 512 else W * INV_SQRT
            units.append(([j], 0, 512, lo))
            units.append(([j], 512, 768, hi))
    else:
        # wm2: diagonal 256-blocks get W; kv pair jp covers block jp
        for jp in range(J // 2):
            for b in range(3):
                sc = W * INV_SQRT if b == jp else INV_SQRT
                units.append(([2 * jp, 2 * jp + 1], 256 * b, 256 * (b + 1),
                              sc))
    return units


def _exp_slices(layer, j, n_q):
    """Per (attention layer, key-chunk j): (col_lo, col_hi, exp scale)."""
    if layer == 1:
        jlo = j * P < 512
        s_lo = W * INV_SQRT if jlo else INV_SQRT
        s_hi = INV_SQRT if jlo else W * INV_SQRT
        return [(0, 512, s_lo), (512, n_q, s_hi)]
    blk = j // 2
    raw = [(b * 256, min((b + 1) * 256, n_q),
            W * INV_SQRT if b == blk else INV_SQRT) for b in range(3)]
    out = [raw[0]]
    for (lo, hi, sc) in raw[1:]:
        plo, phi, psc = out[-1]
        if sc == psc and lo == phi:
            out[-1] = (plo, hi, sc)
        else:
            out.append((lo, hi, sc))
    return out


def _emit_attn_base(nc, cx, work, psum_s, psum_ot, qT_sb, kT_sb, v_sb, ot_sb,
                    layer, n_q, n_kv):
    """Baseline-style attention: per-head S psum tiles + sliced exp."""
    J = n_kv // P
    heads = lambda c: ((slice(0, 64), 2 * c), (slice(64, 128), 2 * c + 1))
    units = [(j, a, b) for j in range(J) for (a, b) in _regions(n_q)]

    def emit_S_unit(c, u, etiles):
        (hb_e, h_e), (hb_o, h_o) = heads(c)
        j, a, b = units[u]
        if j not in etiles:
            etiles[j] = (
                work.tile([P, n_q], BF, tag="exps", bufs=4, name=f"ee{c}_{j}"),
                work.tile([P, n_q], BF, tag="exps", bufs=4, name=f"eo{c}_{j}"),
            )
        e_e, e_o = etiles[j]
        ps_e = psum_s.tile([P, 512], F32, tag="s", name=f"se{c}_{j}_{a}")
        ps_o = psum_s.tile([P, 512], F32, tag="s", name=f"so{c}_{j}_{a}")
        nc.tensor.matmul(ps_e[:, :b - a],
                         kT_sb[hb_e, c, j * P:(j + 1) * P],
                         qT_sb[hb_e, c, a:b], start=True, stop=True)
        nc.tensor.matmul(ps_o[:, :b - a],
                         kT_sb[hb_o, c, j * P:(j + 1) * P],
                         qT_sb[hb_o, c, a:b], start=True, stop=True)
        for e, ps in ((e_e, ps_e), (e_o, ps_o)):
            for (lo, hi, sc) in _exp_slices(layer, j, n_q):
                lo2, hi2 = max(lo, a), min(hi, b)
                if lo2 < hi2:
                    nc.scalar.activation(
                        e[:, lo2:hi2], ps[:, lo2 - a:hi2 - a],
                        AF.Exp, bias=0.0, scale=sc)

    def emit_O_unit(c, u, etiles, po_all):
        j, a, b = units[u]
        e_e, e_o = etiles[j]
        for (hb, h), e in zip(heads(c), (e_e, e_o)):
            nc.tensor.matmul(
                po_all[h][0:65, a:b],
                v_sb[:, j, h, 0:65],
                e[:, a:b],
                start=(j == 0), stop=(j == J - 1))

    for c in range(H // 2):
        po_all = {}
        for hb, h in heads(c):
            po_all[h] = psum_ot.tile([65, 768], F32, tag="ot", name=f"po{h}")
        etiles = {}
        emit_S_unit(c, 0, etiles)
        for u in range(len(units)):
            if u + 1 < len(units):
                emit_S_unit(c, u + 1, etiles)
            emit_O_unit(c, u, etiles, po_all)
        srow = work.tile([1, 2 * n_q], F32, tag="srow", bufs=2,
                         name=f"sr{c}")
        rr = work.tile([1, 2 * n_q], F32, tag="rr", bufs=2, name=f"rr{c}")
        (hb_e, h_e), (hb_o, h_o) = heads(c)
        # stage denominator rows at partition 0 in SBUF: the custom-DVE
        # reciprocal misreads PSUM at base partition 64 on hardware
        nc.vector.tensor_copy(srow[:, 0:n_q], po_all[h_e][64:65, 0:n_q])
        nc.vector.tensor_copy(srow[:, n_q:2 * n_q],
                              po_all[h_o][64:65, 0:n_q])
        nc.vector.reciprocal_approx_fast(out=rr[:, 0:n_q],
                                         in_=srow[:, 0:n_q])
        nc.vector.reciprocal_approx_fast(out=rr[:, n_q:2 * n_q],
                                         in_=srow[:, n_q:2 * n_q])
        nc.scalar.copy(ot_sb[hb_e, c, :], po_all[h_e][0:64, :n_q])
        nc.scalar.copy(ot_sb[hb_o, c, :], po_all[h_o][0:64, :n_q])
        drp = cx.dram.tile([2, n_q], F32, tag="drp", name=f"drp{c}")
        nc.sync.dma_start(drp[0:1, :], rr[:, 0:n_q])
        nc.sync.dma_start(drp[1:2, :], rr[:, n_q:2 * n_q])
        bc = work.tile([P, n_q], F32, tag="attn_bc", name=f"bc{c}")
        nc.gpsimd.dma_start(bc[0:64, :], drp[0:1, :].to_broadcast([64, n_q]))
        nc.gpsimd.dma_start(bc[64:128, :],
                            drp[1:2, :].to_broadcast([64, n_q]))
        nc.vector.tensor_tensor(ot_sb[:, c, :], ot_sb[:, c, :], bc[:],
                                ALU.mult)


def _emit_attn(nc, cx, work, psum_s, psum_ot, qT_sb, kT_sb, v_sb, ot_sb,
               layer, n_q, n_kv, filler=None):
    """Cross-attention. Per unit: the heads of a pair (x kv-pair for layer 2)
    are emitted as ADJACENT K=64 matmuls into PE row-groups 0/64 (concurrent)
    writing one shared PSUM tile; ONE ACT exp call (wm scale folded in)
    covers the whole tile. O.T accumulates per unit with a ones-augmented V
    column so the softmax denominator lands in PSUM row 64. S for unit u+1 is
    emitted before O of unit u so the PE has independent work while ACT
    computes exp.

    filler: optional per-pair list of callables emitting independent
    full-array PE work (borrowing an s-tag PSUM slot). Attention's K=64 /
    65-row matmuls only half-use the PE array and never re-warm the HAM
    clock gate; dense filler matmuls keep it at 2.4 GHz."""
    J = n_kv // P
    units = _attn_units(layer, n_kv)
    heads = lambda c: ((slice(0, 64), 2 * c), (slice(64, 128), 2 * c + 1))

    def ekey(jl):
        return jl[0]

    def emit_S_unit(c, u, etiles):
        jl, a, b, sc = units[u]
        w = b - a
        nj = len(jl)
        if ekey(jl) not in etiles:
            etiles[ekey(jl)] = work.tile([P, nj * 2, n_q], BF, tag="exps",
                                         bufs=3, name=f"e{c}_{ekey(jl)}")
        E = etiles[ekey(jl)]
        # Bank-safe layout: all head-even S in bank 0 ([0:512]), head-odd in
        # bank 1 ([512:1024]) — the two heads' matmuls execute CONCURRENTLY
        # via PE row-groups 0/64, and concurrent drains into the SAME PSUM
        # bank are a hardware fault. u-order (all-he, then all-ho) keeps the
        # psum stride regular (w) so one strided ACT exp covers the tile.
        ps = psum_s.tile([P, 1024], F32, tag="s", name=f"s{c}_{u}")
        (hb_e, h_e), (hb_o, h_o) = heads(c)
        for ji, j in enumerate(jl):
            nc.tensor.matmul(ps[:, ji * w:(ji + 1) * w],
                             kT_sb[hb_e, c, j * P:(j + 1) * P],
                             qT_sb[hb_e, c, a:b], start=True, stop=True)
            nc.tensor.matmul(ps[:, 512 + ji * w:512 + (ji + 1) * w],
                             kT_sb[hb_o, c, j * P:(j + 1) * P],
                             qT_sb[hb_o, c, a:b], start=True, stop=True)
        ps_g = ps[:, 0:1024].rearrange("p (g q) -> p g q", g=2)
        if nj == 1:
            nc.scalar.activation(E[:, :, a:b], ps_g[:, :, 0:w],
                                 AF.Exp, bias=0.0, scale=sc)
        else:
            nc.scalar.activation(
                E[:, :, a:b].rearrange("p (g j) q -> p g j q", g=2),
                ps_g[:, :, 0:nj * w].rearrange("p g (j q) -> p g j q", j=nj),
                AF.Exp, bias=0.0, scale=sc)

    def emit_O_unit(c, u, etiles, po_all):
        jl, a, b, sc = units[u]
        nj = len(jl)
        E = etiles[ekey(jl)]
        # start/stop must be unique per PSUM BANK (start=True clears the
        # whole bank's has_written bits): only the first/last matmul touching
        # a bank carries the flag; sibling regions in the same bank rely on
        # per-element overwrite-then-accumulate semantics.
        first_in_bank = a % 512 == 0
        last_in_bank = (b % 512 == 0) or (b == n_q)
        for ji, j in enumerate(jl):
            for hi, (hb, h) in enumerate(heads(c)):
                nc.tensor.matmul(
                    po_all[h][0:65, a:b],
                    v_sb[:, j, h, 0:65],
                    E[:, hi * nj + ji, a:b],
                    start=(j == 0 and first_in_bank),
                    stop=(j == J - 1 and last_in_bank))

    fill_i = [0]

    def feed_filler(c):
        # filler entries are (gate, fn): fn may only be emitted once pair
        # `gate` has been normalized (so a stalled piece never parks on an
        # s-slot the attention pipeline needs)
        if filler is not None and fill_i[0] < len(filler):
            gate, fn = filler[fill_i[0]]
            if gate < c:
                fn()
                fill_i[0] += 1

    for c in range(H // 2):
        po_all = {}
        for hb, h in heads(c):
            po_all[h] = psum_ot.tile([65, 768], F32, tag="ot", name=f"po{h}")
        etiles = {}
        emit_S_unit(c, 0, etiles)
        for u in range(len(units)):
            if u + 1 < len(units):
                emit_S_unit(c, u + 1, etiles)
            emit_O_unit(c, u, etiles, po_all)
            if layer == 2 or u % 2 == 1:
                feed_filler(c)
        # normalizers: stage denominator rows at partition 0 (custom-DVE
        # reciprocal misreads PSUM at base partition 64 on hardware), recip,
        # DRAM-bounce broadcast. The copies are split across ACT and DVE so
        # the serial tail chain pipelines across both engines.
        srow = work.tile([1, 2 * n_q], F32, tag="srow", bufs=2,
                         name=f"sr{c}")
        rr = work.tile([1, 2 * n_q], F32, tag="rr", bufs=2, name=f"rr{c}")
        (hb_e, h_e), (hb_o, h_o) = heads(c)
        nc.scalar.copy(srow[:, 0:n_q], po_all[h_e][64:65, 0:n_q])
        nc.vector.tensor_copy(srow[:, n_q:2 * n_q],
                              po_all[h_o][64:65, 0:n_q])
        nc.vector.reciprocal_approx_fast(out=rr[:, 0:n_q],
                                         in_=srow[:, 0:n_q])
        nc.vector.reciprocal_approx_fast(out=rr[:, n_q:2 * n_q],
                                         in_=srow[:, n_q:2 * n_q])
        nc.scalar.copy(ot_sb[hb_e, c, :], po_all[h_e][0:64, :n_q])
        nc.vector.tensor_copy(ot_sb[hb_o, c, :], po_all[h_o][0:64, :n_q])
        drp = cx.dram.tile([2, n_q], F32, tag="drp", name=f"drp{c}")
        nc.sync.dma_start(drp[0:1, :], rr[:, 0:n_q])
        nc.sync.dma_start(drp[1:2, :], rr[:, n_q:2 * n_q])
        bc = work.tile([P, n_q], F32, tag="attn_bc", name=f"bc{c}")
        nc.gpsimd.dma_start(bc[0:64, :], drp[0:1, :].to_broadcast([64, n_q]))
        nc.gpsimd.dma_start(bc[64:128, :],
                            drp[1:2, :].to_broadcast([64, n_q]))
        nc.vector.tensor_tensor(ot_sb[:, c, :], ot_sb[:, c, :], bc[:],
                                ALU.mult)
        feed_filler(c + 1)
    # flush remaining filler pieces (their deps are all satisfied now)
    if filler is not None:
        while fill_i[0] < len(filler):
            filler[fill_i[0]][1]()
            fill_i[0] += 1


def _r3(ap):
    """DRAM [K*128, n] -> [128(part), K, n] view for DMA."""
    return ap.rearrange("(ko p) s -> p ko s", p=P)


def _build_program(flags):
    use_bo1, use_bo2, use_fb1, use_fb2, use_g1, use_g2, use_g3 = flags
    nc = _Bacc("TRN2", target_bir_lowering=False, debug=False)

    def din(name, shape, dt=BF):
        return nc.dram_tensor(name, shape, dt, kind="ExternalInput").ap()

    x1T = din("x1T", [D, S1])
    x2T = din("x2T", [D, S2])
    x3T = din("x3T", [D, S3])
    wts = {n: din(n, [D, D]) for n in
           ("wq1T", "wk1T", "wv1T", "wo1T", "wq2T", "wk2T", "wv2T", "wo2T")}
    fw1T = din("fw1T", [D, DF])
    fw2T = din("fw2T", [DF, D])
    bo1 = din("bo1", [P, NCH]) if use_bo1 else None
    bo2 = din("bo2", [P, NCH]) if use_bo2 else None
    fb1 = din("fb1", [P, DF // P]) if use_fb1 else None
    fb2 = din("fb2", [P, NCH]) if use_fb2 else None
    gbd = {}
    for i, use in ((1, use_g1), (2, use_g2), (3, use_g3)):
        gbd[i] = (din(f"g{i}", [P, NCH]),
                  din(f"b{i}", [P, NCH])) if use else None
    yT = nc.dram_tensor("yT", [D, S1], F32, kind="ExternalOutput").ap()
    taps = {}
    if _DEBUG_TAPS:
        for tn in ("t_q1", "t_ot1", "t_y1", "t_ot2", "t_y2"):
            taps[tn] = nc.dram_tensor(tn, [D, S1], BF,
                                      kind="ExternalOutput").ap()

    def tap(name, src):
        if _DEBUG_TAPS:
            nc.sync.dma_start(_r3(taps[name]), src[:])

    with tile.TileContext(nc, pool_alloc_mode="queue") as tc:
        cx = _Ctx()
        cx.tc = tc
        with tc.tile_pool(name="sb", bufs=1) as sb, \
             tc.tile_pool(name="zsq", bufs=1) as zsq_pool, \
             tc.tile_pool(name="wpre", bufs=1) as wpre, \
             tc.tile_pool(name="dram", bufs=2, space="DRAM") as dram:
            cx.sb, cx.zsq_pool, cx.dram = sb, zsq_pool, dram

            ones_bf = sb.tile([P, 1], BF, tag="ones_bf")
            nc.vector.memset(ones_bf[:], 1.0)
            cx.ones_bf = ones_bf
            ones128 = sb.tile([1, P], BF, tag="ones128")
            nc.vector.memset(ones128[:], 1.0)
            cx.ones128 = ones128
            eps_sb = sb.tile([P, 1], F32, tag="eps")
            nc.vector.memset(eps_sb[:], EPS)
            cx.eps_sb = eps_sb

            def load_pm(ap, cols, tag):
                if ap is None:
                    return None
                t = sb.tile([P, cols], BF, tag=tag)
                nc.sync.dma_start(t[:], ap)
                return t

            bo1_sb = load_pm(bo1, NCH, "bo1")
            bo2_sb = load_pm(bo2, NCH, "bo2")
            fb1_sb = load_pm(fb1, DF // P, "fb1")
            fb2_sb = load_pm(fb2, NCH, "fb2")
            gb_sb = {}
            for i in (1, 2, 3):
                gb_sb[i] = None if gbd[i] is None else (
                    load_pm(gbd[i][0], NCH, f"g{i}"),
                    load_pm(gbd[i][1], NCH, f"b{i}"))

            y1_sb = sb.tile([P, NCH, S1], BF, tag="y1")
            y2_sb = sb.tile([P, NCH, S1], BF, tag="y2")

            def copy_cb(dst, eng):
                return lambda m, ps: eng(dst[:, m, :], ps)

            dve_copy = lambda out, ps: nc.vector.tensor_copy(out, ps)
            act_copy = (lambda out, ps: nc.scalar.copy(out, ps)) \
                if _USE_ACT_COPY else dve_copy

            def emit_v_proj(psum_pool, x_sb, wv_sb, v_sb, Jkv):
                nc.vector.memset(v_sb[:, :, :, 64:65], 1.0)
                for j in range(Jkv):
                    ps = psum_pool.tile([P, 1024], F32, tag="proj",
                                        name=f"vps{j}")
                    for ko in range(NCH):
                        nc.tensor.matmul(
                            ps[:, 0:D],
                            x_sb[:, ko, j * P:(j + 1) * P],
                            wv_sb[:, ko, :],
                            start=(ko == 0), stop=(ko == NCH - 1))
                    nc.vector.tensor_copy(
                        v_sb[:, j, :, 0:64],
                        ps[:, 0:D].rearrange("p (h v) -> p h v", h=H))

            # open order is reverse of close order (pool stack is LIFO)
            kv2 = tc.tile_pool(name="kv2", bufs=1)
            with kv2 as kv2p:
                x3_sb = kv2p.tile([P, NCH, S3], BF, tag="xkv")
                wk2_sb = kv2p.tile([P, NCH, D], BF, tag="wk")
                wv2_sb = kv2p.tile([P, NCH, D], BF, tag="wv")
                k2_sb = kv2p.tile([P, NCH, S3], BF, tag="k")
                v2_sb = kv2p.tile([P, S3 // P, H, 65], BF, tag="v")

                otp1 = contextlib.ExitStack()
                otp1p = otp1.enter_context(tc.tile_pool(name="otp1", bufs=1))
                x1_sb = otp1p.tile([P, NCH, S1], BF, tag="x1")
                ot_sb = otp1p.tile([P, NCH, S1], BF, tag="ot1")

                at1_ctx = contextlib.ExitStack()
                at1p = at1_ctx.enter_context(tc.tile_pool(name="at1", bufs=1))
                q_sb = at1p.tile([P, NCH, S1], BF, tag="q")
                k_sb = at1p.tile([P, NCH, S2], BF, tag="k")
                v_sb = at1p.tile([P, S2 // P, H, 65], BF, tag="v")

                kv1_ctx = contextlib.ExitStack()
                kv1p = kv1_ctx.enter_context(tc.tile_pool(name="kv1", bufs=1))
                wq_sb = kv1p.tile([P, NCH, D], BF, tag="wq")
                x2_sb = kv1p.tile([P, NCH, S2], BF, tag="xkv")
                wk_sb = kv1p.tile([P, NCH, D], BF, tag="wk")
                wv_sb = kv1p.tile([P, NCH, D], BF, tag="wv")
                # per-chunk DMAs in consumption order, issued across FOUR
                # engine DGE queues in parallel (descriptor generation is
                # ~1us serial per engine): the ko-major Q1/K1 projections
                # start as soon as their first chunks land
                for ko in range(NCH):
                    nc.sync.dma_start(wq_sb[:, ko, :],
                                      _r3(wts["wq1T"])[:, ko, :])
                    nc.gpsimd.dma_start(x1_sb[:, ko, :], _r3(x1T)[:, ko, :])
                for ko in range(NCH):
                    nc.scalar.dma_start(wk_sb[:, ko, :],
                                        _r3(wts["wk1T"])[:, ko, :])
                    nc.scalar.dma_start(x2_sb[:, ko, :], _r3(x2T)[:, ko, :])
                nc.sync.dma_start(wv_sb[:], _r3(wts["wv1T"]))
                nc.gpsimd.dma_start(x3_sb[:], _r3(x3T))
                nc.sync.dma_start(wk2_sb[:], _r3(wts["wk2T"]))
                nc.sync.dma_start(wv2_sb[:], _r3(wts["wv2T"]))

                # prefetch every later-phase weight now (bf16 fits in SBUF)
                wo1_sb = wpre.tile([P, NCH, D], BF, tag="wo1")
                wq2_sb = wpre.tile([P, NCH, D], BF, tag="wq2")
                wo2_sb = wpre.tile([P, NCH, D], BF, tag="wo2")
                fw1_sb = wpre.tile([P, NCH, DF], BF, tag="fw1")
                fw2_sb = wpre.tile([P, DF // P, D], BF, tag="fw2")
                nc.sync.dma_start(wo1_sb[:], _r3(wts["wo1T"]))
                nc.sync.dma_start(wq2_sb[:], _r3(wts["wq2T"]))
                nc.sync.dma_start(wo2_sb[:], _r3(wts["wo2T"]))
                nc.sync.dma_start(fw1_sb[:], _r3(fw1T))
                nc.sync.dma_start(fw2_sb[:], _r3(fw2T))

                # dense warm-up block: q1,k1,v1 (q1/k1 ko-major so the PE
                # starts on the first DMA'd chunks); k2/v2 are deferred into
                # the WO1/LN1 and WO2/LN2 phases as independent PE filler
                with tc.tile_pool(name="psA", bufs=4, space="PSUM") as psA:
                    _emit_proj(nc, psA, wq_sb, x1_sb, D, S1, NCH,
                               copy_cb(q_sb, act_copy), ko_major=True)
                    _emit_proj(nc, psA, wk_sb, x2_sb, D, S2, NCH,
                               copy_cb(k_sb, act_copy), ko_major=True)
                    emit_v_proj(psA, x2_sb, wv_sb, v_sb, S2 // P)
                kv1_ctx.close()  # frees x2 + wq1/wk1/wv1 SBUF

                # attention 1, with K2/V2 projection pieces as full-array
                # PE filler woven between j-groups (keeps the HAM clock warm
                # through the half-array attention matmuls)
                attn_fn = _emit_attn_base if _USE_BASE_ATTN else _emit_attn
                s_bufs = 4 if _USE_BASE_ATTN else 2
                nc.vector.memset(v2_sb[:, :, :, 64:65], 1.0)
                wo1_acc = otp1p.tile([P, NCH, S1], BF, tag="wo1acc")
                with tc.tile_pool(name="wk1w", bufs=3) as work, \
                     tc.tile_pool(name="ps_s1", bufs=s_bufs,
                                  space="PSUM") as pss, \
                     tc.tile_pool(name="ps_ot1", bufs=2,
                                  space="PSUM") as psot:
                    def mk_k2(m):
                        def f():
                            ps = pss.tile([P, 1024], F32, tag="s",
                                          name=f"fk2_{m}")
                            for ko in range(NCH):
                                for (a, b) in _regions(S3):
                                    nc.tensor.matmul(
                                        ps[:, a:b],
                                        wk2_sb[:, ko, m * P:(m + 1) * P],
                                        x3_sb[:, ko, a:b],
                                        start=(ko == 0),
                                        stop=(ko == NCH - 1))
                            nc.vector.tensor_copy(k2_sb[:, m, :],
                                                  ps[:, 0:S3])
                        return f

                    def mk_v2(j):
                        def f():
                            ps = pss.tile([P, 1024], F32, tag="s",
                                          name=f"fv2_{j}")
                            for ko in range(NCH):
                                nc.tensor.matmul(
                                    ps[:, 0:D],
                                    x3_sb[:, ko, j * P:(j + 1) * P],
                                    wv2_sb[:, ko, :],
                                    start=(ko == 0), stop=(ko == NCH - 1))
                            nc.vector.tensor_copy(
                                v2_sb[:, j, :, 0:64],
                                ps[:, 0:D].rearrange("p (h v) -> p h v",
                                                     h=H))
                        return f

                    def mk_wo(acc, w_sb, src_sb, m, ko, pool):
                        def f():
                            ps = pool.tile([P, 1024], F32, tag="s",
                                           name=f"fwo{m}_{ko}")
                            for (a, b) in _regions(S1):
                                nc.tensor.matmul(
                                    ps[:, a:b],
                                    w_sb[:, ko, m * P:(m + 1) * P],
                                    src_sb[:, ko, a:b],
                                    start=True, stop=True)
                            if ko == 0:
                                nc.vector.tensor_copy(acc[:, m, :],
                                                      ps[:, 0:S1])
                            else:
                                nc.vector.tensor_tensor(acc[:, m, :],
                                                        acc[:, m, :],
                                                        ps[:, 0:S1], ALU.add)
                        return f
                    cx.mk_wo = mk_wo

                    fillers = [(-1, mk_k2(m)) for m in range(NCH)]
                    fillers += [(-1, mk_v2(j)) for j in range(S3 // P)]
                    for ko in range(NCH):
                        fillers += [(ko, mk_wo(wo1_acc, wo1_sb, ot_sb, m,
                                               ko, pss)) for m in range(NCH)]
                    if _USE_BASE_ATTN:
                        attn_fn(nc, cx, work, pss, psot, q_sb, k_sb, v_sb,
                                ot_sb, 1, S1, S2)
                    else:
                        attn_fn(nc, cx, work, pss, psot, q_sb, k_sb, v_sb,
                                ot_sb, 1, S1, S2, filler=fillers)
                tap("t_q1", q_sb)
                tap("t_ot1", ot_sb)
                at1_ctx.close()  # frees q1/k1/v1 SBUF

                # LN1 (wo1 was accumulated into wo1_acc by attention fillers)
                with tc.tile_pool(name="psB1", bufs=2, space="PSUM") as psB:
                    _emit_ln(nc, cx, psB, lambda m: wo1_acc[:, m, :], x1_sb,
                             bo1_sb, y1_sb, y1_sb, S1, gb_sb[1])
                tap("t_y1", y1_sb)
                otp1.close()

                # q2 projection (ko-major: starts as soon as y1 chunk 0 is
                # normalized)
                otp2 = contextlib.ExitStack()
                otp2p = otp2.enter_context(tc.tile_pool(name="otp2", bufs=1))
                ot2_sb = otp2p.tile([P, NCH, S1], BF, tag="ot2")
                q2_sb = otp2p.tile([P, NCH, S1], BF, tag="q2")
                with tc.tile_pool(name="psC", bufs=4, space="PSUM") as psC:
                    _emit_proj(nc, psC, wq2_sb, y1_sb, D, S1, NCH,
                               copy_cb(q2_sb, act_copy), ko_major=_KO_MAJOR)

                # attention 2, with wo2 accumulation as gated filler
                wo2_acc = otp2p.tile([P, NCH, S1], BF, tag="wo2acc")
                with tc.tile_pool(name="wk2w", bufs=3) as work2, \
                     tc.tile_pool(name="ps_s2", bufs=s_bufs,
                                  space="PSUM") as pss2, \
                     tc.tile_pool(name="ps_ot2", bufs=2,
                                  space="PSUM") as psot2:
                    fillers2 = []
                    for ko in range(NCH):
                        fillers2 += [(ko, cx.mk_wo(wo2_acc, wo2_sb, ot2_sb,
                                                   m, ko, pss2))
                                     for m in range(NCH)]
                    attn_fn(nc, cx, work2, pss2, psot2, q2_sb, k2_sb,
                            v2_sb, ot2_sb, 2, S1, S3, filler=fillers2)
                tap("t_ot2", ot2_sb)

                # LN2
                with tc.tile_pool(name="psD", bufs=2, space="PSUM") as psD:
                    _emit_ln(nc, cx, psD, lambda m: wo2_acc[:, m, :], y1_sb,
                             bo2_sb, y2_sb, y2_sb, S1, gb_sb[2])
                tap("t_y2", y2_sb)
                otp2.close()

            # FFN + LN3
            zbuf = sb.tile([P, NCH, S1], BF, tag="y1")  # reuse y1 slot
            yT_sb = sb.tile([P, NCH, S1], F32, tag="yT")
            with tc.tile_pool(name="ffn1", bufs=1) as f1p:
                h_sb = f1p.tile([P, DF // P, S1], BF, tag="hT")
                with tc.tile_pool(name="psE", bufs=3, space="PSUM") as psE:
                    def gelu_consume(m, ps):
                        nc.scalar.activation(
                            h_sb[:, m, :], ps[:, 0:S1], AF.Gelu,
                            bias=(fb1_sb[:, m:m + 1]
                                  if fb1_sb is not None else 0.0),
                            scale=1.0)
                    # ko-major pairs: the first FFN1 matmuls need only y2
                    # chunk 0, starting inside LN2's normalize window
                    _emit_proj(nc, psE, fw1_sb, y2_sb, DF, S1, NCH,
                               gelu_consume, ko_major=True, m_group=2)

                with tc.tile_pool(name="psF", bufs=2, space="PSUM") as psF:
                    f2_ps = {}
                    _emit_proj(nc, psF, fw2_sb, h_sb, D, S1, DF // P,
                               lambda m, ps: f2_ps.__setitem__(m, ps))

                    def out_dma(m):
                        nc.sync.dma_start(_r3(yT)[:, m, :], yT_sb[:, m, :])

                    _emit_ln(nc, cx, psF, lambda m: f2_ps[m], y2_sb, fb2_sb,
                             zbuf, yT_sb, S1, gb_sb[3], out_cb=out_dma)

    nc.finalize()
    return nc


def _to_pm(vec, cols):
    """[cols*128] vector -> [128, cols] partition-major layout (bf16)."""
    return np.ascontiguousarray(vec.reshape(cols, P).T).astype(
        ml_dtypes.bfloat16)


def _bf(a):
    return np.ascontiguousarray(a).astype(ml_dtypes.bfloat16)


def kernel(**inputs):
    cords = np.asarray(inputs["cords_features"], np.float32)
    spatial = np.asarray(inputs["spatial_features"], np.float32)
    speed = np.asarray(inputs["speed_features"], np.float32)
    B = cords.shape[0]
    assert B == 8

    def g(name):
        return np.asarray(inputs[name], np.float32)

    flags = (
        not np.allclose(g("bo1"), 0), not np.allclose(g("bo2"), 0),
        not np.allclose(g("ffn_b1"), 0), not np.allclose(g("ffn_b2"), 0),
        not (np.allclose(g("ln1_g"), 1) and np.allclose(g("ln1_b"), 0)),
        not (np.allclose(g("ln2_g"), 1) and np.allclose(g("ln2_b"), 0)),
        not (np.allclose(g("ln3_g"), 1) and np.allclose(g("ln3_b"), 0)),
    )
    if flags not in _PROGRAM_CACHE:
        _PROGRAM_CACHE[flags] = _build_program(flags)
    nc = _PROGRAM_CACHE[flags]

    shared = {
        "wq1T": _bf(g("wq1").T), "wk1T": _bf(g("wk1").T),
        "wv1T": _bf(g("wv1").T), "wo1T": _bf(g("wo1").T),
        "wq2T": _bf(g("wq2").T), "wk2T": _bf(g("wk2").T),
        "wv2T": _bf(g("wv2").T), "wo2T": _bf(g("wo2").T),
        "fw1T": _bf(g("ffn_w1").T), "fw2T": _bf(g("ffn_w2").T),
    }
    use_bo1, use_bo2, use_fb1, use_fb2, use_g1, use_g2, use_g3 = flags
    if use_bo1:
        shared["bo1"] = _to_pm(g("bo1"), NCH)
    if use_bo2:
        shared["bo2"] = _to_pm(g("bo2"), NCH)
    if use_fb1:
        shared["fb1"] = _to_pm(g("ffn_b1"), DF // P)
    if use_fb2:
        shared["fb2"] = _to_pm(g("ffn_b2"), NCH)
    for i, use in ((1, use_g1), (2, use_g2), (3, use_g3)):
        if use:
            shared[f"g{i}"] = _to_pm(g(f"ln{i}_g"), NCH)
            shared[f"b{i}"] = _to_pm(g(f"ln{i}_b"), NCH)

    in_maps = []
    for b in range(B):
        m = dict(shared)
        m["x1T"] = _bf(cords[b].T)
        m["x2T"] = _bf(spatial[b].T)
        m["x3T"] = _bf(speed[b].T)
        in_maps.append(m)

    global _LAST_IN_MAPS
    _LAST_IN_MAPS = in_maps
    res = run_bass_kernel_spmd(nc, in_maps, core_ids=list(range(B)))
    out = np.stack([res.results[b]["yT"].T for b in range(B)], axis=0)
    return np.ascontiguousarray(out.astype(np.float32))


# revision 70
# speedup vs baseline: 1.1415x; 1.1415x over previous
"""DualCrossAttention Trainium2 kernel (bf16).

Data-parallel: batch=8 across 8 NeuronCores, one batch element per core.
Per core: two cross-attentions + FFN + 3 LayerNorms on [768, 512] activations.

Layout: feature-major activations (x.T: [feature(part), seq(free)]); weights
host-pre-transposed and cast to bf16 so every projection is a full-rate PE
matmul (bf16 streams 1 cycle/row vs 1.5 for fp32-HIGH, and enables FWL).
Attention: S.T = k_h @ q_h.T with the two heads of a pair emitted as adjacent
matmuls into PE row-groups 0/64 (concurrent execution) writing one shared
PSUM tile, so a single wide ACT exp call covers both heads; the wm scale is
folded into the exp affine. O.T accumulates with a ones-augmented V column so
the softmax denominator lands in PSUM row 64. All per-position normalizers
(attn 1/denominator, LN rstd/mean*rstd) are broadcast across partitions with
tiny K=1 PE matmuls into PSUM — no DRAM bounce. All weights are prefetched
at kernel start (bf16 halves the SBUF/DMA footprint).
"""
import contextlib

import numpy as np
import ml_dtypes

import concourse.bacc as bacc
import concourse.bass as bass
import concourse.tile as tile
from concourse import mybir
from concourse.bass_utils import run_bass_kernel_spmd
class _Bacc(bacc.Bacc):
    """Bacc with Exp/Ln pinned to the natural_log_exp_and_others ACT table
    set: the default chooser alternates between exp_and_others (attention
    softmax) and natural_log (LayerNorm rstd), paying a ~2.7us table load at
    every switch. Removing Exp/Ln from the single-function sets makes both
    resolve to the combined set, leaving only the Gelu switches."""

    def insert_act_table_loads(self):
        from concourse.hw_specs import get_activation_tables
        import bass_rust as _bass_rust
        has_activation = any(
            isinstance(i, mybir.InstActivation)
            for b in self.main_func.blocks
            for i in b.instructions
        )
        if not has_activation:
            return
        AFt = mybir.ActivationFunctionType
        tables = []
        for name, fns in get_activation_tables(self.m.arch).items():
            if name == "exp_and_others":
                fns = fns - {AFt.Exp}
            elif name == "natural_log":
                fns = fns - {AFt.Ln}
            tables.append((name, fns))
        _bass_rust.insert_act_table_loads(self, tables)


F32 = mybir.dt.float32
F32R = mybir.dt.float32r
BF = mybir.dt.bfloat16
AF = mybir.ActivationFunctionType
ALU = mybir.AluOpType

H, KD, VD = 8, 64, 64
D, DF = 512, 2048
S1, S2, S3 = 768, 1024, 768
P = 128
NCH = D // P            # 4 feature chunks of the 512-dim residual stream
W = 1.25                # wm weight scale
INV_SQRT = 0.125        # 1/sqrt(64)
EPS = 1e-5

_PROGRAM_CACHE = {}
_USE_BASE_ATTN = False
_USE_ACT_COPY = True
_KO_MAJOR = True
_DEBUG_TAPS = False


def _regions(n):
    """Split free dim n into <=512 column regions (PSUM-bank aligned)."""
    out = []
    s = 0
    while s < n:
        e = min(s + 512, n)
        out.append((s, e))
        s = e
    return out


class _Ctx:
    """Shared handles for the emit helpers."""
    pass


def _emit_proj(nc, psum_pool, wT_sb, xT_sb, n_out, n_seq, k_chunks, consume,
               ko_major=False, m_group=None, tag="proj"):
    """out.T[o, i] = sum_d wT[d, o] * xT[d, i]; calls consume(m, psum_ap).

    ko_major=True holds m-tiles live and loops ko outer / m inner, so the
    first matmuls only need xT chunk 0 (pipelines into a producer of xT).
    m_group limits how many m-tiles are live at once (PSUM pressure): the
    m-range is processed in groups, each group ko-major.
    """
    if ko_major:
        n_m = n_out // P
        if m_group is None:
            m_group = n_m
        for m0 in range(0, n_m, m_group):
            ms = range(m0, min(m0 + m_group, n_m))
            tiles = {m: psum_pool.tile([P, 1024], F32, tag=tag,
                                       name=f"pp{tag}{m}") for m in ms}
            for ko in range(k_chunks):
                for m in ms:
                    for (a, b) in _regions(n_seq):
                        nc.tensor.matmul(
                            tiles[m][:, a:b],
                            wT_sb[:, ko, m * P:(m + 1) * P],
                            xT_sb[:, ko, a:b],
                            start=(ko == 0), stop=(ko == k_chunks - 1),
                        )
            for m in ms:
                consume(m, tiles[m][:, :n_seq])
        return
    for m in range(n_out // P):
        ps = psum_pool.tile([P, 1024], F32, tag="proj", name=f"pp{m}")
        for ko in range(k_chunks):
            for (a, b) in _regions(n_seq):
                nc.tensor.matmul(
                    ps[:, a:b],
                    wT_sb[:, ko, m * P:(m + 1) * P],
                    xT_sb[:, ko, a:b],
                    start=(ko == 0), stop=(ko == k_chunks - 1),
                )
        consume(m, ps[:, :n_seq])


def _emit_ln(nc, cx, psum_pool, z_src, resid, bias_pm, z_sb, y_sb, n_seq,
             scale_gb, out_cb=None, bc_tags=("proj", "proj")):
    """LayerNorm over the feature axis (partitions x NCH chunks).

    z_src(m) -> psum AP [P, n_seq] (projection output chunk m);
    z = psum + bias + resid is built in z_sb (bf16); stats via ones-matmuls
    (partition reduction on PE); rstd via exp(-0.5 ln(var+eps)); rstd and
    mean*rstd are broadcast to [P, n_seq] with K=1 PE matmuls into recycled
    proj-tag PSUM slots; y_sb = z*rstd_bc - mr_bc (may alias z_sb).
    """
    sb = cx.sb
    stat_z = psum_pool.tile([1, 768], F32, tag="ln_stat_z", bufs=1)
    stat_zsq = psum_pool.tile([1, 768], F32, tag="ln_stat_zsq", bufs=1)
    for m in range(NCH):
        ps = z_src(m)
        if bias_pm is not None:
            nc.vector.scalar_tensor_tensor(
                z_sb[:, m, :], ps, bias_pm[:, m:m + 1], resid[:, m, :],
                op0=ALU.add, op1=ALU.add)
        else:
            nc.vector.tensor_tensor(z_sb[:, m, :], ps, resid[:, m, :], ALU.add)
        zsq = cx.zsq_pool.tile([P, 768], BF, tag="ln_zsq", name=f"zsq{m}")
        # last chunk's square on DVE (shorter critical path); others on the
        # otherwise-idle gpsimd
        sq_eng = nc.vector if m == NCH - 1 else nc.gpsimd
        sq_eng.tensor_tensor(zsq[:, :n_seq], z_sb[:, m, :], z_sb[:, m, :],
                             ALU.mult)
        for (a, b) in _regions(n_seq):
            nc.tensor.matmul(stat_z[0:1, a:b], cx.ones_bf[:, 0:1],
                             z_sb[:, m, a:b],
                             start=(m == 0), stop=(m == NCH - 1))
            nc.tensor.matmul(stat_zsq[0:1, a:b], cx.ones_bf[:, 0:1],
                             zsq[:, a:b],
                             start=(m == 0), stop=(m == NCH - 1))
    # small per-position vectors, all on partition 0
    # msq = (sum_z/D)^2 on ACT (Square is in the pinned table set);
    # var = sum(z^2)/D - msq -> ln(var+eps) -> rstd = exp(-0.5*ln);
    # mr = (sum_z/D)*rstd fused in one scalar_tensor_tensor
    msq = sb.tile([1, n_seq], F32, tag="ln_msq")
    nc.scalar.activation(msq[:], stat_z[0:1, :n_seq], AF.Square, bias=0.0,
                         scale=1.0 / D)
    rstd_t = sb.tile([1, n_seq], F32, tag="ln_rstd")
    nc.vector.scalar_tensor_tensor(rstd_t[:], stat_zsq[0:1, :n_seq], 1.0 / D,
                                   msq[:], op0=ALU.mult, op1=ALU.subtract)
    nc.scalar.activation(rstd_t[:], rstd_t[:], AF.Ln, bias=cx.eps_sb[0:1, :],
                         scale=1.0)
    lnv = sb.tile([1, 2 * n_seq], BF, tag="lnv")
    nc.scalar.activation(lnv[:, 0:n_seq], rstd_t[:], AF.Exp, bias=0.0,
                         scale=-0.5)
    nc.vector.scalar_tensor_tensor(lnv[:, n_seq:2 * n_seq],
                                   stat_z[0:1, :n_seq], 1.0 / D,
                                   lnv[:, 0:n_seq], op0=ALU.mult,
                                   op1=ALU.mult)
    # broadcast rstd / mean*rstd across partitions with K=1 matmuls into
    # recycled proj-tag PSUM slots (no DRAM bounce)
    rstd_bc = psum_pool.tile([P, 1024], F32, tag=bc_tags[0], name="rstdbc")
    mr_bc = psum_pool.tile([P, 1024], F32, tag=bc_tags[1], name="mrbc")
    for (a, b) in _regions(n_seq):
        nc.tensor.matmul(rstd_bc[:, a:b], cx.ones128[:, :], lnv[0:1, a:b],
                         start=True, stop=True)
        nc.tensor.matmul(mr_bc[:, a:b], cx.ones128[:, :],
                         lnv[0:1, n_seq + a:n_seq + b], start=True, stop=True)
    for m in range(NCH):
        nc.vector.tensor_tensor(y_sb[:, m, :], z_sb[:, m, :],
                                rstd_bc[:, :n_seq], ALU.mult)
        nc.vector.tensor_tensor(y_sb[:, m, :], y_sb[:, m, :],
                                mr_bc[:, :n_seq], ALU.subtract)
        if scale_gb is not None:
            g_sb, b_sb = scale_gb
            nc.vector.tensor_scalar(
                y_sb[:, m, :], y_sb[:, m, :],
                g_sb[:, m:m + 1], b_sb[:, m:m + 1], op0=ALU.mult, op1=ALU.add)
        if out_cb is not None:
            out_cb(m)


def _attn_units(layer, n_kv):
    """Unit list: (j_list, qa, qb, exp_scale). One exp call per unit covers
    both heads of the pair (and both j's of a pair for layer 2)."""
    J = n_kv // P
    units = []
    if layer == 1:
        # wm1: (q<512, kv<512) and (q>=512, kv>=512) get W
        for j in range(J):
            lo = W * INV_SQRT if j * P < 512 else INV_SQRT
            hi = INV_SQRT if j * P < 512 else W * INV_SQRT
            units.append(([j], 0, 512, lo))
            units.append(([j], 512, 768, hi))
    else:
        # wm2: diagonal 256-blocks get W; kv pair jp covers block jp
        for jp in range(J // 2):
            for b in range(3):
                sc = W * INV_SQRT if b == jp else INV_SQRT
                units.append(([2 * jp, 2 * jp + 1], 256 * b, 256 * (b + 1),
                              sc))
    return units


def _exp_slices(layer, j, n_q):
    """Per (attention layer, key-chunk j): (col_lo, col_hi, exp scale)."""
    if layer == 1:
        jlo = j * P < 512
        s_lo = W * INV_SQRT if jlo else INV_SQRT
        s_hi = INV_SQRT if jlo else W * INV_SQRT
        return [(0, 512, s_lo), (512, n_q, s_hi)]
    blk = j // 2
    raw = [(b * 256, min((b + 1) * 256, n_q),
            W * INV_SQRT if b == blk else INV_SQRT) for b in range(3)]
    out = [raw[0]]
    for (lo, hi, sc) in raw[1:]:
        plo, phi, psc = out[-1]
        if sc == psc and lo == phi:
            out[-1] = (plo, hi, sc)
        else:
            out.append((lo, hi, sc))
    return out


def _emit_attn_base(nc, cx, work, psum_s, psum_ot, qT_sb, kT_sb, v_sb, ot_sb,
                    layer, n_q, n_kv):
    """Baseline-style attention: per-head S psum tiles + sliced exp."""
    J = n_kv // P
    heads = lambda c: ((slice(0, 64), 2 * c), (slice(64, 128), 2 * c + 1))
    units = [(j, a, b) for j in range(J) for (a, b) in _regions(n_q)]

    def emit_S_unit(c, u, etiles):
        (hb_e, h_e), (hb_o, h_o) = heads(c)
        j, a, b = units[u]
        if j not in etiles:
            etiles[j] = (
                work.tile([P, n_q], BF, tag="exps", bufs=4, name=f"ee{c}_{j}"),
                work.tile([P, n_q], BF, tag="exps", bufs=4, name=f"eo{c}_{j}"),
            )
        e_e, e_o = etiles[j]
        ps_e = psum_s.tile([P, 512], F32, tag="s", name=f"se{c}_{j}_{a}")
        ps_o = psum_s.tile([P, 512], F32, tag="s", name=f"so{c}_{j}_{a}")
        nc.tensor.matmul(ps_e[:, :b - a],
                         kT_sb[hb_e, c, j * P:(j + 1) * P],
                         qT_sb[hb_e, c, a:b], start=True, stop=True)
        nc.tensor.matmul(ps_o[:, :b - a],
                         kT_sb[hb_o, c, j * P:(j + 1) * P],
                         qT_sb[hb_o, c, a:b], start=True, stop=True)
        for e, ps in ((e_e, ps_e), (e_o, ps_o)):
            for (lo, hi, sc) in _exp_slices(layer, j, n_q):
                lo2, hi2 = max(lo, a), min(hi, b)
                if lo2 < hi2:
                    nc.scalar.activation(
                        e[:, lo2:hi2], ps[:, lo2 - a:hi2 - a],
                        AF.Exp, bias=0.0, scale=sc)

    def emit_O_unit(c, u, etiles, po_all):
        j, a, b = units[u]
        e_e, e_o = etiles[j]
        for (hb, h), e in zip(heads(c), (e_e, e_o)):
            nc.tensor.matmul(
                po_all[h][0:65, a:b],
                v_sb[:, j, h, 0:65],
                e[:, a:b],
                start=(j == 0), stop=(j == J - 1))

    for c in range(H // 2):
        po_all = {}
        for hb, h in heads(c):
            po_all[h] = psum_ot.tile([65, 768], F32, tag="ot", name=f"po{h}")
        etiles = {}
        emit_S_unit(c, 0, etiles)
        for u in range(len(units)):
            if u + 1 < len(units):
                emit_S_unit(c, u + 1, etiles)
            emit_O_unit(c, u, etiles, po_all)
        srow = work.tile([1, 2 * n_q], F32, tag="srow", bufs=2,
                         name=f"sr{c}")
        rr = work.tile([1, 2 * n_q], F32, tag="rr", bufs=2, name=f"rr{c}")
        (hb_e, h_e), (hb_o, h_o) = heads(c)
        # stage denominator rows at partition 0 in SBUF: the custom-DVE
        # reciprocal misreads PSUM at base partition 64 on hardware
        nc.vector.tensor_copy(srow[:, 0:n_q], po_all[h_e][64:65, 0:n_q])
        nc.vector.tensor_copy(srow[:, n_q:2 * n_q],
                              po_all[h_o][64:65, 0:n_q])
        nc.vector.reciprocal_approx_fast(out=rr[:, 0:n_q],
                                         in_=srow[:, 0:n_q])
        nc.vector.reciprocal_approx_fast(out=rr[:, n_q:2 * n_q],
                                         in_=srow[:, n_q:2 * n_q])
        nc.scalar.copy(ot_sb[hb_e, c, :], po_all[h_e][0:64, :n_q])
        nc.scalar.copy(ot_sb[hb_o, c, :], po_all[h_o][0:64, :n_q])
        drp = cx.dram.tile([2, n_q], F32, tag="drp", name=f"drp{c}")
        nc.sync.dma_start(drp[0:1, :], rr[:, 0:n_q])
        nc.sync.dma_start(drp[1:2, :], rr[:, n_q:2 * n_q])
        bc = work.tile([P, n_q], F32, tag="attn_bc", name=f"bc{c}")
        nc.gpsimd.dma_start(bc[0:64, :], drp[0:1, :].to_broadcast([64, n_q]))
        nc.gpsimd.dma_start(bc[64:128, :],
                            drp[1:2, :].to_broadcast([64, n_q]))
        nc.vector.tensor_tensor(ot_sb[:, c, :], ot_sb[:, c, :], bc[:],
                                ALU.mult)


def _emit_attn(nc, cx, work, psum_s, psum_ot, qT_sb, kT_sb, v_sb, ot_sb,
               layer, n_q, n_kv, filler=None):
    """Cross-attention. Per unit: the heads of a pair (x kv-pair for layer 2)
    are emitted as ADJACENT K=64 matmuls into PE row-groups 0/64 (concurrent)
    writing one shared PSUM tile; ONE ACT exp call (wm scale folded in)
    covers the whole tile. O.T accumulates per unit with a ones-augmented V
    column so the softmax denominator lands in PSUM row 64. S for unit u+1 is
    emitted before O of unit u so the PE has independent work while ACT
    computes exp.

    filler: optional per-pair list of callables emitting independent
    full-array PE work (borrowing an s-tag PSUM slot). Attention's K=64 /
    65-row matmuls only half-use the PE array and never re-warm the HAM
    clock gate; dense filler matmuls keep it at 2.4 GHz."""
    J = n_kv // P
    units = _attn_units(layer, n_kv)
    heads = lambda c: ((slice(0, 64), 2 * c), (slice(64, 128), 2 * c + 1))

    def ekey(jl):
        return jl[0]

    def emit_S_unit(c, u, etiles):
        jl, a, b, sc = units[u]
        w = b - a
        nj = len(jl)
        if ekey(jl) not in etiles:
            etiles[ekey(jl)] = work.tile([P, nj * 2, n_q], BF, tag="exps",
                                         bufs=3, name=f"e{c}_{ekey(jl)}")
        E = etiles[ekey(jl)]
        # Bank-safe layout: all head-even S in bank 0 ([0:512]), head-odd in
        # bank 1 ([512:1024]) — the two heads' matmuls execute CONCURRENTLY
        # via PE row-groups 0/64, and concurrent drains into the SAME PSUM
        # bank are a hardware fault. u-order (all-he, then all-ho) keeps the
        # psum stride regular (w) so one strided ACT exp covers the tile.
        ps = psum_s.tile([P, 1024], F32, tag="s", name=f"s{c}_{u}")
        (hb_e, h_e), (hb_o, h_o) = heads(c)
        for ji, j in enumerate(jl):
            nc.tensor.matmul(ps[:, ji * w:(ji + 1) * w],
                             kT_sb[hb_e, c, j * P:(j + 1) * P],
                             qT_sb[hb_e, c, a:b], start=True, stop=True)
            nc.tensor.matmul(ps[:, 512 + ji * w:512 + (ji + 1) * w],
                             kT_sb[hb_o, c, j * P:(j + 1) * P],
                             qT_sb[hb_o, c, a:b], start=True, stop=True)
        ps_g = ps[:, 0:1024].rearrange("p (g q) -> p g q", g=2)
        if nj == 1:
            nc.scalar.activation(E[:, :, a:b], ps_g[:, :, 0:w],
                                 AF.Exp, bias=0.0, scale=sc)
        else:
            nc.scalar.activation(
                E[:, :, a:b].rearrange("p (g j) q -> p g j q", g=2),
                ps_g[:, :, 0:nj * w].rearrange("p g (j q) -> p g j q", j=nj),
                AF.Exp, bias=0.0, scale=sc)

    def emit_O_unit(c, u, etiles, po_all):
        jl, a, b, sc = units[u]
        nj = len(jl)
        E = etiles[ekey(jl)]
        # start/stop must be unique per PSUM BANK (start=True clears the
        # whole bank's has_written bits): only the first/last matmul touching
        # a bank carries the flag; sibling regions in the same bank rely on
        # per-element overwrite-then-accumulate semantics.
        first_in_bank = a % 512 == 0
        last_in_bank = (b % 512 == 0) or (b == n_q)
        for ji, j in enumerate(jl):
            for hi, (hb, h) in enumerate(heads(c)):
                nc.tensor.matmul(
                    po_all[h][0:65, a:b],
                    v_sb[:, j, h, 0:65],
                    E[:, hi * nj + ji, a:b],
                    start=(j == 0 and first_in_bank),
                    stop=(j == J - 1 and last_in_bank))

    fill_i = [0]

    def feed_filler(c):
        # filler entries are (gate, fn): fn may only be emitted once pair
        # `gate` has been normalized (so a stalled piece never parks on an
        # s-slot the attention pipeline needs)
        if filler is not None and fill_i[0] < len(filler):
            gate, fn = filler[fill_i[0]]
            if gate < c:
                fn()
                fill_i[0] += 1

    for c in range(H // 2):
        po_all = {}
        for hb, h in heads(c):
            po_all[h] = psum_ot.tile([65, 768], F32, tag="ot", name=f"po{h}")
        etiles = {}
        emit_S_unit(c, 0, etiles)
        for u in range(len(units)):
            if u + 1 < len(units):
                emit_S_unit(c, u + 1, etiles)
            emit_O_unit(c, u, etiles, po_all)
            if layer == 2 or u % 2 == 1:
                feed_filler(c)
        # normalizers: stage denominator rows at partition 0 (custom-DVE
        # reciprocal misreads PSUM at base partition 64 on hardware), recip,
        # DRAM-bounce broadcast. The copies are split across ACT and DVE so
        # the serial tail chain pipelines across both engines.
        srow = work.tile([1, 2 * n_q], F32, tag="srow", bufs=2,
                         name=f"sr{c}")
        rr = work.tile([1, 2 * n_q], F32, tag="rr", bufs=2, name=f"rr{c}")
        (hb_e, h_e), (hb_o, h_o) = heads(c)
        nc.scalar.copy(srow[:, 0:n_q], po_all[h_e][64:65, 0:n_q])
        nc.vector.tensor_copy(srow[:, n_q:2 * n_q],
                              po_all[h_o][64:65, 0:n_q])
        nc.vector.reciprocal_approx_fast(out=rr[:, 0:n_q],
                                         in_=srow[:, 0:n_q])
        nc.vector.reciprocal_approx_fast(out=rr[:, n_q:2 * n_q],
                                         in_=srow[:, n_q:2 * n_q])
        nc.scalar.copy(ot_sb[hb_e, c, :], po_all[h_e][0:64, :n_q])
        nc.vector.tensor_copy(ot_sb[hb_o, c, :], po_all[h_o][0:64, :n_q])
        drp = cx.dram.tile([2, n_q], F32, tag="drp", name=f"drp{c}")
        nc.sync.dma_start(drp[0:1, :], rr[:, 0:n_q])
        nc.sync.dma_start(drp[1:2, :], rr[:, n_q:2 * n_q])
        bc = work.tile([P, n_q], F32, tag="attn_bc", name=f"bc{c}")
        nc.gpsimd.dma_start(bc[0:64, :], drp[0:1, :].to_broadcast([64, n_q]))
        nc.gpsimd.dma_start(bc[64:128, :],
                            drp[1:2, :].to_broadcast([64, n_q]))
        nc.vector.tensor_tensor(ot_sb[:, c, :], ot_sb[:, c, :], bc[:],
                                ALU.mult)
        feed_filler(c + 1)
    # flush remaining filler pieces (their deps are all satisfied now)
    if filler is not None:
        while fill_i[0] < len(filler):
            filler[fill_i[0]][1]()
            fill_i[0] += 1


def _r3(ap):
    """DRAM [K*128, n] -> [128(part), K, n] view for DMA."""
    return ap.rearrange("(ko p) s -> p ko s", p=P)


def _build_program(flags):
    use_bo1, use_bo2, use_fb1, use_fb2, use_g1, use_g2, use_g3 = flags
    nc = _Bacc("TRN2", target_bir_lowering=False, debug=False)

    def din(name, shape, dt=BF):
        return nc.dram_tensor(name, shape, dt, kind="ExternalInput").ap()

    x1T = din("x1T", [D, S1])
    x2T = din("x2T", [D, S2])
    x3T = din("x3T", [D, S3])
    wts = {n: din(n, [D, D]) for n in
           ("wq1T", "wk1T", "wv1T", "wo1T", "wq2T", "wk2T", "wv2T", "wo2T")}
    fw1T = din("fw1T", [D, DF])
    fw2T = din("fw2T", [DF, D])
    bo1 = din("bo1", [P, NCH]) if use_bo1 else None
    bo2 = din("bo2", [P, NCH]) if use_bo2 else None
    fb1 = din("fb1", [P, DF // P]) if use_fb1 else None
    fb2 = din("fb2", [P, NCH]) if use_fb2 else None
    gbd = {}
    for i, use in ((1, use_g1), (2, use_g2), (3, use_g3)):
        gbd[i] = (din(f"g{i}", [P, NCH]),
                  din(f"b{i}", [P, NCH])) if use else None
    yT = nc.dram_tensor("yT", [D, S1], F32, kind="ExternalOutput").ap()
    taps = {}
    if _DEBUG_TAPS:
        for tn in ("t_q1", "t_ot1", "t_y1", "t_ot2", "t_y2"):
            taps[tn] = nc.dram_tensor(tn, [D, S1], BF,
                                      kind="ExternalOutput").ap()

    def tap(name, src):
        if _DEBUG_TAPS:
            nc.sync.dma_start(_r3(taps[name]), src[:])

    with tile.TileContext(nc, pool_alloc_mode="queue") as tc:
        cx = _Ctx()
        cx.tc = tc
        with tc.tile_pool(name="sb", bufs=1) as sb, \
             tc.tile_pool(name="zsq", bufs=1) as zsq_pool, \
             tc.tile_pool(name="wpre", bufs=1) as wpre, \
             tc.tile_pool(name="dram", bufs=2, space="DRAM") as dram:
            cx.sb, cx.zsq_pool, cx.dram = sb, zsq_pool, dram

            ones_bf = sb.tile([P, 1], BF, tag="ones_bf")
            nc.vector.memset(ones_bf[:], 1.0)
            cx.ones_bf = ones_bf
            ones128 = sb.tile([1, P], BF, tag="ones128")
            nc.vector.memset(ones128[:], 1.0)
            cx.ones128 = ones128
            eps_sb = sb.tile([P, 1], F32, tag="eps")
            nc.vector.memset(eps_sb[:], EPS)
            cx.eps_sb = eps_sb

            def load_pm(ap, cols, tag):
                if ap is None:
                    return None
                t = sb.tile([P, cols], BF, tag=tag)
                nc.sync.dma_start(t[:], ap)
                return t

            bo1_sb = load_pm(bo1, NCH, "bo1")
            bo2_sb = load_pm(bo2, NCH, "bo2")
            fb1_sb = load_pm(fb1, DF // P, "fb1")
            fb2_sb = load_pm(fb2, NCH, "fb2")
            gb_sb = {}
            for i in (1, 2, 3):
                gb_sb[i] = None if gbd[i] is None else (
                    load_pm(gbd[i][0], NCH, f"g{i}"),
                    load_pm(gbd[i][1], NCH, f"b{i}"))

            y1_sb = sb.tile([P, NCH, S1], BF, tag="y1")
            y2_sb = sb.tile([P, NCH, S1], BF, tag="y2")

            def copy_cb(dst, eng):
                return lambda m, ps: eng(dst[:, m, :], ps)

            dve_copy = lambda out, ps: nc.vector.tensor_copy(out, ps)
            act_copy = (lambda out, ps: nc.scalar.copy(out, ps)) \
                if _USE_ACT_COPY else dve_copy

            def emit_v_proj(psum_pool, x_sb, wv_sb, v_sb, Jkv):
                nc.vector.memset(v_sb[:, :, :, 64:65], 1.0)
                for j in range(Jkv):
                    ps = psum_pool.tile([P, 1024], F32, tag="proj",
                                        name=f"vps{j}")
                    for ko in range(NCH):
                        nc.tensor.matmul(
                            ps[:, 0:D],
                            x_sb[:, ko, j * P:(j + 1) * P],
                            wv_sb[:, ko, :],
                            start=(ko == 0), stop=(ko == NCH - 1))
                    nc.vector.tensor_copy(
                        v_sb[:, j, :, 0:64],
                        ps[:, 0:D].rearrange("p (h v) -> p h v", h=H))

            # open order is reverse of close order (pool stack is LIFO)
            kv2 = tc.tile_pool(name="kv2", bufs=1)
            with kv2 as kv2p:
                x3_sb = kv2p.tile([P, NCH, S3], BF, tag="xkv")
                wk2_sb = kv2p.tile([P, NCH, D], BF, tag="wk")
                wv2_sb = kv2p.tile([P, NCH, D], BF, tag="wv")
                k2_sb = kv2p.tile([P, NCH, S3], BF, tag="k")
                v2_sb = kv2p.tile([P, S3 // P, H, 65], BF, tag="v")

                otp1 = contextlib.ExitStack()
                otp1p = otp1.enter_context(tc.tile_pool(name="otp1", bufs=1))
                x1_sb = otp1p.tile([P, NCH, S1], BF, tag="x1")
                ot_sb = otp1p.tile([P, NCH, S1], BF, tag="ot1")

                at1_ctx = contextlib.ExitStack()
                at1p = at1_ctx.enter_context(tc.tile_pool(name="at1", bufs=1))
                q_sb = at1p.tile([P, NCH, S1], BF, tag="q")
                k_sb = at1p.tile([P, NCH, S2], BF, tag="k")
                v_sb = at1p.tile([P, S2 // P, H, 65], BF, tag="v")

                kv1_ctx = contextlib.ExitStack()
                kv1p = kv1_ctx.enter_context(tc.tile_pool(name="kv1", bufs=1))
                wq_sb = kv1p.tile([P, NCH, D], BF, tag="wq")
                x2_sb = kv1p.tile([P, NCH, S2], BF, tag="xkv")
                wk_sb = kv1p.tile([P, NCH, D], BF, tag="wk")
                wv_sb = kv1p.tile([P, NCH, D], BF, tag="wv")
                # per-chunk DMAs in consumption order, issued across FOUR
                # engine DGE queues in parallel (descriptor generation is
                # ~1us serial per engine): the ko-major Q1/K1 projections
                # start as soon as their first chunks land
                for ko in range(NCH):
                    nc.sync.dma_start(wq_sb[:, ko, :],
                                      _r3(wts["wq1T"])[:, ko, :])
                    nc.gpsimd.dma_start(x1_sb[:, ko, :], _r3(x1T)[:, ko, :])
                for ko in range(NCH):
                    nc.scalar.dma_start(wk_sb[:, ko, :],
                                        _r3(wts["wk1T"])[:, ko, :])
                    nc.scalar.dma_start(x2_sb[:, ko, :], _r3(x2T)[:, ko, :])
                nc.sync.dma_start(wv_sb[:], _r3(wts["wv1T"]))
                nc.gpsimd.dma_start(x3_sb[:], _r3(x3T))
                nc.sync.dma_start(wk2_sb[:], _r3(wts["wk2T"]))
                nc.sync.dma_start(wv2_sb[:], _r3(wts["wv2T"]))

                # prefetch every later-phase weight now (bf16 fits in SBUF)
                wo1_sb = wpre.tile([P, NCH, D], BF, tag="wo1")
                wq2_sb = wpre.tile([P, NCH, D], BF, tag="wq2")
                wo2_sb = wpre.tile([P, NCH, D], BF, tag="wo2")
                fw1_sb = wpre.tile([P, NCH, DF], BF, tag="fw1")
                fw2_sb = wpre.tile([P, DF // P, D], BF, tag="fw2")
                nc.sync.dma_start(wo1_sb[:], _r3(wts["wo1T"]))
                nc.sync.dma_start(wq2_sb[:], _r3(wts["wq2T"]))
                nc.sync.dma_start(wo2_sb[:], _r3(wts["wo2T"]))
                nc.sync.dma_start(fw1_sb[:], _r3(fw1T))
                nc.sync.dma_start(fw2_sb[:], _r3(fw2T))

                # dense warm-up block: q1,k1,v1 (q1/k1 ko-major so the PE
                # starts on the first DMA'd chunks); k2/v2 are deferred into
                # the WO1/LN1 and WO2/LN2 phases as independent PE filler
                with tc.tile_pool(name="psA", bufs=4, space="PSUM") as psA:
                    _emit_proj(nc, psA, wq_sb, x1_sb, D, S1, NCH,
                               copy_cb(q_sb, act_copy), ko_major=True)
                    _emit_proj(nc, psA, wk_sb, x2_sb, D, S2, NCH,
                               copy_cb(k_sb, act_copy), ko_major=True)
                    emit_v_proj(psA, x2_sb, wv_sb, v_sb, S2 // P)
                kv1_ctx.close()  # frees x2 + wq1/wk1/wv1 SBUF

                # attention 1, with K2/V2 projection pieces as full-array
                # PE filler woven between j-groups (keeps the HAM clock warm
                # through the half-array attention matmuls)
                attn_fn = _emit_attn_base if _USE_BASE_ATTN else _emit_attn
                s_bufs = 4 if _USE_BASE_ATTN else 2
                nc.vector.memset(v2_sb[:, :, :, 64:65], 1.0)
                with tc.tile_pool(name="wk1w", bufs=3) as work, \
                     tc.tile_pool(name="ps_s1", bufs=s_bufs,
                                  space="PSUM") as pss, \
                     tc.tile_pool(name="ps_ot1", bufs=2,
                                  space="PSUM") as psot:
                    def mk_k2(m, a, b):
                        def f():
                            ps = pss.tile([P, 1024], F32, tag="s",
                                          name=f"fk2_{m}_{a}")
                            for ko in range(NCH):
                                nc.tensor.matmul(
                                    ps[:, 0:b - a],
                                    wk2_sb[:, ko, m * P:(m + 1) * P],
                                    x3_sb[:, ko, a:b],
                                    start=(ko == 0), stop=(ko == NCH - 1))
                            nc.vector.tensor_copy(k2_sb[:, m, a:b],
                                                  ps[:, 0:b - a])
                        return f

                    def mk_v2(j, half):
                        def f():
                            ps = pss.tile([P, 1024], F32, tag="s",
                                          name=f"fv2_{j}_{half}")
                            a = half * 256
                            for ko in range(NCH):
                                nc.tensor.matmul(
                                    ps[:, 0:256],
                                    x3_sb[:, ko, j * P:(j + 1) * P],
                                    wv2_sb[:, ko, a:a + 256],
                                    start=(ko == 0), stop=(ko == NCH - 1))
                            nc.vector.tensor_copy(
                                v2_sb[:, j, 4 * half:4 * half + 4, 0:64],
                                ps[:, 0:256].rearrange("p (h v) -> p h v",
                                                       h=4))
                        return f

                    fillers = []
                    for m in range(NCH):
                        fillers.append((-1, mk_k2(m, 0, 256)))
                        fillers.append((-1, mk_k2(m, 256, 512)))
                        fillers.append((-1, mk_k2(m, 512, 768)))
                    for j in range(S3 // P):
                        fillers.append((-1, mk_v2(j, 0)))
                        fillers.append((-1, mk_v2(j, 1)))
                    if _USE_BASE_ATTN:
                        attn_fn(nc, cx, work, pss, psot, q_sb, k_sb, v_sb,
                                ot_sb, 1, S1, S2)
                    else:
                        attn_fn(nc, cx, work, pss, psot, q_sb, k_sb, v_sb,
                                ot_sb, 1, S1, S2, filler=fillers)
                tap("t_q1", q_sb)
                tap("t_ot1", ot_sb)
                at1_ctx.close()  # frees q1/k1/v1 SBUF

                # wo1 + LN1 (wo1 ko-major in 2-m groups: the first WO matmuls
                # need only attention pair 0's normalized output)
                with tc.tile_pool(name="psB1", bufs=2, space="PSUM") as psB:
                    wo_ps = {}
                    _emit_proj(nc, psB, wo1_sb, ot_sb, D, S1, NCH,
                               lambda m, ps: wo_ps.__setitem__(m, ps),
                               ko_major=True, m_group=2)
                    _emit_ln(nc, cx, psB, lambda m: wo_ps[m], x1_sb,
                             bo1_sb, y1_sb, y1_sb, S1, gb_sb[1])
                tap("t_y1", y1_sb)
                otp1.close()

                # q2 projection (ko-major: starts as soon as y1 chunk 0 is
                # normalized)
                otp2 = contextlib.ExitStack()
                otp2p = otp2.enter_context(tc.tile_pool(name="otp2", bufs=1))
                ot2_sb = otp2p.tile([P, NCH, S1], BF, tag="ot2")
                q2_sb = otp2p.tile([P, NCH, S1], BF, tag="q2")
                with tc.tile_pool(name="psC", bufs=4, space="PSUM") as psC:
                    _emit_proj(nc, psC, wq2_sb, y1_sb, D, S1, NCH,
                               copy_cb(q2_sb, act_copy), ko_major=_KO_MAJOR)

                # attention 2
                with tc.tile_pool(name="wk2w", bufs=3) as work2, \
                     tc.tile_pool(name="ps_s2", bufs=s_bufs,
                                  space="PSUM") as pss2, \
                     tc.tile_pool(name="ps_ot2", bufs=2,
                                  space="PSUM") as psot2:
                    attn_fn(nc, cx, work2, pss2, psot2, q2_sb, k2_sb,
                            v2_sb, ot2_sb, 2, S1, S3)
                tap("t_ot2", ot2_sb)

                # wo2 + LN2
                with tc.tile_pool(name="psD", bufs=2, space="PSUM") as psD:
                    wo2_ps = {}
                    _emit_proj(nc, psD, wo2_sb, ot2_sb, D, S1, NCH,
                               lambda m, ps: wo2_ps.__setitem__(m, ps),
                               ko_major=True, m_group=2)
                    _emit_ln(nc, cx, psD, lambda m: wo2_ps[m], y1_sb,
                             bo2_sb, y2_sb, y2_sb, S1, gb_sb[2])
                tap("t_y2", y2_sb)
                otp2.close()

            # FFN + LN3
            zbuf = sb.tile([P, NCH, S1], BF, tag="y1")  # reuse y1 slot
            yT_sb = sb.tile([P, NCH, S1], F32, tag="yT")
            with tc.tile_pool(name="ffn1", bufs=1) as f1p:
                h_sb = f1p.tile([P, DF // P, S1], BF, tag="hT")
                with tc.tile_pool(name="psE", bufs=3, space="PSUM") as psE:
                    def gelu_consume(m, ps):
                        nc.scalar.activation(
                            h_sb[:, m, :], ps[:, 0:S1], AF.Gelu,
                            bias=(fb1_sb[:, m:m + 1]
                                  if fb1_sb is not None else 0.0),
                            scale=1.0)
                    # ko-major pairs: the first FFN1 matmuls need only y2
                    # chunk 0, starting inside LN2's normalize window
                    _emit_proj(nc, psE, fw1_sb, y2_sb, DF, S1, NCH,
                               gelu_consume, ko_major=True, m_group=2)

                with tc.tile_pool(name="psF", bufs=2, space="PSUM") as psF:
                    f2_ps = {}
                    _emit_proj(nc, psF, fw2_sb, h_sb, D, S1, DF // P,
                               lambda m, ps: f2_ps.__setitem__(m, ps))

                    def out_dma(m):
                        nc.sync.dma_start(_r3(yT)[:, m, :], yT_sb[:, m, :])

                    _emit_ln(nc, cx, psF, lambda m: f2_ps[m], y2_sb, fb2_sb,
                             zbuf, yT_sb, S1, gb_sb[3], out_cb=out_dma)

    nc.finalize()
    return nc


def _to_pm(vec, cols):
    """[cols*128] vector -> [128, cols] partition-major layout (bf16)."""
    return np.ascontiguousarray(vec.reshape(cols, P).T).astype(
        ml_dtypes.bfloat16)


def _bf(a):
    return np.ascontiguousarray(a).astype(ml_dtypes.bfloat16)


def kernel(**inputs):
    cords = np.asarray(inputs["cords_features"], np.float32)
    spatial = np.asarray(inputs["spatial_features"], np.float32)
    speed = np.asarray(inputs["speed_features"], np.float32)
    B = cords.shape[0]
    assert B == 8

    def g(name):
        return np.asarray(inputs[name], np.float32)

    flags = (
        not np.allclose(g("bo1"), 0), not np.allclose(g("bo2"), 0),
        not np.allclose(g("ffn_b1"), 0), not np.allclose(g("ffn_b2"), 0),
        not (np.allclose(g("ln1_g"), 1) and np.allclose(g("ln1_b"), 0)),
        not (np.allclose(g("ln2_g"), 1) and np.allclose(g("ln2_b"), 0)),
        not (np.allclose(g("ln3_g"), 1) and np.allclose(g("ln3_b"), 0)),
    )
    if flags not in _PROGRAM_CACHE:
        _PROGRAM_CACHE[flags] = _build_program(flags)
    nc = _PROGRAM_CACHE[flags]

    shared = {
        "wq1T": _bf(g("wq1").T), "wk1T": _bf(g("wk1").T),
        "wv1T": _bf(g("wv1").T), "wo1T": _bf(g("wo1").T),
        "wq2T": _bf(g("wq2").T), "wk2T": _bf(g("wk2").T),
        "wv2T": _bf(g("wv2").T), "wo2T": _bf(g("wo2").T),
        "fw1T": _bf(g("ffn_w1").T), "fw2T": _bf(g("ffn_w2").T),
    }
    use_bo1, use_bo2, use_fb1, use_fb2, use_g1, use_g2, use_g3 = flags
    if use_bo1:
        shared["bo1"] = _to_pm(g("bo1"), NCH)
    if use_bo2:
        shared["bo2"] = _to_pm(g("bo2"), NCH)
    if use_fb1:
        shared["fb1"] = _to_pm(g("ffn_b1"), DF // P)
    if use_fb2:
        shared["fb2"] = _to_pm(g("ffn_b2"), NCH)
    for i, use in ((1, use_g1), (2, use_g2), (3, use_g3)):
        if use:
            shared[f"g{i}"] = _to_pm(g(f"ln{i}_g"), NCH)
            shared[f"b{i}"] = _to_pm(g(f"ln{i}_b"), NCH)

    in_maps = []
    for b in range(B):
        m = dict(shared)
        m["x1T"] = _bf(cords[b].T)
        m["x2T"] = _bf(spatial[b].T)
        m["x3T"] = _bf(speed[b].T)
        in_maps.append(m)

    global _LAST_IN_MAPS
    _LAST_IN_MAPS = in_maps
    res = run_bass_kernel_spmd(nc, in_maps, core_ids=list(range(B)))
    out = np.stack([res.results[b]["yT"].T for b in range(B)], axis=0)
    return np.ascontiguousarray(out.astype(np.float32))


# revision 71
# speedup vs baseline: 1.1586x; 1.0150x over previous
"""DualCrossAttention Trainium2 kernel (bf16).

Data-parallel: batch=8 across 8 NeuronCores, one batch element per core.
Per core: two cross-attentions + FFN + 3 LayerNorms on [768, 512] activations.

Layout: feature-major activations (x.T: [feature(part), seq(free)]); weights
host-pre-transposed and cast to bf16 so every projection is a full-rate PE
matmul (bf16 streams 1 cycle/row vs 1.5 for fp32-HIGH, and enables FWL).
Attention: S.T = k_h @ q_h.T with the two heads of a pair emitted as adjacent
matmuls into PE row-groups 0/64 (concurrent execution) writing one shared
PSUM tile, so a single wide ACT exp call covers both heads; the wm scale is
folded into the exp affine. O.T accumulates with a ones-augmented V column so
the softmax denominator lands in PSUM row 64. All per-position normalizers
(attn 1/denominator, LN rstd/mean*rstd) are broadcast across partitions with
tiny K=1 PE matmuls into PSUM — no DRAM bounce. All weights are prefetched
at kernel start (bf16 halves the SBUF/DMA footprint).
"""
import contextlib

import numpy as np
import ml_dtypes

import concourse.bacc as bacc
import concourse.bass as bass
import concourse.tile as tile
from concourse import mybir
from concourse.bass_utils import run_bass_kernel_spmd
class _Bacc(bacc.Bacc):
    """Bacc with Exp/Ln pinned to the natural_log_exp_and_others ACT table
    set: the default chooser alternates between exp_and_others (attention
    softmax) and natural_log (LayerNorm rstd), paying a ~2.7us table load at
    every switch. Removing Exp/Ln from the single-function sets makes both
    resolve to the combined set, leaving only the Gelu switches."""

    def insert_act_table_loads(self):
        from concourse.hw_specs import get_activation_tables
        import bass_rust as _bass_rust
        has_activation = any(
            isinstance(i, mybir.InstActivation)
            for b in self.main_func.blocks
            for i in b.instructions
        )
        if not has_activation:
            return
        AFt = mybir.ActivationFunctionType
        tables = []
        for name, fns in get_activation_tables(self.m.arch).items():
            if name == "exp_and_others":
                fns = fns - {AFt.Exp}
            elif name == "natural_log":
                fns = fns - {AFt.Ln}
            tables.append((name, fns))
        _bass_rust.insert_act_table_loads(self, tables)


F32 = mybir.dt.float32
F32R = mybir.dt.float32r
BF = mybir.dt.bfloat16
AF = mybir.ActivationFunctionType
ALU = mybir.AluOpType

H, KD, VD = 8, 64, 64
D, DF = 512, 2048
S1, S2, S3 = 768, 1024, 768
P = 128
NCH = D // P            # 4 feature chunks of the 512-dim residual stream
W = 1.25                # wm weight scale
INV_SQRT = 0.125        # 1/sqrt(64)
EPS = 1e-5

_PROGRAM_CACHE = {}
_USE_BASE_ATTN = False
_USE_ACT_COPY = True
_KO_MAJOR = True
_DEBUG_TAPS = False


def _regions(n):
    """Split free dim n into <=512 column regions (PSUM-bank aligned)."""
    out = []
    s = 0
    while s < n:
        e = min(s + 512, n)
        out.append((s, e))
        s = e
    return out


class _Ctx:
    """Shared handles for the emit helpers."""
    pass


def _emit_proj(nc, psum_pool, wT_sb, xT_sb, n_out, n_seq, k_chunks, consume,
               ko_major=False, m_group=None, tag="proj"):
    """out.T[o, i] = sum_d wT[d, o] * xT[d, i]; calls consume(m, psum_ap).

    ko_major=True holds m-tiles live and loops ko outer / m inner, so the
    first matmuls only need xT chunk 0 (pipelines into a producer of xT).
    m_group limits how many m-tiles are live at once (PSUM pressure): the
    m-range is processed in groups, each group ko-major.
    """
    if ko_major:
        n_m = n_out // P
        if m_group is None:
            m_group = n_m
        for m0 in range(0, n_m, m_group):
            ms = range(m0, min(m0 + m_group, n_m))
            tiles = {m: psum_pool.tile([P, 1024], F32, tag=tag,
                                       name=f"pp{tag}{m}") for m in ms}
            for ko in range(k_chunks):
                for m in ms:
                    for (a, b) in _regions(n_seq):
                        nc.tensor.matmul(
                            tiles[m][:, a:b],
                            wT_sb[:, ko, m * P:(m + 1) * P],
                            xT_sb[:, ko, a:b],
                            start=(ko == 0), stop=(ko == k_chunks - 1),
                        )
            for m in ms:
                consume(m, tiles[m][:, :n_seq])
        return
    for m in range(n_out // P):
        ps = psum_pool.tile([P, 1024], F32, tag="proj", name=f"pp{m}")
        for ko in range(k_chunks):
            for (a, b) in _regions(n_seq):
                nc.tensor.matmul(
                    ps[:, a:b],
                    wT_sb[:, ko, m * P:(m + 1) * P],
                    xT_sb[:, ko, a:b],
                    start=(ko == 0), stop=(ko == k_chunks - 1),
                )
        consume(m, ps[:, :n_seq])


def _emit_ln(nc, cx, psum_pool, z_src, resid, bias_pm, z_sb, y_sb, n_seq,
             scale_gb, out_cb=None, bc_tags=("proj", "proj")):
    """LayerNorm over the feature axis (partitions x NCH chunks).

    z_src(m) -> psum AP [P, n_seq] (projection output chunk m);
    z = psum + bias + resid is built in z_sb (bf16); stats via ones-matmuls
    (partition reduction on PE); rstd via exp(-0.5 ln(var+eps)); rstd and
    mean*rstd are broadcast to [P, n_seq] with K=1 PE matmuls into recycled
    proj-tag PSUM slots; y_sb = z*rstd_bc - mr_bc (may alias z_sb).
    """
    sb = cx.sb
    stat_z = psum_pool.tile([1, 768], F32, tag="ln_stat_z", bufs=1)
    stat_zsq = psum_pool.tile([1, 768], F32, tag="ln_stat_zsq", bufs=1)
    for m in range(NCH):
        ps = z_src(m)
        if bias_pm is not None:
            nc.vector.scalar_tensor_tensor(
                z_sb[:, m, :], ps, bias_pm[:, m:m + 1], resid[:, m, :],
                op0=ALU.add, op1=ALU.add)
        else:
            nc.vector.tensor_tensor(z_sb[:, m, :], ps, resid[:, m, :], ALU.add)
        zsq = cx.zsq_pool.tile([P, 768], BF, tag="ln_zsq", name=f"zsq{m}")
        # last chunk's square on DVE (shorter critical path); others on the
        # otherwise-idle gpsimd
        sq_eng = nc.vector if m == NCH - 1 else nc.gpsimd
        sq_eng.tensor_tensor(zsq[:, :n_seq], z_sb[:, m, :], z_sb[:, m, :],
                             ALU.mult)
        for (a, b) in _regions(n_seq):
            nc.tensor.matmul(stat_z[0:1, a:b], cx.ones_bf[:, 0:1],
                             z_sb[:, m, a:b],
                             start=(m == 0), stop=(m == NCH - 1))
            nc.tensor.matmul(stat_zsq[0:1, a:b], cx.ones_bf[:, 0:1],
                             zsq[:, a:b],
                             start=(m == 0), stop=(m == NCH - 1))
    # small per-position vectors, all on partition 0
    # msq = (sum_z/D)^2 on ACT (Square is in the pinned table set);
    # var = sum(z^2)/D - msq -> ln(var+eps) -> rstd = exp(-0.5*ln);
    # mr = (sum_z/D)*rstd fused in one scalar_tensor_tensor
    msq = sb.tile([1, n_seq], F32, tag="ln_msq")
    nc.scalar.activation(msq[:], stat_z[0:1, :n_seq], AF.Square, bias=0.0,
                         scale=1.0 / D)
    rstd_t = sb.tile([1, n_seq], F32, tag="ln_rstd")
    nc.vector.scalar_tensor_tensor(rstd_t[:], stat_zsq[0:1, :n_seq], 1.0 / D,
                                   msq[:], op0=ALU.mult, op1=ALU.subtract)
    nc.scalar.activation(rstd_t[:], rstd_t[:], AF.Ln, bias=cx.eps_sb[0:1, :],
                         scale=1.0)
    lnv = sb.tile([1, 2 * n_seq], BF, tag="lnv")
    nc.scalar.activation(lnv[:, 0:n_seq], rstd_t[:], AF.Exp, bias=0.0,
                         scale=-0.5)
    nc.vector.scalar_tensor_tensor(lnv[:, n_seq:2 * n_seq],
                                   stat_z[0:1, :n_seq], 1.0 / D,
                                   lnv[:, 0:n_seq], op0=ALU.mult,
                                   op1=ALU.mult)
    # broadcast rstd / mean*rstd across partitions with K=1 matmuls into
    # recycled proj-tag PSUM slots (no DRAM bounce)
    rstd_bc = psum_pool.tile([P, 1024], F32, tag=bc_tags[0], name="rstdbc")
    mr_bc = psum_pool.tile([P, 1024], F32, tag=bc_tags[1], name="mrbc")
    for (a, b) in _regions(n_seq):
        nc.tensor.matmul(rstd_bc[:, a:b], cx.ones128[:, :], lnv[0:1, a:b],
                         start=True, stop=True)
        nc.tensor.matmul(mr_bc[:, a:b], cx.ones128[:, :],
                         lnv[0:1, n_seq + a:n_seq + b], start=True, stop=True)
    for m in range(NCH):
        nc.vector.tensor_tensor(y_sb[:, m, :], z_sb[:, m, :],
                                rstd_bc[:, :n_seq], ALU.mult)
        nc.vector.tensor_tensor(y_sb[:, m, :], y_sb[:, m, :],
                                mr_bc[:, :n_seq], ALU.subtract)
        if scale_gb is not None:
            g_sb, b_sb = scale_gb
            nc.vector.tensor_scalar(
                y_sb[:, m, :], y_sb[:, m, :],
                g_sb[:, m:m + 1], b_sb[:, m:m + 1], op0=ALU.mult, op1=ALU.add)
        if out_cb is not None:
            out_cb(m)


def _attn_units(layer, n_kv):
    """Unit list: (j_list, qa, qb, exp_scale). One exp call per unit covers
    both heads of the pair (and both j's of a pair for layer 2)."""
    J = n_kv // P
    units = []
    if layer == 1:
        # wm1: (q<512, kv<512) and (q>=512, kv>=512) get W
        for j in range(J):
            lo = W * INV_SQRT if j * P < 512 else INV_SQRT
            hi = INV_SQRT if j * P < 512 else W * INV_SQRT
            units.append(([j], 0, 512, lo))
            units.append(([j], 512, 768, hi))
    else:
        # wm2: diagonal 256-blocks get W; kv pair jp covers block jp
        for jp in range(J // 2):
            for b in range(3):
                sc = W * INV_SQRT if b == jp else INV_SQRT
                units.append(([2 * jp, 2 * jp + 1], 256 * b, 256 * (b + 1),
                              sc))
    return units


def _exp_slices(layer, j, n_q):
    """Per (attention layer, key-chunk j): (col_lo, col_hi, exp scale)."""
    if layer == 1:
        jlo = j * P < 512
        s_lo = W * INV_SQRT if jlo else INV_SQRT
        s_hi = INV_SQRT if jlo else W * INV_SQRT
        return [(0, 512, s_lo), (512, n_q, s_hi)]
    blk = j // 2
    raw = [(b * 256, min((b + 1) * 256, n_q),
            W * INV_SQRT if b == blk else INV_SQRT) for b in range(3)]
    out = [raw[0]]
    for (lo, hi, sc) in raw[1:]:
        plo, phi, psc = out[-1]
        if sc == psc and lo == phi:
            out[-1] = (plo, hi, sc)
        else:
            out.append((lo, hi, sc))
    return out


def _emit_attn_base(nc, cx, work, psum_s, psum_ot, qT_sb, kT_sb, v_sb, ot_sb,
                    layer, n_q, n_kv):
    """Baseline-style attention: per-head S psum tiles + sliced exp."""
    J = n_kv // P
    heads = lambda c: ((slice(0, 64), 2 * c), (slice(64, 128), 2 * c + 1))
    units = [(j, a, b) for j in range(J) for (a, b) in _regions(n_q)]

    def emit_S_unit(c, u, etiles):
        (hb_e, h_e), (hb_o, h_o) = heads(c)
        j, a, b = units[u]
        if j not in etiles:
            etiles[j] = (
                work.tile([P, n_q], BF, tag="exps", bufs=4, name=f"ee{c}_{j}"),
                work.tile([P, n_q], BF, tag="exps", bufs=4, name=f"eo{c}_{j}"),
            )
        e_e, e_o = etiles[j]
        ps_e = psum_s.tile([P, 512], F32, tag="s", name=f"se{c}_{j}_{a}")
        ps_o = psum_s.tile([P, 512], F32, tag="s", name=f"so{c}_{j}_{a}")
        nc.tensor.matmul(ps_e[:, :b - a],
                         kT_sb[hb_e, c, j * P:(j + 1) * P],
                         qT_sb[hb_e, c, a:b], start=True, stop=True)
        nc.tensor.matmul(ps_o[:, :b - a],
                         kT_sb[hb_o, c, j * P:(j + 1) * P],
                         qT_sb[hb_o, c, a:b], start=True, stop=True)
        for e, ps in ((e_e, ps_e), (e_o, ps_o)):
            for (lo, hi, sc) in _exp_slices(layer, j, n_q):
                lo2, hi2 = max(lo, a), min(hi, b)
                if lo2 < hi2:
                    nc.scalar.activation(
                        e[:, lo2:hi2], ps[:, lo2 - a:hi2 - a],
                        AF.Exp, bias=0.0, scale=sc)

    def emit_O_unit(c, u, etiles, po_all):
        j, a, b = units[u]
        e_e, e_o = etiles[j]
        for (hb, h), e in zip(heads(c), (e_e, e_o)):
            nc.tensor.matmul(
                po_all[h][0:65, a:b],
                v_sb[:, j, h, 0:65],
                e[:, a:b],
                start=(j == 0), stop=(j == J - 1))

    for c in range(H // 2):
        po_all = {}
        for hb, h in heads(c):
            po_all[h] = psum_ot.tile([65, 768], F32, tag="ot", name=f"po{h}")
        etiles = {}
        emit_S_unit(c, 0, etiles)
        for u in range(len(units)):
            if u + 1 < len(units):
                emit_S_unit(c, u + 1, etiles)
            emit_O_unit(c, u, etiles, po_all)
        srow = work.tile([1, 2 * n_q], F32, tag="srow", bufs=2,
                         name=f"sr{c}")
        rr = work.tile([1, 2 * n_q], F32, tag="rr", bufs=2, name=f"rr{c}")
        (hb_e, h_e), (hb_o, h_o) = heads(c)
        # stage denominator rows at partition 0 in SBUF: the custom-DVE
        # reciprocal misreads PSUM at base partition 64 on hardware
        nc.vector.tensor_copy(srow[:, 0:n_q], po_all[h_e][64:65, 0:n_q])
        nc.vector.tensor_copy(srow[:, n_q:2 * n_q],
                              po_all[h_o][64:65, 0:n_q])
        nc.vector.reciprocal_approx_fast(out=rr[:, 0:n_q],
                                         in_=srow[:, 0:n_q])
        nc.vector.reciprocal_approx_fast(out=rr[:, n_q:2 * n_q],
                                         in_=srow[:, n_q:2 * n_q])
        nc.scalar.copy(ot_sb[hb_e, c, :], po_all[h_e][0:64, :n_q])
        nc.scalar.copy(ot_sb[hb_o, c, :], po_all[h_o][0:64, :n_q])
        drp = cx.dram.tile([2, n_q], F32, tag="drp", name=f"drp{c}")
        nc.sync.dma_start(drp[0:1, :], rr[:, 0:n_q])
        nc.sync.dma_start(drp[1:2, :], rr[:, n_q:2 * n_q])
        bc = work.tile([P, n_q], F32, tag="attn_bc", name=f"bc{c}")
        nc.gpsimd.dma_start(bc[0:64, :], drp[0:1, :].to_broadcast([64, n_q]))
        nc.gpsimd.dma_start(bc[64:128, :],
                            drp[1:2, :].to_broadcast([64, n_q]))
        nc.vector.tensor_tensor(ot_sb[:, c, :], ot_sb[:, c, :], bc[:],
                                ALU.mult)


def _emit_attn(nc, cx, work, psum_s, psum_ot, qT_sb, kT_sb, v_sb, ot_sb,
               layer, n_q, n_kv, filler=None):
    """Cross-attention. Per unit: the heads of a pair (x kv-pair for layer 2)
    are emitted as ADJACENT K=64 matmuls into PE row-groups 0/64 (concurrent)
    writing one shared PSUM tile; ONE ACT exp call (wm scale folded in)
    covers the whole tile. O.T accumulates per unit with a ones-augmented V
    column so the softmax denominator lands in PSUM row 64. S for unit u+1 is
    emitted before O of unit u so the PE has independent work while ACT
    computes exp.

    filler: optional per-pair list of callables emitting independent
    full-array PE work (borrowing an s-tag PSUM slot). Attention's K=64 /
    65-row matmuls only half-use the PE array and never re-warm the HAM
    clock gate; dense filler matmuls keep it at 2.4 GHz."""
    J = n_kv // P
    units = _attn_units(layer, n_kv)
    heads = lambda c: ((slice(0, 64), 2 * c), (slice(64, 128), 2 * c + 1))

    def ekey(jl):
        return jl[0]

    def emit_S_unit(c, u, etiles):
        jl, a, b, sc = units[u]
        w = b - a
        nj = len(jl)
        if ekey(jl) not in etiles:
            etiles[ekey(jl)] = work.tile([P, nj * 2, n_q], BF, tag="exps",
                                         bufs=3, name=f"e{c}_{ekey(jl)}")
        E = etiles[ekey(jl)]
        # Bank-safe layout: all head-even S in bank 0 ([0:512]), head-odd in
        # bank 1 ([512:1024]) — the two heads' matmuls execute CONCURRENTLY
        # via PE row-groups 0/64, and concurrent drains into the SAME PSUM
        # bank are a hardware fault. u-order (all-he, then all-ho) keeps the
        # psum stride regular (w) so one strided ACT exp covers the tile.
        ps = psum_s.tile([P, 1024], F32, tag="s", name=f"s{c}_{u}")
        (hb_e, h_e), (hb_o, h_o) = heads(c)
        for ji, j in enumerate(jl):
            nc.tensor.matmul(ps[:, ji * w:(ji + 1) * w],
                             kT_sb[hb_e, c, j * P:(j + 1) * P],
                             qT_sb[hb_e, c, a:b], start=True, stop=True)
            nc.tensor.matmul(ps[:, 512 + ji * w:512 + (ji + 1) * w],
                             kT_sb[hb_o, c, j * P:(j + 1) * P],
                             qT_sb[hb_o, c, a:b], start=True, stop=True)
        ps_g = ps[:, 0:1024].rearrange("p (g q) -> p g q", g=2)
        if nj == 1:
            nc.scalar.activation(E[:, :, a:b], ps_g[:, :, 0:w],
                                 AF.Exp, bias=0.0, scale=sc)
        else:
            nc.scalar.activation(
                E[:, :, a:b].rearrange("p (g j) q -> p g j q", g=2),
                ps_g[:, :, 0:nj * w].rearrange("p g (j q) -> p g j q", j=nj),
                AF.Exp, bias=0.0, scale=sc)

    def emit_O_unit(c, u, etiles, po_all):
        jl, a, b, sc = units[u]
        nj = len(jl)
        E = etiles[ekey(jl)]
        # start/stop must be unique per PSUM BANK (start=True clears the
        # whole bank's has_written bits): only the first/last matmul touching
        # a bank carries the flag; sibling regions in the same bank rely on
        # per-element overwrite-then-accumulate semantics.
        first_in_bank = a % 512 == 0
        last_in_bank = (b % 512 == 0) or (b == n_q)
        for ji, j in enumerate(jl):
            for hi, (hb, h) in enumerate(heads(c)):
                nc.tensor.matmul(
                    po_all[h][0:65, a:b],
                    v_sb[:, j, h, 0:65],
                    E[:, hi * nj + ji, a:b],
                    start=(j == 0 and first_in_bank),
                    stop=(j == J - 1 and last_in_bank))

    fill_i = [0]

    def feed_filler(c):
        # filler entries are (gate, fn): fn may only be emitted once pair
        # `gate` has been normalized (so a stalled piece never parks on an
        # s-slot the attention pipeline needs)
        if filler is not None and fill_i[0] < len(filler):
            gate, fn = filler[fill_i[0]]
            if gate < c:
                fn()
                fill_i[0] += 1

    for c in range(H // 2):
        po_all = {}
        for hb, h in heads(c):
            po_all[h] = psum_ot.tile([65, 768], F32, tag="ot", name=f"po{h}")
        etiles = {}
        emit_S_unit(c, 0, etiles)
        for u in range(len(units)):
            if u + 1 < len(units):
                emit_S_unit(c, u + 1, etiles)
            emit_O_unit(c, u, etiles, po_all)
            if layer == 2 or u % 2 == 1:
                feed_filler(c)
        # normalizers: stage denominator rows at partition 0 (custom-DVE
        # reciprocal misreads PSUM at base partition 64 on hardware), recip,
        # DRAM-bounce broadcast. The copies are split across ACT and DVE so
        # the serial tail chain pipelines across both engines.
        srow = work.tile([1, 2 * n_q], F32, tag="srow", bufs=2,
                         name=f"sr{c}")
        rr = work.tile([1, 2 * n_q], F32, tag="rr", bufs=2, name=f"rr{c}")
        (hb_e, h_e), (hb_o, h_o) = heads(c)
        nc.scalar.copy(srow[:, 0:n_q], po_all[h_e][64:65, 0:n_q])
        nc.vector.tensor_copy(srow[:, n_q:2 * n_q],
                              po_all[h_o][64:65, 0:n_q])
        nc.vector.reciprocal_approx_fast(out=rr[:, 0:n_q],
                                         in_=srow[:, 0:n_q])
        nc.vector.reciprocal_approx_fast(out=rr[:, n_q:2 * n_q],
                                         in_=srow[:, n_q:2 * n_q])
        nc.scalar.copy(ot_sb[hb_e, c, :], po_all[h_e][0:64, :n_q])
        nc.vector.tensor_copy(ot_sb[hb_o, c, :], po_all[h_o][0:64, :n_q])
        drp = cx.dram.tile([2, n_q], F32, tag="drp", name=f"drp{c}")
        nc.sync.dma_start(drp[0:1, :], rr[:, 0:n_q])
        nc.sync.dma_start(drp[1:2, :], rr[:, n_q:2 * n_q])
        bc = work.tile([P, n_q], F32, tag="attn_bc", name=f"bc{c}")
        nc.gpsimd.dma_start(bc[0:64, :], drp[0:1, :].to_broadcast([64, n_q]))
        nc.gpsimd.dma_start(bc[64:128, :],
                            drp[1:2, :].to_broadcast([64, n_q]))
        nc.vector.tensor_tensor(ot_sb[:, c, :], ot_sb[:, c, :], bc[:],
                                ALU.mult)
        feed_filler(c + 1)
    # flush remaining filler pieces (their deps are all satisfied now)
    if filler is not None:
        while fill_i[0] < len(filler):
            filler[fill_i[0]][1]()
            fill_i[0] += 1


def _r3(ap):
    """DRAM [K*128, n] -> [128(part), K, n] view for DMA."""
    return ap.rearrange("(ko p) s -> p ko s", p=P)


def _build_program(flags):
    use_bo1, use_bo2, use_fb1, use_fb2, use_g1, use_g2, use_g3 = flags
    nc = _Bacc("TRN2", target_bir_lowering=False, debug=False)

    def din(name, shape, dt=BF):
        return nc.dram_tensor(name, shape, dt, kind="ExternalInput").ap()

    x1T = din("x1T", [D, S1])
    x2T = din("x2T", [D, S2])
    x3T = din("x3T", [D, S3])
    wts = {n: din(n, [D, D]) for n in
           ("wq1T", "wk1T", "wv1T", "wo1T", "wq2T", "wk2T", "wv2T", "wo2T")}
    fw1T = din("fw1T", [D, DF])
    fw2T = din("fw2T", [DF, D])
    bo1 = din("bo1", [P, NCH]) if use_bo1 else None
    bo2 = din("bo2", [P, NCH]) if use_bo2 else None
    fb1 = din("fb1", [P, DF // P]) if use_fb1 else None
    fb2 = din("fb2", [P, NCH]) if use_fb2 else None
    gbd = {}
    for i, use in ((1, use_g1), (2, use_g2), (3, use_g3)):
        gbd[i] = (din(f"g{i}", [P, NCH]),
                  din(f"b{i}", [P, NCH])) if use else None
    yT = nc.dram_tensor("yT", [D, S1], F32, kind="ExternalOutput").ap()
    taps = {}
    if _DEBUG_TAPS:
        for tn in ("t_q1", "t_ot1", "t_y1", "t_ot2", "t_y2"):
            taps[tn] = nc.dram_tensor(tn, [D, S1], BF,
                                      kind="ExternalOutput").ap()

    def tap(name, src):
        if _DEBUG_TAPS:
            nc.sync.dma_start(_r3(taps[name]), src[:])

    with tile.TileContext(nc, pool_alloc_mode="queue") as tc:
        cx = _Ctx()
        cx.tc = tc
        with tc.tile_pool(name="sb", bufs=1) as sb, \
             tc.tile_pool(name="zsq", bufs=1) as zsq_pool, \
             tc.tile_pool(name="wpre", bufs=1) as wpre, \
             tc.tile_pool(name="dram", bufs=2, space="DRAM") as dram:
            cx.sb, cx.zsq_pool, cx.dram = sb, zsq_pool, dram

            ones_bf = sb.tile([P, 1], BF, tag="ones_bf")
            nc.vector.memset(ones_bf[:], 1.0)
            cx.ones_bf = ones_bf
            ones128 = sb.tile([1, P], BF, tag="ones128")
            nc.vector.memset(ones128[:], 1.0)
            cx.ones128 = ones128
            eps_sb = sb.tile([P, 1], F32, tag="eps")
            nc.vector.memset(eps_sb[:], EPS)
            cx.eps_sb = eps_sb

            def load_pm(ap, cols, tag):
                if ap is None:
                    return None
                t = sb.tile([P, cols], BF, tag=tag)
                nc.sync.dma_start(t[:], ap)
                return t

            bo1_sb = load_pm(bo1, NCH, "bo1")
            bo2_sb = load_pm(bo2, NCH, "bo2")
            fb1_sb = load_pm(fb1, DF // P, "fb1")
            fb2_sb = load_pm(fb2, NCH, "fb2")
            gb_sb = {}
            for i in (1, 2, 3):
                gb_sb[i] = None if gbd[i] is None else (
                    load_pm(gbd[i][0], NCH, f"g{i}"),
                    load_pm(gbd[i][1], NCH, f"b{i}"))

            y1_sb = sb.tile([P, NCH, S1], BF, tag="y1")
            y2_sb = sb.tile([P, NCH, S1], BF, tag="y2")

            def copy_cb(dst, eng):
                return lambda m, ps: eng(dst[:, m, :], ps)

            dve_copy = lambda out, ps: nc.vector.tensor_copy(out, ps)
            act_copy = (lambda out, ps: nc.scalar.copy(out, ps)) \
                if _USE_ACT_COPY else dve_copy

            def emit_v_proj(psum_pool, x_sb, wv_sb, v_sb, Jkv):
                nc.vector.memset(v_sb[:, :, :, 64:65], 1.0)
                for j in range(Jkv):
                    ps = psum_pool.tile([P, 1024], F32, tag="proj",
                                        name=f"vps{j}")
                    for ko in range(NCH):
                        nc.tensor.matmul(
                            ps[:, 0:D],
                            x_sb[:, ko, j * P:(j + 1) * P],
                            wv_sb[:, ko, :],
                            start=(ko == 0), stop=(ko == NCH - 1))
                    nc.vector.tensor_copy(
                        v_sb[:, j, :, 0:64],
                        ps[:, 0:D].rearrange("p (h v) -> p h v", h=H))

            # open order is reverse of close order (pool stack is LIFO)
            kv2 = tc.tile_pool(name="kv2", bufs=1)
            with kv2 as kv2p:
                x3_sb = kv2p.tile([P, NCH, S3], BF, tag="xkv")
                wk2_sb = kv2p.tile([P, NCH, D], BF, tag="wk")
                wv2_sb = kv2p.tile([P, NCH, D], BF, tag="wv")
                k2_sb = kv2p.tile([P, NCH, S3], BF, tag="k")
                v2_sb = kv2p.tile([P, S3 // P, H, 65], BF, tag="v")

                otp1 = contextlib.ExitStack()
                otp1p = otp1.enter_context(tc.tile_pool(name="otp1", bufs=1))
                x1_sb = otp1p.tile([P, NCH, S1], BF, tag="x1")
                ot_sb = otp1p.tile([P, NCH, S1], BF, tag="ot1")

                at1_ctx = contextlib.ExitStack()
                at1p = at1_ctx.enter_context(tc.tile_pool(name="at1", bufs=1))
                q_sb = at1p.tile([P, NCH, S1], BF, tag="q")
                k_sb = at1p.tile([P, NCH, S2], BF, tag="k")
                v_sb = at1p.tile([P, S2 // P, H, 65], BF, tag="v")

                kv1_ctx = contextlib.ExitStack()
                kv1p = kv1_ctx.enter_context(tc.tile_pool(name="kv1", bufs=1))
                wq_sb = kv1p.tile([P, NCH, D], BF, tag="wq")
                x2_sb = kv1p.tile([P, NCH, S2], BF, tag="xkv")
                wk_sb = kv1p.tile([P, NCH, D], BF, tag="wk")
                wv_sb = kv1p.tile([P, NCH, D], BF, tag="wv")
                # per-chunk DMAs in consumption order, issued across FOUR
                # engine DGE queues in parallel (descriptor generation is
                # ~1us serial per engine): the ko-major Q1/K1 projections
                # start as soon as their first chunks land
                for ko in range(NCH):
                    nc.sync.dma_start(wq_sb[:, ko, :],
                                      _r3(wts["wq1T"])[:, ko, :])
                    nc.gpsimd.dma_start(x1_sb[:, ko, :], _r3(x1T)[:, ko, :])
                for ko in range(NCH):
                    nc.scalar.dma_start(wk_sb[:, ko, :],
                                        _r3(wts["wk1T"])[:, ko, :])
                    nc.scalar.dma_start(x2_sb[:, ko, :], _r3(x2T)[:, ko, :])
                nc.sync.dma_start(wv_sb[:], _r3(wts["wv1T"]))
                nc.gpsimd.dma_start(x3_sb[:], _r3(x3T))
                nc.sync.dma_start(wk2_sb[:], _r3(wts["wk2T"]))
                nc.sync.dma_start(wv2_sb[:], _r3(wts["wv2T"]))

                # prefetch every later-phase weight now (bf16 fits in SBUF)
                wo1_sb = wpre.tile([P, NCH, D], BF, tag="wo1")
                wq2_sb = wpre.tile([P, NCH, D], BF, tag="wq2")
                wo2_sb = wpre.tile([P, NCH, D], BF, tag="wo2")
                fw1_sb = wpre.tile([P, NCH, DF], BF, tag="fw1")
                fw2_sb = wpre.tile([P, DF // P, D], BF, tag="fw2")
                nc.sync.dma_start(wo1_sb[:], _r3(wts["wo1T"]))
                nc.sync.dma_start(wq2_sb[:], _r3(wts["wq2T"]))
                nc.sync.dma_start(wo2_sb[:], _r3(wts["wo2T"]))
                nc.sync.dma_start(fw1_sb[:], _r3(fw1T))
                nc.sync.dma_start(fw2_sb[:], _r3(fw2T))

                # dense warm-up block: q1,k1,v1 (q1/k1 ko-major so the PE
                # starts on the first DMA'd chunks); k2/v2 are deferred into
                # the WO1/LN1 and WO2/LN2 phases as independent PE filler
                with tc.tile_pool(name="psA", bufs=4, space="PSUM") as psA:
                    _emit_proj(nc, psA, wq_sb, x1_sb, D, S1, NCH,
                               copy_cb(q_sb, act_copy), ko_major=True)
                    _emit_proj(nc, psA, wk_sb, x2_sb, D, S2, NCH,
                               copy_cb(k_sb, act_copy), ko_major=True)
                    emit_v_proj(psA, x2_sb, wv_sb, v_sb, S2 // P)
                kv1_ctx.close()  # frees x2 + wq1/wk1/wv1 SBUF

                # attention 1, with K2/V2 projection pieces as full-array
                # PE filler woven between j-groups (keeps the HAM clock warm
                # through the half-array attention matmuls)
                attn_fn = _emit_attn_base if _USE_BASE_ATTN else _emit_attn
                s_bufs = 4 if _USE_BASE_ATTN else 2
                nc.vector.memset(v2_sb[:, :, :, 64:65], 1.0)
                with tc.tile_pool(name="wk1w", bufs=3) as work, \
                     tc.tile_pool(name="ps_s1", bufs=s_bufs,
                                  space="PSUM") as pss, \
                     tc.tile_pool(name="ps_ot1", bufs=2,
                                  space="PSUM") as psot:
                    def mk_k2(m, a, b):
                        def f():
                            ps = pss.tile([P, 1024], F32, tag="s",
                                          name=f"fk2_{m}_{a}")
                            for ko in range(NCH):
                                nc.tensor.matmul(
                                    ps[:, 0:b - a],
                                    wk2_sb[:, ko, m * P:(m + 1) * P],
                                    x3_sb[:, ko, a:b],
                                    start=(ko == 0), stop=(ko == NCH - 1))
                            nc.vector.tensor_copy(k2_sb[:, m, a:b],
                                                  ps[:, 0:b - a])
                        return f

                    def mk_v2(j, half):
                        def f():
                            ps = pss.tile([P, 1024], F32, tag="s",
                                          name=f"fv2_{j}_{half}")
                            a = half * 256
                            for ko in range(NCH):
                                nc.tensor.matmul(
                                    ps[:, 0:256],
                                    x3_sb[:, ko, j * P:(j + 1) * P],
                                    wv2_sb[:, ko, a:a + 256],
                                    start=(ko == 0), stop=(ko == NCH - 1))
                            nc.vector.tensor_copy(
                                v2_sb[:, j, 4 * half:4 * half + 4, 0:64],
                                ps[:, 0:256].rearrange("p (h v) -> p h v",
                                                       h=4))
                        return f

                    fillers = []
                    for m in range(NCH):
                        fillers.append((-1, mk_k2(m, 0, 256)))
                        fillers.append((-1, mk_k2(m, 256, 512)))
                        fillers.append((-1, mk_k2(m, 512, 768)))
                    for j in range(S3 // P):
                        fillers.append((-1, mk_v2(j, 0)))
                        fillers.append((-1, mk_v2(j, 1)))
                    if _USE_BASE_ATTN:
                        attn_fn(nc, cx, work, pss, psot, q_sb, k_sb, v_sb,
                                ot_sb, 1, S1, S2)
                    else:
                        attn_fn(nc, cx, work, pss, psot, q_sb, k_sb, v_sb,
                                ot_sb, 1, S1, S2, filler=fillers)
                tap("t_q1", q_sb)
                tap("t_ot1", ot_sb)
                at1_ctx.close()  # frees q1/k1/v1 SBUF

                # wo1 + LN1 (wo1 ko-major in 2-m groups: the first WO matmuls
                # need only attention pair 0's normalized output)
                with tc.tile_pool(name="psB1", bufs=2, space="PSUM") as psB:
                    wo_ps = {}
                    _emit_proj(nc, psB, wo1_sb, ot_sb, D, S1, NCH,
                               lambda m, ps: wo_ps.__setitem__(m, ps),
                               ko_major=True, m_group=2)
                    _emit_ln(nc, cx, psB, lambda m: wo_ps[m], x1_sb,
                             bo1_sb, y1_sb, y1_sb, S1, gb_sb[1])
                tap("t_y1", y1_sb)
                otp1.close()

                # q2 projection (ko-major: starts as soon as y1 chunk 0 is
                # normalized)
                otp2 = contextlib.ExitStack()
                otp2p = otp2.enter_context(tc.tile_pool(name="otp2", bufs=1))
                ot2_sb = otp2p.tile([P, NCH, S1], BF, tag="ot2")
                q2_sb = otp2p.tile([P, NCH, S1], BF, tag="q2")
                with tc.tile_pool(name="psC", bufs=4, space="PSUM") as psC:
                    _emit_proj(nc, psC, wq2_sb, y1_sb, D, S1, NCH,
                               copy_cb(q2_sb, act_copy), ko_major=_KO_MAJOR)

                # attention 2
                with tc.tile_pool(name="wk2w", bufs=3) as work2, \
                     tc.tile_pool(name="ps_s2", bufs=s_bufs,
                                  space="PSUM") as pss2, \
                     tc.tile_pool(name="ps_ot2", bufs=2,
                                  space="PSUM") as psot2:
                    attn_fn(nc, cx, work2, pss2, psot2, q2_sb, k2_sb,
                            v2_sb, ot2_sb, 2, S1, S3)
                tap("t_ot2", ot2_sb)

                # wo2 + LN2
                with tc.tile_pool(name="psD", bufs=2, space="PSUM") as psD:
                    wo2_ps = {}
                    _emit_proj(nc, psD, wo2_sb, ot2_sb, D, S1, NCH,
                               lambda m, ps: wo2_ps.__setitem__(m, ps),
                               ko_major=True, m_group=2)
                    _emit_ln(nc, cx, psD, lambda m: wo2_ps[m], y1_sb,
                             bo2_sb, y2_sb, y2_sb, S1, gb_sb[2])
                tap("t_y2", y2_sb)
                otp2.close()

            # FFN + LN3
            zbuf = sb.tile([P, NCH, S1], BF, tag="y1")  # reuse y1 slot
            yT_sb = sb.tile([P, NCH, S1], F32, tag="yT")
            with tc.tile_pool(name="ffn1", bufs=1) as f1p:
                h_sb = f1p.tile([P, DF // P, S1], BF, tag="hT")
                with tc.tile_pool(name="psE", bufs=3, space="PSUM") as psE:
                    def gelu_consume(m, ps):
                        nc.scalar.activation(
                            h_sb[:, m, :], ps[:, 0:S1], AF.Gelu,
                            bias=(fb1_sb[:, m:m + 1]
                                  if fb1_sb is not None else 0.0),
                            scale=1.0)
                    # ko-major pairs: the first FFN1 matmuls need only y2
                    # chunk 0, starting inside LN2's normalize window
                    _emit_proj(nc, psE, fw1_sb, y2_sb, DF, S1, NCH,
                               gelu_consume, ko_major=True, m_group=2)

                with tc.tile_pool(name="psF", bufs=2, space="PSUM") as psF:
                    f2_ps = {}
                    _emit_proj(nc, psF, fw2_sb, h_sb, D, S1, DF // P,
                               lambda m, ps: f2_ps.__setitem__(m, ps))

                    out_eng = [nc.sync, nc.scalar, nc.gpsimd, nc.sync]

                    def out_dma(m):
                        out_eng[m].dma_start(_r3(yT)[:, m, :],
                                             yT_sb[:, m, :])

                    _emit_ln(nc, cx, psF, lambda m: f2_ps[m], y2_sb, fb2_sb,
                             zbuf, yT_sb, S1, gb_sb[3], out_cb=out_dma)

    nc.finalize()
    return nc


def _to_pm(vec, cols):
    """[cols*128] vector -> [128, cols] partition-major layout (bf16)."""
    return np.ascontiguousarray(vec.reshape(cols, P).T).astype(
        ml_dtypes.bfloat16)


def _bf(a):
    return np.ascontiguousarray(a).astype(ml_dtypes.bfloat16)


def kernel(**inputs):
    cords = np.asarray(inputs["cords_features"], np.float32)
    spatial = np.asarray(inputs["spatial_features"], np.float32)
    speed = np.asarray(inputs["speed_features"], np.float32)
    B = cords.shape[0]
    assert B == 8

    def g(name):
        return np.asarray(inputs[name], np.float32)

    flags = (
        not np.allclose(g("bo1"), 0), not np.allclose(g("bo2"), 0),
        not np.allclose(g("ffn_b1"), 0), not np.allclose(g("ffn_b2"), 0),
        not (np.allclose(g("ln1_g"), 1) and np.allclose(g("ln1_b"), 0)),
        not (np.allclose(g("ln2_g"), 1) and np.allclose(g("ln2_b"), 0)),
        not (np.allclose(g("ln3_g"), 1) and np.allclose(g("ln3_b"), 0)),
    )
    if flags not in _PROGRAM_CACHE:
        _PROGRAM_CACHE[flags] = _build_program(flags)
    nc = _PROGRAM_CACHE[flags]

    shared = {
        "wq1T": _bf(g("wq1").T), "wk1T": _bf(g("wk1").T),
        "wv1T": _bf(g("wv1").T), "wo1T": _bf(g("wo1").T),
        "wq2T": _bf(g("wq2").T), "wk2T": _bf(g("wk2").T),
        "wv2T": _bf(g("wv2").T), "wo2T": _bf(g("wo2").T),
        "fw1T": _bf(g("ffn_w1").T), "fw2T": _bf(g("ffn_w2").T),
    }
    use_bo1, use_bo2, use_fb1, use_fb2, use_g1, use_g2, use_g3 = flags
    if use_bo1:
        shared["bo1"] = _to_pm(g("bo1"), NCH)
    if use_bo2:
        shared["bo2"] = _to_pm(g("bo2"), NCH)
    if use_fb1:
        shared["fb1"] = _to_pm(g("ffn_b1"), DF // P)
    if use_fb2:
        shared["fb2"] = _to_pm(g("ffn_b2"), NCH)
    for i, use in ((1, use_g1), (2, use_g2), (3, use_g3)):
        if use:
            shared[f"g{i}"] = _to_pm(g(f"ln{i}_g"), NCH)
            shared[f"b{i}"] = _to_pm(g(f"ln{i}_b"), NCH)

    in_maps = []
    for b in range(B):
        m = dict(shared)
        m["x1T"] = _bf(cords[b].T)
        m["x2T"] = _bf(spatial[b].T)
        m["x3T"] = _bf(speed[b].T)
        in_maps.append(m)

    global _LAST_IN_MAPS
    _LAST_IN_MAPS = in_maps
    res = run_bass_kernel_spmd(nc, in_maps, core_ids=list(range(B)))
    out = np.stack([res.results[b]["yT"].T for b in range(B)], axis=0)
    return np.ascontiguousarray(out.astype(np.float32))


# revision 72
# speedup vs baseline: 1.2510x; 1.0797x over previous
"""DualCrossAttention Trainium2 kernel (bf16).

Data-parallel: batch=8 across 8 NeuronCores, one batch element per core.
Per core: two cross-attentions + FFN + 3 LayerNorms on [768, 512] activations.

Layout: feature-major activations (x.T: [feature(part), seq(free)]); weights
host-pre-transposed and cast to bf16 so every projection is a full-rate PE
matmul (bf16 streams 1 cycle/row vs 1.5 for fp32-HIGH, and enables FWL).
Attention: S.T = k_h @ q_h.T with the two heads of a pair emitted as adjacent
matmuls into PE row-groups 0/64 (concurrent execution) writing one shared
PSUM tile, so a single wide ACT exp call covers both heads; the wm scale is
folded into the exp affine. O.T accumulates with a ones-augmented V column so
the softmax denominator lands in PSUM row 64. All per-position normalizers
(attn 1/denominator, LN rstd/mean*rstd) are broadcast across partitions with
tiny K=1 PE matmuls into PSUM — no DRAM bounce. All weights are prefetched
at kernel start (bf16 halves the SBUF/DMA footprint).
"""
import contextlib

import numpy as np
import ml_dtypes

import concourse.bacc as bacc
import concourse.bass as bass
import concourse.tile as tile
from concourse import mybir
from concourse.bass_utils import run_bass_kernel_spmd
class _Bacc(bacc.Bacc):
    """Bacc with Exp/Ln pinned to the natural_log_exp_and_others ACT table
    set: the default chooser alternates between exp_and_others (attention
    softmax) and natural_log (LayerNorm rstd), paying a ~2.7us table load at
    every switch. Removing Exp/Ln from the single-function sets makes both
    resolve to the combined set, leaving only the Gelu switches."""

    def insert_act_table_loads(self):
        from concourse.hw_specs import get_activation_tables
        import bass_rust as _bass_rust
        has_activation = any(
            isinstance(i, mybir.InstActivation)
            for b in self.main_func.blocks
            for i in b.instructions
        )
        if not has_activation:
            return
        AFt = mybir.ActivationFunctionType
        tables = []
        for name, fns in get_activation_tables(self.m.arch).items():
            if name == "exp_and_others":
                fns = fns - {AFt.Exp}
            elif name == "natural_log":
                fns = fns - {AFt.Ln}
            tables.append((name, fns))
        _bass_rust.insert_act_table_loads(self, tables)


F32 = mybir.dt.float32
F32R = mybir.dt.float32r
BF = mybir.dt.bfloat16
AF = mybir.ActivationFunctionType
ALU = mybir.AluOpType

H, KD, VD = 8, 64, 64
D, DF = 512, 2048
S1, S2, S3 = 768, 1024, 768
P = 128
NCH = D // P            # 4 feature chunks of the 512-dim residual stream
W = 1.25                # wm weight scale
INV_SQRT = 0.125        # 1/sqrt(64)
EPS = 1e-5

_PROGRAM_CACHE = {}
_USE_BASE_ATTN = False
_USE_ACT_COPY = True
_KO_MAJOR = True
_DEBUG_TAPS = False


def _regions(n):
    """Split free dim n into <=512 column regions (PSUM-bank aligned)."""
    out = []
    s = 0
    while s < n:
        e = min(s + 512, n)
        out.append((s, e))
        s = e
    return out


class _Ctx:
    """Shared handles for the emit helpers."""
    pass


def _emit_proj(nc, psum_pool, wT_sb, xT_sb, n_out, n_seq, k_chunks, consume,
               ko_major=False, m_group=None, tag="proj"):
    """out.T[o, i] = sum_d wT[d, o] * xT[d, i]; calls consume(m, psum_ap).

    ko_major=True holds m-tiles live and loops ko outer / m inner, so the
    first matmuls only need xT chunk 0 (pipelines into a producer of xT).
    m_group limits how many m-tiles are live at once (PSUM pressure): the
    m-range is processed in groups, each group ko-major.
    """
    if ko_major:
        n_m = n_out // P
        if m_group is None:
            m_group = n_m
        for m0 in range(0, n_m, m_group):
            ms = range(m0, min(m0 + m_group, n_m))
            tiles = {m: psum_pool.tile([P, 1024], F32, tag=tag,
                                       name=f"pp{tag}{m}") for m in ms}
            for ko in range(k_chunks):
                for m in ms:
                    for (a, b) in _regions(n_seq):
                        nc.tensor.matmul(
                            tiles[m][:, a:b],
                            wT_sb[:, ko, m * P:(m + 1) * P],
                            xT_sb[:, ko, a:b],
                            start=(ko == 0), stop=(ko == k_chunks - 1),
                        )
            for m in ms:
                consume(m, tiles[m][:, :n_seq])
        return
    for m in range(n_out // P):
        ps = psum_pool.tile([P, 1024], F32, tag="proj", name=f"pp{m}")
        for ko in range(k_chunks):
            for (a, b) in _regions(n_seq):
                nc.tensor.matmul(
                    ps[:, a:b],
                    wT_sb[:, ko, m * P:(m + 1) * P],
                    xT_sb[:, ko, a:b],
                    start=(ko == 0), stop=(ko == k_chunks - 1),
                )
        consume(m, ps[:, :n_seq])


def _emit_ln(nc, cx, psum_pool, z_src, resid, bias_pm, z_sb, y_sb, n_seq,
             scale_gb, out_cb=None, bc_tags=("proj", "proj")):
    """LayerNorm over the feature axis (partitions x NCH chunks).

    z_src(m) -> psum AP [P, n_seq] (projection output chunk m);
    z = psum + bias + resid is built in z_sb (bf16); stats via ones-matmuls
    (partition reduction on PE); rstd via exp(-0.5 ln(var+eps)); rstd and
    mean*rstd are broadcast to [P, n_seq] with K=1 PE matmuls into recycled
    proj-tag PSUM slots; y_sb = z*rstd_bc - mr_bc (may alias z_sb).
    """
    sb = cx.sb
    stat_z = psum_pool.tile([1, 768], F32, tag="ln_stat_z", bufs=1)
    stat_zsq = psum_pool.tile([1, 768], F32, tag="ln_stat_zsq", bufs=1)
    for m in range(NCH):
        ps = z_src(m)
        if bias_pm is not None:
            nc.vector.scalar_tensor_tensor(
                z_sb[:, m, :], ps, bias_pm[:, m:m + 1], resid[:, m, :],
                op0=ALU.add, op1=ALU.add)
        else:
            nc.vector.tensor_tensor(z_sb[:, m, :], ps, resid[:, m, :], ALU.add)
        zsq = cx.zsq_pool.tile([P, 768], BF, tag="ln_zsq", name=f"zsq{m}")
        # last chunk's square on DVE (shorter critical path); others on the
        # otherwise-idle gpsimd
        sq_eng = nc.vector if m == NCH - 1 else nc.gpsimd
        sq_eng.tensor_tensor(zsq[:, :n_seq], z_sb[:, m, :], z_sb[:, m, :],
                             ALU.mult)
        for (a, b) in _regions(n_seq):
            nc.tensor.matmul(stat_z[0:1, a:b], cx.ones_bf[:, 0:1],
                             z_sb[:, m, a:b],
                             start=(m == 0), stop=(m == NCH - 1))
            nc.tensor.matmul(stat_zsq[0:1, a:b], cx.ones_bf[:, 0:1],
                             zsq[:, a:b],
                             start=(m == 0), stop=(m == NCH - 1))
    # small per-position vectors, all on partition 0
    # msq = (sum_z/D)^2 on ACT (Square is in the pinned table set);
    # var = sum(z^2)/D - msq -> ln(var+eps) -> rstd = exp(-0.5*ln);
    # mr = (sum_z/D)*rstd fused in one scalar_tensor_tensor
    msq = sb.tile([1, n_seq], F32, tag="ln_msq")
    nc.scalar.activation(msq[:], stat_z[0:1, :n_seq], AF.Square, bias=0.0,
                         scale=1.0 / D)
    rstd_t = sb.tile([1, n_seq], F32, tag="ln_rstd")
    nc.vector.scalar_tensor_tensor(rstd_t[:], stat_zsq[0:1, :n_seq], 1.0 / D,
                                   msq[:], op0=ALU.mult, op1=ALU.subtract)
    nc.scalar.activation(rstd_t[:], rstd_t[:], AF.Ln, bias=cx.eps_sb[0:1, :],
                         scale=1.0)
    lnv = sb.tile([1, 2 * n_seq], BF, tag="lnv")
    nc.scalar.activation(lnv[:, 0:n_seq], rstd_t[:], AF.Exp, bias=0.0,
                         scale=-0.5)
    nc.vector.scalar_tensor_tensor(lnv[:, n_seq:2 * n_seq],
                                   stat_z[0:1, :n_seq], 1.0 / D,
                                   lnv[:, 0:n_seq], op0=ALU.mult,
                                   op1=ALU.mult)
    # broadcast rstd / mean*rstd across partitions with K=1 matmuls into
    # recycled proj-tag PSUM slots (no DRAM bounce)
    rstd_bc = psum_pool.tile([P, 1024], F32, tag=bc_tags[0], name="rstdbc")
    mr_bc = psum_pool.tile([P, 1024], F32, tag=bc_tags[1], name="mrbc")
    for (a, b) in _regions(n_seq):
        nc.tensor.matmul(rstd_bc[:, a:b], cx.ones128[:, :], lnv[0:1, a:b],
                         start=True, stop=True)
        nc.tensor.matmul(mr_bc[:, a:b], cx.ones128[:, :],
                         lnv[0:1, n_seq + a:n_seq + b], start=True, stop=True)
    for m in range(NCH):
        nc.vector.tensor_tensor(y_sb[:, m, :], z_sb[:, m, :],
                                rstd_bc[:, :n_seq], ALU.mult)
        nc.vector.tensor_tensor(y_sb[:, m, :], y_sb[:, m, :],
                                mr_bc[:, :n_seq], ALU.subtract)
        if scale_gb is not None:
            g_sb, b_sb = scale_gb
            nc.vector.tensor_scalar(
                y_sb[:, m, :], y_sb[:, m, :],
                g_sb[:, m:m + 1], b_sb[:, m:m + 1], op0=ALU.mult, op1=ALU.add)
        if out_cb is not None:
            out_cb(m)


def _attn_units(layer, n_kv):
    """Unit list: (j_list, qa, qb, exp_scale). One exp call per unit covers
    both heads of the pair (and both j's of a pair for layer 2)."""
    J = n_kv // P
    units = []
    if layer == 1:
        # wm1: (q<512, kv<512) and (q>=512, kv>=512) get W. kv-pairs never
        # straddle the 512 boundary, so scale is uniform per (q-block, pair)
        # and each unit covers two kv chunks x both heads in one exp call.
        for jp in range(J // 2):
            for b in range(3):
                sc = (W * INV_SQRT if (b * 256 < 512) == (jp * 2 * P < 512)
                      else INV_SQRT)
                units.append(([2 * jp, 2 * jp + 1], 256 * b, 256 * (b + 1),
                              sc))
    else:
        # wm2: diagonal 256-blocks get W; kv pair jp covers block jp
        for jp in range(J // 2):
            for b in range(3):
                sc = W * INV_SQRT if b == jp else INV_SQRT
                units.append(([2 * jp, 2 * jp + 1], 256 * b, 256 * (b + 1),
                              sc))
    return units


def _exp_slices(layer, j, n_q):
    """Per (attention layer, key-chunk j): (col_lo, col_hi, exp scale)."""
    if layer == 1:
        jlo = j * P < 512
        s_lo = W * INV_SQRT if jlo else INV_SQRT
        s_hi = INV_SQRT if jlo else W * INV_SQRT
        return [(0, 512, s_lo), (512, n_q, s_hi)]
    blk = j // 2
    raw = [(b * 256, min((b + 1) * 256, n_q),
            W * INV_SQRT if b == blk else INV_SQRT) for b in range(3)]
    out = [raw[0]]
    for (lo, hi, sc) in raw[1:]:
        plo, phi, psc = out[-1]
        if sc == psc and lo == phi:
            out[-1] = (plo, hi, sc)
        else:
            out.append((lo, hi, sc))
    return out


def _emit_attn_base(nc, cx, work, psum_s, psum_ot, qT_sb, kT_sb, v_sb, ot_sb,
                    layer, n_q, n_kv):
    """Baseline-style attention: per-head S psum tiles + sliced exp."""
    J = n_kv // P
    heads = lambda c: ((slice(0, 64), 2 * c), (slice(64, 128), 2 * c + 1))
    units = [(j, a, b) for j in range(J) for (a, b) in _regions(n_q)]

    def emit_S_unit(c, u, etiles):
        (hb_e, h_e), (hb_o, h_o) = heads(c)
        j, a, b = units[u]
        if j not in etiles:
            etiles[j] = (
                work.tile([P, n_q], BF, tag="exps", bufs=4, name=f"ee{c}_{j}"),
                work.tile([P, n_q], BF, tag="exps", bufs=4, name=f"eo{c}_{j}"),
            )
        e_e, e_o = etiles[j]
        ps_e = psum_s.tile([P, 512], F32, tag="s", name=f"se{c}_{j}_{a}")
        ps_o = psum_s.tile([P, 512], F32, tag="s", name=f"so{c}_{j}_{a}")
        nc.tensor.matmul(ps_e[:, :b - a],
                         kT_sb[hb_e, c, j * P:(j + 1) * P],
                         qT_sb[hb_e, c, a:b], start=True, stop=True)
        nc.tensor.matmul(ps_o[:, :b - a],
                         kT_sb[hb_o, c, j * P:(j + 1) * P],
                         qT_sb[hb_o, c, a:b], start=True, stop=True)
        for e, ps in ((e_e, ps_e), (e_o, ps_o)):
            for (lo, hi, sc) in _exp_slices(layer, j, n_q):
                lo2, hi2 = max(lo, a), min(hi, b)
                if lo2 < hi2:
                    nc.scalar.activation(
                        e[:, lo2:hi2], ps[:, lo2 - a:hi2 - a],
                        AF.Exp, bias=0.0, scale=sc)

    def emit_O_unit(c, u, etiles, po_all):
        j, a, b = units[u]
        e_e, e_o = etiles[j]
        for (hb, h), e in zip(heads(c), (e_e, e_o)):
            nc.tensor.matmul(
                po_all[h][0:65, a:b],
                v_sb[:, j, h, 0:65],
                e[:, a:b],
                start=(j == 0), stop=(j == J - 1))

    for c in range(H // 2):
        po_all = {}
        for hb, h in heads(c):
            po_all[h] = psum_ot.tile([65, 768], F32, tag="ot", name=f"po{h}")
        etiles = {}
        emit_S_unit(c, 0, etiles)
        for u in range(len(units)):
            if u + 1 < len(units):
                emit_S_unit(c, u + 1, etiles)
            emit_O_unit(c, u, etiles, po_all)
        srow = work.tile([1, 2 * n_q], F32, tag="srow", bufs=2,
                         name=f"sr{c}")
        rr = work.tile([1, 2 * n_q], F32, tag="rr", bufs=2, name=f"rr{c}")
        (hb_e, h_e), (hb_o, h_o) = heads(c)
        # stage denominator rows at partition 0 in SBUF: the custom-DVE
        # reciprocal misreads PSUM at base partition 64 on hardware
        nc.vector.tensor_copy(srow[:, 0:n_q], po_all[h_e][64:65, 0:n_q])
        nc.vector.tensor_copy(srow[:, n_q:2 * n_q],
                              po_all[h_o][64:65, 0:n_q])
        nc.vector.reciprocal_approx_fast(out=rr[:, 0:n_q],
                                         in_=srow[:, 0:n_q])
        nc.vector.reciprocal_approx_fast(out=rr[:, n_q:2 * n_q],
                                         in_=srow[:, n_q:2 * n_q])
        nc.scalar.copy(ot_sb[hb_e, c, :], po_all[h_e][0:64, :n_q])
        nc.scalar.copy(ot_sb[hb_o, c, :], po_all[h_o][0:64, :n_q])
        drp = cx.dram.tile([2, n_q], F32, tag="drp", name=f"drp{c}")
        nc.sync.dma_start(drp[0:1, :], rr[:, 0:n_q])
        nc.sync.dma_start(drp[1:2, :], rr[:, n_q:2 * n_q])
        bc = work.tile([P, n_q], F32, tag="attn_bc", name=f"bc{c}")
        nc.gpsimd.dma_start(bc[0:64, :], drp[0:1, :].to_broadcast([64, n_q]))
        nc.gpsimd.dma_start(bc[64:128, :],
                            drp[1:2, :].to_broadcast([64, n_q]))
        nc.vector.tensor_tensor(ot_sb[:, c, :], ot_sb[:, c, :], bc[:],
                                ALU.mult)


def _emit_attn(nc, cx, work, psum_s, psum_ot, qT_sb, kT_sb, v_sb, ot_sb,
               layer, n_q, n_kv, filler=None):
    """Cross-attention. Per unit: the heads of a pair (x kv-pair for layer 2)
    are emitted as ADJACENT K=64 matmuls into PE row-groups 0/64 (concurrent)
    writing one shared PSUM tile; ONE ACT exp call (wm scale folded in)
    covers the whole tile. O.T accumulates per unit with a ones-augmented V
    column so the softmax denominator lands in PSUM row 64. S for unit u+1 is
    emitted before O of unit u so the PE has independent work while ACT
    computes exp.

    filler: optional per-pair list of callables emitting independent
    full-array PE work (borrowing an s-tag PSUM slot). Attention's K=64 /
    65-row matmuls only half-use the PE array and never re-warm the HAM
    clock gate; dense filler matmuls keep it at 2.4 GHz."""
    J = n_kv // P
    units = _attn_units(layer, n_kv)
    heads = lambda c: ((slice(0, 64), 2 * c), (slice(64, 128), 2 * c + 1))

    def ekey(jl):
        return jl[0]

    def emit_S_unit(c, u, etiles):
        jl, a, b, sc = units[u]
        w = b - a
        nj = len(jl)
        if ekey(jl) not in etiles:
            etiles[ekey(jl)] = work.tile([P, nj * 2, n_q], BF, tag="exps",
                                         bufs=3, name=f"e{c}_{ekey(jl)}")
        E = etiles[ekey(jl)]
        # Bank-safe layout: all head-even S in bank 0 ([0:512]), head-odd in
        # bank 1 ([512:1024]) — the two heads' matmuls execute CONCURRENTLY
        # via PE row-groups 0/64, and concurrent drains into the SAME PSUM
        # bank are a hardware fault. u-order (all-he, then all-ho) keeps the
        # psum stride regular (w) so one strided ACT exp covers the tile.
        ps = psum_s.tile([P, 1024], F32, tag="s", name=f"s{c}_{u}")
        (hb_e, h_e), (hb_o, h_o) = heads(c)
        for ji, j in enumerate(jl):
            nc.tensor.matmul(ps[:, ji * w:(ji + 1) * w],
                             kT_sb[hb_e, c, j * P:(j + 1) * P],
                             qT_sb[hb_e, c, a:b], start=True, stop=True)
            nc.tensor.matmul(ps[:, 512 + ji * w:512 + (ji + 1) * w],
                             kT_sb[hb_o, c, j * P:(j + 1) * P],
                             qT_sb[hb_o, c, a:b], start=True, stop=True)
        ps_g = ps[:, 0:1024].rearrange("p (g q) -> p g q", g=2)
        if nj == 1:
            nc.scalar.activation(E[:, :, a:b], ps_g[:, :, 0:w],
                                 AF.Exp, bias=0.0, scale=sc)
        else:
            nc.scalar.activation(
                E[:, :, a:b].rearrange("p (g j) q -> p g j q", g=2),
                ps_g[:, :, 0:nj * w].rearrange("p g (j q) -> p g j q", j=nj),
                AF.Exp, bias=0.0, scale=sc)

    def emit_O_unit(c, u, etiles, po_all):
        jl, a, b, sc = units[u]
        nj = len(jl)
        E = etiles[ekey(jl)]
        # start/stop must be unique per PSUM BANK (start=True clears the
        # whole bank's has_written bits): only the first/last matmul touching
        # a bank carries the flag; sibling regions in the same bank rely on
        # per-element overwrite-then-accumulate semantics.
        first_in_bank = a % 512 == 0
        last_in_bank = (b % 512 == 0) or (b == n_q)
        for ji, j in enumerate(jl):
            for hi, (hb, h) in enumerate(heads(c)):
                nc.tensor.matmul(
                    po_all[h][0:65, a:b],
                    v_sb[:, j, h, 0:65],
                    E[:, hi * nj + ji, a:b],
                    start=(j == 0 and first_in_bank),
                    stop=(j == J - 1 and last_in_bank))

    fill_i = [0]

    def feed_filler(c):
        # filler entries are (gate, fn): fn may only be emitted once pair
        # `gate` has been normalized (so a stalled piece never parks on an
        # s-slot the attention pipeline needs)
        if filler is not None and fill_i[0] < len(filler):
            gate, fn = filler[fill_i[0]]
            if gate < c:
                fn()
                fill_i[0] += 1

    for c in range(H // 2):
        po_all = {}
        for hb, h in heads(c):
            po_all[h] = psum_ot.tile([65, 768], F32, tag="ot", name=f"po{h}")
        etiles = {}
        emit_S_unit(c, 0, etiles)
        for u in range(len(units)):
            if u + 1 < len(units):
                emit_S_unit(c, u + 1, etiles)
            emit_O_unit(c, u, etiles, po_all)
            if layer == 2 or u % 2 == 1:
                feed_filler(c)
        # normalizers: stage denominator rows at partition 0 (custom-DVE
        # reciprocal misreads PSUM at base partition 64 on hardware), recip,
        # DRAM-bounce broadcast. The copies are split across ACT and DVE so
        # the serial tail chain pipelines across both engines.
        srow = work.tile([1, 2 * n_q], F32, tag="srow", bufs=2,
                         name=f"sr{c}")
        rr = work.tile([1, 2 * n_q], F32, tag="rr", bufs=2, name=f"rr{c}")
        (hb_e, h_e), (hb_o, h_o) = heads(c)
        nc.scalar.copy(srow[:, 0:n_q], po_all[h_e][64:65, 0:n_q])
        nc.vector.tensor_copy(srow[:, n_q:2 * n_q],
                              po_all[h_o][64:65, 0:n_q])
        nc.vector.reciprocal_approx_fast(out=rr[:, 0:n_q],
                                         in_=srow[:, 0:n_q])
        nc.vector.reciprocal_approx_fast(out=rr[:, n_q:2 * n_q],
                                         in_=srow[:, n_q:2 * n_q])
        nc.scalar.copy(ot_sb[hb_e, c, :], po_all[h_e][0:64, :n_q])
        nc.vector.tensor_copy(ot_sb[hb_o, c, :], po_all[h_o][0:64, :n_q])
        drp = cx.dram.tile([2, n_q], F32, tag="drp", name=f"drp{c}")
        nc.sync.dma_start(drp[0:1, :], rr[:, 0:n_q])
        nc.sync.dma_start(drp[1:2, :], rr[:, n_q:2 * n_q])
        bc = work.tile([P, n_q], F32, tag="attn_bc", name=f"bc{c}")
        nc.gpsimd.dma_start(bc[0:64, :], drp[0:1, :].to_broadcast([64, n_q]))
        nc.gpsimd.dma_start(bc[64:128, :],
                            drp[1:2, :].to_broadcast([64, n_q]))
        nc.vector.tensor_tensor(ot_sb[:, c, :], ot_sb[:, c, :], bc[:],
                                ALU.mult)
        feed_filler(c + 1)
    # flush remaining filler pieces (their deps are all satisfied now)
    if filler is not None:
        while fill_i[0] < len(filler):
            filler[fill_i[0]][1]()
            fill_i[0] += 1


def _r3(ap):
    """DRAM [K*128, n] -> [128(part), K, n] view for DMA."""
    return ap.rearrange("(ko p) s -> p ko s", p=P)


def _build_program(flags):
    use_bo1, use_bo2, use_fb1, use_fb2, use_g1, use_g2, use_g3 = flags
    nc = _Bacc("TRN2", target_bir_lowering=False, debug=False)

    def din(name, shape, dt=BF):
        return nc.dram_tensor(name, shape, dt, kind="ExternalInput").ap()

    x1T = din("x1T", [D, S1])
    x2T = din("x2T", [D, S2])
    x3T = din("x3T", [D, S3])
    wts = {n: din(n, [D, D]) for n in
           ("wq1T", "wk1T", "wv1T", "wo1T", "wq2T", "wk2T", "wv2T", "wo2T")}
    fw1T = din("fw1T", [D, DF])
    fw2T = din("fw2T", [DF, D])
    bo1 = din("bo1", [P, NCH]) if use_bo1 else None
    bo2 = din("bo2", [P, NCH]) if use_bo2 else None
    fb1 = din("fb1", [P, DF // P]) if use_fb1 else None
    fb2 = din("fb2", [P, NCH]) if use_fb2 else None
    gbd = {}
    for i, use in ((1, use_g1), (2, use_g2), (3, use_g3)):
        gbd[i] = (din(f"g{i}", [P, NCH]),
                  din(f"b{i}", [P, NCH])) if use else None
    yT = nc.dram_tensor("yT", [D, S1], F32, kind="ExternalOutput").ap()
    taps = {}
    if _DEBUG_TAPS:
        for tn in ("t_q1", "t_ot1", "t_y1", "t_ot2", "t_y2"):
            taps[tn] = nc.dram_tensor(tn, [D, S1], BF,
                                      kind="ExternalOutput").ap()

    def tap(name, src):
        if _DEBUG_TAPS:
            nc.sync.dma_start(_r3(taps[name]), src[:])

    with tile.TileContext(nc, pool_alloc_mode="queue") as tc:
        cx = _Ctx()
        cx.tc = tc
        with tc.tile_pool(name="sb", bufs=1) as sb, \
             tc.tile_pool(name="zsq", bufs=1) as zsq_pool, \
             tc.tile_pool(name="wpre", bufs=1) as wpre, \
             tc.tile_pool(name="dram", bufs=2, space="DRAM") as dram:
            cx.sb, cx.zsq_pool, cx.dram = sb, zsq_pool, dram

            ones_bf = sb.tile([P, 1], BF, tag="ones_bf")
            nc.vector.memset(ones_bf[:], 1.0)
            cx.ones_bf = ones_bf
            ones128 = sb.tile([1, P], BF, tag="ones128")
            nc.vector.memset(ones128[:], 1.0)
            cx.ones128 = ones128
            eps_sb = sb.tile([P, 1], F32, tag="eps")
            nc.vector.memset(eps_sb[:], EPS)
            cx.eps_sb = eps_sb

            def load_pm(ap, cols, tag):
                if ap is None:
                    return None
                t = sb.tile([P, cols], BF, tag=tag)
                nc.sync.dma_start(t[:], ap)
                return t

            bo1_sb = load_pm(bo1, NCH, "bo1")
            bo2_sb = load_pm(bo2, NCH, "bo2")
            fb1_sb = load_pm(fb1, DF // P, "fb1")
            fb2_sb = load_pm(fb2, NCH, "fb2")
            gb_sb = {}
            for i in (1, 2, 3):
                gb_sb[i] = None if gbd[i] is None else (
                    load_pm(gbd[i][0], NCH, f"g{i}"),
                    load_pm(gbd[i][1], NCH, f"b{i}"))

            y1_sb = sb.tile([P, NCH, S1], BF, tag="y1")
            y2_sb = sb.tile([P, NCH, S1], BF, tag="y2")

            def copy_cb(dst, eng):
                return lambda m, ps: eng(dst[:, m, :], ps)

            dve_copy = lambda out, ps: nc.vector.tensor_copy(out, ps)
            act_copy = (lambda out, ps: nc.scalar.copy(out, ps)) \
                if _USE_ACT_COPY else dve_copy

            def emit_v_proj(psum_pool, x_sb, wv_sb, v_sb, Jkv):
                nc.vector.memset(v_sb[:, :, :, 64:65], 1.0)
                for j in range(Jkv):
                    ps = psum_pool.tile([P, 1024], F32, tag="proj",
                                        name=f"vps{j}")
                    for ko in range(NCH):
                        nc.tensor.matmul(
                            ps[:, 0:D],
                            x_sb[:, ko, j * P:(j + 1) * P],
                            wv_sb[:, ko, :],
                            start=(ko == 0), stop=(ko == NCH - 1))
                    nc.vector.tensor_copy(
                        v_sb[:, j, :, 0:64],
                        ps[:, 0:D].rearrange("p (h v) -> p h v", h=H))

            # open order is reverse of close order (pool stack is LIFO)
            kv2 = tc.tile_pool(name="kv2", bufs=1)
            with kv2 as kv2p:
                x3_sb = kv2p.tile([P, NCH, S3], BF, tag="xkv")
                wk2_sb = kv2p.tile([P, NCH, D], BF, tag="wk")
                wv2_sb = kv2p.tile([P, NCH, D], BF, tag="wv")
                k2_sb = kv2p.tile([P, NCH, S3], BF, tag="k")
                v2_sb = kv2p.tile([P, S3 // P, H, 65], BF, tag="v")

                otp1 = contextlib.ExitStack()
                otp1p = otp1.enter_context(tc.tile_pool(name="otp1", bufs=1))
                x1_sb = otp1p.tile([P, NCH, S1], BF, tag="x1")
                ot_sb = otp1p.tile([P, NCH, S1], BF, tag="ot1")

                at1_ctx = contextlib.ExitStack()
                at1p = at1_ctx.enter_context(tc.tile_pool(name="at1", bufs=1))
                q_sb = at1p.tile([P, NCH, S1], BF, tag="q")
                k_sb = at1p.tile([P, NCH, S2], BF, tag="k")
                v_sb = at1p.tile([P, S2 // P, H, 65], BF, tag="v")

                kv1_ctx = contextlib.ExitStack()
                kv1p = kv1_ctx.enter_context(tc.tile_pool(name="kv1", bufs=1))
                wq_sb = kv1p.tile([P, NCH, D], BF, tag="wq")
                x2_sb = kv1p.tile([P, NCH, S2], BF, tag="xkv")
                wk_sb = kv1p.tile([P, NCH, D], BF, tag="wk")
                wv_sb = kv1p.tile([P, NCH, D], BF, tag="wv")
                # per-chunk DMAs in consumption order, issued across FOUR
                # engine DGE queues in parallel (descriptor generation is
                # ~1us serial per engine): the ko-major Q1/K1 projections
                # start as soon as their first chunks land
                for ko in range(NCH):
                    nc.sync.dma_start(wq_sb[:, ko, :],
                                      _r3(wts["wq1T"])[:, ko, :])
                    nc.gpsimd.dma_start(x1_sb[:, ko, :], _r3(x1T)[:, ko, :])
                for ko in range(NCH):
                    nc.scalar.dma_start(wk_sb[:, ko, :],
                                        _r3(wts["wk1T"])[:, ko, :])
                    nc.scalar.dma_start(x2_sb[:, ko, :], _r3(x2T)[:, ko, :])
                nc.sync.dma_start(wv_sb[:], _r3(wts["wv1T"]))
                nc.gpsimd.dma_start(x3_sb[:], _r3(x3T))
                nc.sync.dma_start(wk2_sb[:], _r3(wts["wk2T"]))
                nc.sync.dma_start(wv2_sb[:], _r3(wts["wv2T"]))

                # prefetch every later-phase weight now (bf16 fits in SBUF)
                wo1_sb = wpre.tile([P, NCH, D], BF, tag="wo1")
                wq2_sb = wpre.tile([P, NCH, D], BF, tag="wq2")
                wo2_sb = wpre.tile([P, NCH, D], BF, tag="wo2")
                fw1_sb = wpre.tile([P, NCH, DF], BF, tag="fw1")
                fw2_sb = wpre.tile([P, DF // P, D], BF, tag="fw2")
                nc.sync.dma_start(wo1_sb[:], _r3(wts["wo1T"]))
                nc.sync.dma_start(wq2_sb[:], _r3(wts["wq2T"]))
                nc.sync.dma_start(wo2_sb[:], _r3(wts["wo2T"]))
                nc.sync.dma_start(fw1_sb[:], _r3(fw1T))
                nc.sync.dma_start(fw2_sb[:], _r3(fw2T))

                # dense warm-up block: q1,k1,v1 (q1/k1 ko-major so the PE
                # starts on the first DMA'd chunks); k2/v2 are deferred into
                # the WO1/LN1 and WO2/LN2 phases as independent PE filler
                with tc.tile_pool(name="psA", bufs=4, space="PSUM") as psA:
                    _emit_proj(nc, psA, wq_sb, x1_sb, D, S1, NCH,
                               copy_cb(q_sb, act_copy), ko_major=True)
                    _emit_proj(nc, psA, wk_sb, x2_sb, D, S2, NCH,
                               copy_cb(k_sb, act_copy), ko_major=True)
                    emit_v_proj(psA, x2_sb, wv_sb, v_sb, S2 // P)
                kv1_ctx.close()  # frees x2 + wq1/wk1/wv1 SBUF

                # attention 1, with K2/V2 projection pieces as full-array
                # PE filler woven between j-groups (keeps the HAM clock warm
                # through the half-array attention matmuls)
                attn_fn = _emit_attn_base if _USE_BASE_ATTN else _emit_attn
                s_bufs = 4 if _USE_BASE_ATTN else 2
                nc.vector.memset(v2_sb[:, :, :, 64:65], 1.0)
                with tc.tile_pool(name="wk1w", bufs=3) as work, \
                     tc.tile_pool(name="ps_s1", bufs=s_bufs,
                                  space="PSUM") as pss, \
                     tc.tile_pool(name="ps_ot1", bufs=2,
                                  space="PSUM") as psot:
                    def mk_k2(m, a, b):
                        def f():
                            ps = pss.tile([P, 1024], F32, tag="s",
                                          name=f"fk2_{m}_{a}")
                            for ko in range(NCH):
                                nc.tensor.matmul(
                                    ps[:, 0:b - a],
                                    wk2_sb[:, ko, m * P:(m + 1) * P],
                                    x3_sb[:, ko, a:b],
                                    start=(ko == 0), stop=(ko == NCH - 1))
                            nc.vector.tensor_copy(k2_sb[:, m, a:b],
                                                  ps[:, 0:b - a])
                        return f

                    def mk_v2(j, half):
                        def f():
                            ps = pss.tile([P, 1024], F32, tag="s",
                                          name=f"fv2_{j}_{half}")
                            a = half * 256
                            for ko in range(NCH):
                                nc.tensor.matmul(
                                    ps[:, 0:256],
                                    x3_sb[:, ko, j * P:(j + 1) * P],
                                    wv2_sb[:, ko, a:a + 256],
                                    start=(ko == 0), stop=(ko == NCH - 1))
                            nc.vector.tensor_copy(
                                v2_sb[:, j, 4 * half:4 * half + 4, 0:64],
                                ps[:, 0:256].rearrange("p (h v) -> p h v",
                                                       h=4))
                        return f

                    fillers = []
                    for m in range(NCH):
                        fillers.append((-1, mk_k2(m, 0, 256)))
                        fillers.append((-1, mk_k2(m, 256, 512)))
                        fillers.append((-1, mk_k2(m, 512, 768)))
                    for j in range(S3 // P):
                        fillers.append((-1, mk_v2(j, 0)))
                        fillers.append((-1, mk_v2(j, 1)))
                    if _USE_BASE_ATTN:
                        attn_fn(nc, cx, work, pss, psot, q_sb, k_sb, v_sb,
                                ot_sb, 1, S1, S2)
                    else:
                        attn_fn(nc, cx, work, pss, psot, q_sb, k_sb, v_sb,
                                ot_sb, 1, S1, S2, filler=fillers)
                tap("t_q1", q_sb)
                tap("t_ot1", ot_sb)
                at1_ctx.close()  # frees q1/k1/v1 SBUF

                # wo1 + LN1 (wo1 ko-major in 2-m groups: the first WO matmuls
                # need only attention pair 0's normalized output)
                with tc.tile_pool(name="psB1", bufs=2, space="PSUM") as psB:
                    wo_ps = {}
                    _emit_proj(nc, psB, wo1_sb, ot_sb, D, S1, NCH,
                               lambda m, ps: wo_ps.__setitem__(m, ps),
                               ko_major=True, m_group=2)
                    _emit_ln(nc, cx, psB, lambda m: wo_ps[m], x1_sb,
                             bo1_sb, y1_sb, y1_sb, S1, gb_sb[1])
                tap("t_y1", y1_sb)
                otp1.close()

                # q2 projection (ko-major: starts as soon as y1 chunk 0 is
                # normalized)
                otp2 = contextlib.ExitStack()
                otp2p = otp2.enter_context(tc.tile_pool(name="otp2", bufs=1))
                ot2_sb = otp2p.tile([P, NCH, S1], BF, tag="ot2")
                q2_sb = otp2p.tile([P, NCH, S1], BF, tag="q2")
                with tc.tile_pool(name="psC", bufs=4, space="PSUM") as psC:
                    _emit_proj(nc, psC, wq2_sb, y1_sb, D, S1, NCH,
                               copy_cb(q2_sb, act_copy), ko_major=_KO_MAJOR)

                # attention 2
                with tc.tile_pool(name="wk2w", bufs=3) as work2, \
                     tc.tile_pool(name="ps_s2", bufs=s_bufs,
                                  space="PSUM") as pss2, \
                     tc.tile_pool(name="ps_ot2", bufs=2,
                                  space="PSUM") as psot2:
                    attn_fn(nc, cx, work2, pss2, psot2, q2_sb, k2_sb,
                            v2_sb, ot2_sb, 2, S1, S3)
                tap("t_ot2", ot2_sb)

                # wo2 + LN2
                with tc.tile_pool(name="psD", bufs=2, space="PSUM") as psD:
                    wo2_ps = {}
                    _emit_proj(nc, psD, wo2_sb, ot2_sb, D, S1, NCH,
                               lambda m, ps: wo2_ps.__setitem__(m, ps),
                               ko_major=True, m_group=2)
                    _emit_ln(nc, cx, psD, lambda m: wo2_ps[m], y1_sb,
                             bo2_sb, y2_sb, y2_sb, S1, gb_sb[2])
                tap("t_y2", y2_sb)
                otp2.close()

            # FFN + LN3
            zbuf = sb.tile([P, NCH, S1], BF, tag="y1")  # reuse y1 slot
            yT_sb = sb.tile([P, NCH, S1], F32, tag="yT")
            with tc.tile_pool(name="ffn1", bufs=1) as f1p:
                h_sb = f1p.tile([P, DF // P, S1], BF, tag="hT")
                with tc.tile_pool(name="psE", bufs=3, space="PSUM") as psE:
                    def gelu_consume(m, ps):
                        nc.scalar.activation(
                            h_sb[:, m, :], ps[:, 0:S1], AF.Gelu,
                            bias=(fb1_sb[:, m:m + 1]
                                  if fb1_sb is not None else 0.0),
                            scale=1.0)
                    # ko-major pairs: the first FFN1 matmuls need only y2
                    # chunk 0, starting inside LN2's normalize window
                    _emit_proj(nc, psE, fw1_sb, y2_sb, DF, S1, NCH,
                               gelu_consume, ko_major=True, m_group=2)

                with tc.tile_pool(name="psF", bufs=2, space="PSUM") as psF:
                    f2_ps = {}
                    _emit_proj(nc, psF, fw2_sb, h_sb, D, S1, DF // P,
                               lambda m, ps: f2_ps.__setitem__(m, ps))

                    out_eng = [nc.sync, nc.scalar, nc.gpsimd, nc.sync]

                    def out_dma(m):
                        out_eng[m].dma_start(_r3(yT)[:, m, :],
                                             yT_sb[:, m, :])

                    _emit_ln(nc, cx, psF, lambda m: f2_ps[m], y2_sb, fb2_sb,
                             zbuf, yT_sb, S1, gb_sb[3], out_cb=out_dma)

    nc.finalize()
    return nc


def _to_pm(vec, cols):
    """[cols*128] vector -> [128, cols] partition-major layout (bf16)."""
    return np.ascontiguousarray(vec.reshape(cols, P).T).astype(
        ml_dtypes.bfloat16)


def _bf(a):
    return np.ascontiguousarray(a).astype(ml_dtypes.bfloat16)


def kernel(**inputs):
    cords = np.asarray(inputs["cords_features"], np.float32)
    spatial = np.asarray(inputs["spatial_features"], np.float32)
    speed = np.asarray(inputs["speed_features"], np.float32)
    B = cords.shape[0]
    assert B == 8

    def g(name):
        return np.asarray(inputs[name], np.float32)

    flags = (
        not np.allclose(g("bo1"), 0), not np.allclose(g("bo2"), 0),
        not np.allclose(g("ffn_b1"), 0), not np.allclose(g("ffn_b2"), 0),
        not (np.allclose(g("ln1_g"), 1) and np.allclose(g("ln1_b"), 0)),
        not (np.allclose(g("ln2_g"), 1) and np.allclose(g("ln2_b"), 0)),
        not (np.allclose(g("ln3_g"), 1) and np.allclose(g("ln3_b"), 0)),
    )
    if flags not in _PROGRAM_CACHE:
        _PROGRAM_CACHE[flags] = _build_program(flags)
    nc = _PROGRAM_CACHE[flags]

    shared = {
        "wq1T": _bf(g("wq1").T), "wk1T": _bf(g("wk1").T),
        "wv1T": _bf(g("wv1").T), "wo1T": _bf(g("wo1").T),
        "wq2T": _bf(g("wq2").T), "wk2T": _bf(g("wk2").T),
        "wv2T": _bf(g("wv2").T), "wo2T": _bf(g("wo2").T),
        "fw1T": _bf(g("ffn_w1").T), "fw2T": _bf(g("ffn_w2").T),
    }
    use_bo1, use_bo2, use_fb1, use_fb2, use_g1, use_g2, use_g3 = flags
    if use_bo1:
        shared["bo1"] = _to_pm(g("bo1"), NCH)
    if use_bo2:
        shared["bo2"] = _to_pm(g("bo2"), NCH)
    if use_fb1:
        shared["fb1"] = _to_pm(g("ffn_b1"), DF // P)
    if use_fb2:
        shared["fb2"] = _to_pm(g("ffn_b2"), NCH)
    for i, use in ((1, use_g1), (2, use_g2), (3, use_g3)):
        if use:
            shared[f"g{i}"] = _to_pm(g(f"ln{i}_g"), NCH)
            shared[f"b{i}"] = _to_pm(g(f"ln{i}_b"), NCH)

    in_maps = []
    for b in range(B):
        m = dict(shared)
        m["x1T"] = _bf(cords[b].T)
        m["x2T"] = _bf(spatial[b].T)
        m["x3T"] = _bf(speed[b].T)
        in_maps.append(m)

    global _LAST_IN_MAPS
    _LAST_IN_MAPS = in_maps
    res = run_bass_kernel_spmd(nc, in_maps, core_ids=list(range(B)))
    out = np.stack([res.results[b]["yT"].T for b in range(B)], axis=0)
    return np.ascontiguousarray(out.astype(np.float32))


# revision 73
# speedup vs baseline: 1.2840x; 1.0264x over previous
"""DualCrossAttention Trainium2 kernel (bf16).

Data-parallel: batch=8 across 8 NeuronCores, one batch element per core.
Per core: two cross-attentions + FFN + 3 LayerNorms on [768, 512] activations.

Layout: feature-major activations (x.T: [feature(part), seq(free)]); weights
host-pre-transposed and cast to bf16 so every projection is a full-rate PE
matmul (bf16 streams 1 cycle/row vs 1.5 for fp32-HIGH, and enables FWL).
Attention: S.T = k_h @ q_h.T with the two heads of a pair emitted as adjacent
matmuls into PE row-groups 0/64 (concurrent execution) writing one shared
PSUM tile, so a single wide ACT exp call covers both heads; the wm scale is
folded into the exp affine. O.T accumulates with a ones-augmented V column so
the softmax denominator lands in PSUM row 64. All per-position normalizers
(attn 1/denominator, LN rstd/mean*rstd) are broadcast across partitions with
tiny K=1 PE matmuls into PSUM — no DRAM bounce. All weights are prefetched
at kernel start (bf16 halves the SBUF/DMA footprint).
"""
import contextlib

import numpy as np
import ml_dtypes

import concourse.bacc as bacc
import concourse.bass as bass
import concourse.tile as tile
from concourse import mybir
from concourse.bass_utils import run_bass_kernel_spmd
class _Bacc(bacc.Bacc):
    """Bacc with Exp/Ln pinned to the natural_log_exp_and_others ACT table
    set: the default chooser alternates between exp_and_others (attention
    softmax) and natural_log (LayerNorm rstd), paying a ~2.7us table load at
    every switch. Removing Exp/Ln from the single-function sets makes both
    resolve to the combined set, leaving only the Gelu switches."""

    def insert_act_table_loads(self):
        from concourse.hw_specs import get_activation_tables
        import bass_rust as _bass_rust
        has_activation = any(
            isinstance(i, mybir.InstActivation)
            for b in self.main_func.blocks
            for i in b.instructions
        )
        if not has_activation:
            return
        AFt = mybir.ActivationFunctionType
        tables = []
        for name, fns in get_activation_tables(self.m.arch).items():
            if name == "exp_and_others":
                fns = fns - {AFt.Exp}
            elif name == "natural_log":
                fns = fns - {AFt.Ln}
            tables.append((name, fns))
        _bass_rust.insert_act_table_loads(self, tables)


F32 = mybir.dt.float32
F32R = mybir.dt.float32r
BF = mybir.dt.bfloat16
AF = mybir.ActivationFunctionType
ALU = mybir.AluOpType

H, KD, VD = 8, 64, 64
D, DF = 512, 2048
S1, S2, S3 = 768, 1024, 768
P = 128
NCH = D // P            # 4 feature chunks of the 512-dim residual stream
W = 1.25                # wm weight scale
INV_SQRT = 0.125        # 1/sqrt(64)
EPS = 1e-5

_PROGRAM_CACHE = {}
_USE_BASE_ATTN = False
_USE_ACT_COPY = True
_KO_MAJOR = True
_DEBUG_TAPS = False


def _regions(n):
    """Split free dim n into <=512 column regions (PSUM-bank aligned)."""
    out = []
    s = 0
    while s < n:
        e = min(s + 512, n)
        out.append((s, e))
        s = e
    return out


class _Ctx:
    """Shared handles for the emit helpers."""
    pass


def _emit_proj(nc, psum_pool, wT_sb, xT_sb, n_out, n_seq, k_chunks, consume,
               ko_major=False, m_group=None, tag="proj"):
    """out.T[o, i] = sum_d wT[d, o] * xT[d, i]; calls consume(m, psum_ap).

    ko_major=True holds m-tiles live and loops ko outer / m inner, so the
    first matmuls only need xT chunk 0 (pipelines into a producer of xT).
    m_group limits how many m-tiles are live at once (PSUM pressure): the
    m-range is processed in groups, each group ko-major.
    """
    if ko_major:
        n_m = n_out // P
        if m_group is None:
            m_group = n_m
        for m0 in range(0, n_m, m_group):
            ms = range(m0, min(m0 + m_group, n_m))
            tiles = {m: psum_pool.tile([P, 1024], F32, tag=tag,
                                       name=f"pp{tag}{m}") for m in ms}
            for ko in range(k_chunks):
                for m in ms:
                    for (a, b) in _regions(n_seq):
                        nc.tensor.matmul(
                            tiles[m][:, a:b],
                            wT_sb[:, ko, m * P:(m + 1) * P],
                            xT_sb[:, ko, a:b],
                            start=(ko == 0), stop=(ko == k_chunks - 1),
                        )
            for m in ms:
                consume(m, tiles[m][:, :n_seq])
        return
    for m in range(n_out // P):
        ps = psum_pool.tile([P, 1024], F32, tag="proj", name=f"pp{m}")
        for ko in range(k_chunks):
            for (a, b) in _regions(n_seq):
                nc.tensor.matmul(
                    ps[:, a:b],
                    wT_sb[:, ko, m * P:(m + 1) * P],
                    xT_sb[:, ko, a:b],
                    start=(ko == 0), stop=(ko == k_chunks - 1),
                )
        consume(m, ps[:, :n_seq])


def _emit_ln(nc, cx, psum_pool, z_src, resid, bias_pm, z_sb, y_sb, n_seq,
             scale_gb, out_cb=None, bc_tags=("proj", "proj")):
    """LayerNorm over the feature axis (partitions x NCH chunks).

    z_src(m) -> psum AP [P, n_seq] (projection output chunk m);
    z = psum + bias + resid is built in z_sb (bf16); stats via ones-matmuls
    (partition reduction on PE); rstd via exp(-0.5 ln(var+eps)); rstd and
    mean*rstd are broadcast to [P, n_seq] with K=1 PE matmuls into recycled
    proj-tag PSUM slots; y_sb = z*rstd_bc - mr_bc (may alias z_sb).
    """
    sb = cx.sb
    stat_z = psum_pool.tile([1, 768], F32, tag="ln_stat_z", bufs=1)
    stat_zsq = psum_pool.tile([1, 768], F32, tag="ln_stat_zsq", bufs=1)
    for m in range(NCH):
        ps = z_src(m)
        if bias_pm is not None:
            nc.vector.scalar_tensor_tensor(
                z_sb[:, m, :], ps, bias_pm[:, m:m + 1], resid[:, m, :],
                op0=ALU.add, op1=ALU.add)
        else:
            nc.vector.tensor_tensor(z_sb[:, m, :], ps, resid[:, m, :], ALU.add)
        zsq = cx.zsq_pool.tile([P, 768], BF, tag="ln_zsq", name=f"zsq{m}")
        # last chunk's square on DVE (shorter critical path); others on the
        # otherwise-idle gpsimd
        sq_eng = nc.vector if m == NCH - 1 else nc.gpsimd
        sq_eng.tensor_tensor(zsq[:, :n_seq], z_sb[:, m, :], z_sb[:, m, :],
                             ALU.mult)
        for (a, b) in _regions(n_seq):
            nc.tensor.matmul(stat_z[0:1, a:b], cx.ones_bf[:, 0:1],
                             z_sb[:, m, a:b],
                             start=(m == 0), stop=(m == NCH - 1))
            nc.tensor.matmul(stat_zsq[0:1, a:b], cx.ones_bf[:, 0:1],
                             zsq[:, a:b],
                             start=(m == 0), stop=(m == NCH - 1))
    # y = (z - mean) * rstd, ordered so only the rstd-multiply trails the
    # serial Square->var->Ln->Exp chain: the mean broadcast and the
    # mean-subtract of every chunk run as soon as the stats land.
    lnv = sb.tile([1, 2 * n_seq], BF, tag="lnv")
    nc.vector.tensor_scalar_mul(lnv[:, n_seq:2 * n_seq],
                                stat_z[0:1, :n_seq], 1.0 / D)
    mean_bc = psum_pool.tile([P, 1024], F32, tag=bc_tags[1], name="meanbc")
    for (a, b) in _regions(n_seq):
        nc.tensor.matmul(mean_bc[:, a:b], cx.ones128[:, :],
                         lnv[0:1, n_seq + a:n_seq + b], start=True, stop=True)
    for m in range(NCH):
        nc.vector.tensor_tensor(y_sb[:, m, :], z_sb[:, m, :],
                                mean_bc[:, :n_seq], ALU.subtract)
    msq = sb.tile([1, n_seq], F32, tag="ln_msq")
    nc.scalar.activation(msq[:], stat_z[0:1, :n_seq], AF.Square, bias=0.0,
                         scale=1.0 / D)
    rstd_t = sb.tile([1, n_seq], F32, tag="ln_rstd")
    nc.vector.scalar_tensor_tensor(rstd_t[:], stat_zsq[0:1, :n_seq], 1.0 / D,
                                   msq[:], op0=ALU.mult, op1=ALU.subtract)
    nc.scalar.activation(rstd_t[:], rstd_t[:], AF.Ln, bias=cx.eps_sb[0:1, :],
                         scale=1.0)
    nc.scalar.activation(lnv[:, 0:n_seq], rstd_t[:], AF.Exp, bias=0.0,
                         scale=-0.5)
    rstd_bc = psum_pool.tile([P, 1024], F32, tag=bc_tags[0], name="rstdbc")
    for (a, b) in _regions(n_seq):
        nc.tensor.matmul(rstd_bc[:, a:b], cx.ones128[:, :], lnv[0:1, a:b],
                         start=True, stop=True)
    for m in range(NCH):
        nc.vector.tensor_tensor(y_sb[:, m, :], y_sb[:, m, :],
                                rstd_bc[:, :n_seq], ALU.mult)
        if scale_gb is not None:
            g_sb, b_sb = scale_gb
            nc.vector.tensor_scalar(
                y_sb[:, m, :], y_sb[:, m, :],
                g_sb[:, m:m + 1], b_sb[:, m:m + 1], op0=ALU.mult, op1=ALU.add)
        if out_cb is not None:
            out_cb(m)


def _attn_units(layer, n_kv):
    """Unit list: (j_list, qa, qb, exp_scale). One exp call per unit covers
    both heads of the pair (and both j's of a pair for layer 2)."""
    J = n_kv // P
    units = []
    if layer == 1:
        # wm1: (q<512, kv<512) and (q>=512, kv>=512) get W. kv-pairs never
        # straddle the 512 boundary, so scale is uniform per (q-block, pair)
        # and each unit covers two kv chunks x both heads in one exp call.
        for jp in range(J // 2):
            for b in range(3):
                sc = (W * INV_SQRT if (b * 256 < 512) == (jp * 2 * P < 512)
                      else INV_SQRT)
                units.append(([2 * jp, 2 * jp + 1], 256 * b, 256 * (b + 1),
                              sc))
    else:
        # wm2: diagonal 256-blocks get W; kv pair jp covers block jp
        for jp in range(J // 2):
            for b in range(3):
                sc = W * INV_SQRT if b == jp else INV_SQRT
                units.append(([2 * jp, 2 * jp + 1], 256 * b, 256 * (b + 1),
                              sc))
    return units


def _exp_slices(layer, j, n_q):
    """Per (attention layer, key-chunk j): (col_lo, col_hi, exp scale)."""
    if layer == 1:
        jlo = j * P < 512
        s_lo = W * INV_SQRT if jlo else INV_SQRT
        s_hi = INV_SQRT if jlo else W * INV_SQRT
        return [(0, 512, s_lo), (512, n_q, s_hi)]
    blk = j // 2
    raw = [(b * 256, min((b + 1) * 256, n_q),
            W * INV_SQRT if b == blk else INV_SQRT) for b in range(3)]
    out = [raw[0]]
    for (lo, hi, sc) in raw[1:]:
        plo, phi, psc = out[-1]
        if sc == psc and lo == phi:
            out[-1] = (plo, hi, sc)
        else:
            out.append((lo, hi, sc))
    return out


def _emit_attn_base(nc, cx, work, psum_s, psum_ot, qT_sb, kT_sb, v_sb, ot_sb,
                    layer, n_q, n_kv):
    """Baseline-style attention: per-head S psum tiles + sliced exp."""
    J = n_kv // P
    heads = lambda c: ((slice(0, 64), 2 * c), (slice(64, 128), 2 * c + 1))
    units = [(j, a, b) for j in range(J) for (a, b) in _regions(n_q)]

    def emit_S_unit(c, u, etiles):
        (hb_e, h_e), (hb_o, h_o) = heads(c)
        j, a, b = units[u]
        if j not in etiles:
            etiles[j] = (
                work.tile([P, n_q], BF, tag="exps", bufs=4, name=f"ee{c}_{j}"),
                work.tile([P, n_q], BF, tag="exps", bufs=4, name=f"eo{c}_{j}"),
            )
        e_e, e_o = etiles[j]
        ps_e = psum_s.tile([P, 512], F32, tag="s", name=f"se{c}_{j}_{a}")
        ps_o = psum_s.tile([P, 512], F32, tag="s", name=f"so{c}_{j}_{a}")
        nc.tensor.matmul(ps_e[:, :b - a],
                         kT_sb[hb_e, c, j * P:(j + 1) * P],
                         qT_sb[hb_e, c, a:b], start=True, stop=True)
        nc.tensor.matmul(ps_o[:, :b - a],
                         kT_sb[hb_o, c, j * P:(j + 1) * P],
                         qT_sb[hb_o, c, a:b], start=True, stop=True)
        for e, ps in ((e_e, ps_e), (e_o, ps_o)):
            for (lo, hi, sc) in _exp_slices(layer, j, n_q):
                lo2, hi2 = max(lo, a), min(hi, b)
                if lo2 < hi2:
                    nc.scalar.activation(
                        e[:, lo2:hi2], ps[:, lo2 - a:hi2 - a],
                        AF.Exp, bias=0.0, scale=sc)

    def emit_O_unit(c, u, etiles, po_all):
        j, a, b = units[u]
        e_e, e_o = etiles[j]
        for (hb, h), e in zip(heads(c), (e_e, e_o)):
            nc.tensor.matmul(
                po_all[h][0:65, a:b],
                v_sb[:, j, h, 0:65],
                e[:, a:b],
                start=(j == 0), stop=(j == J - 1))

    for c in range(H // 2):
        po_all = {}
        for hb, h in heads(c):
            po_all[h] = psum_ot.tile([65, 768], F32, tag="ot", name=f"po{h}")
        etiles = {}
        emit_S_unit(c, 0, etiles)
        for u in range(len(units)):
            if u + 1 < len(units):
                emit_S_unit(c, u + 1, etiles)
            emit_O_unit(c, u, etiles, po_all)
        srow = work.tile([1, 2 * n_q], F32, tag="srow", bufs=2,
                         name=f"sr{c}")
        rr = work.tile([1, 2 * n_q], F32, tag="rr", bufs=2, name=f"rr{c}")
        (hb_e, h_e), (hb_o, h_o) = heads(c)
        # stage denominator rows at partition 0 in SBUF: the custom-DVE
        # reciprocal misreads PSUM at base partition 64 on hardware
        nc.vector.tensor_copy(srow[:, 0:n_q], po_all[h_e][64:65, 0:n_q])
        nc.vector.tensor_copy(srow[:, n_q:2 * n_q],
                              po_all[h_o][64:65, 0:n_q])
        nc.vector.reciprocal_approx_fast(out=rr[:, 0:n_q],
                                         in_=srow[:, 0:n_q])
        nc.vector.reciprocal_approx_fast(out=rr[:, n_q:2 * n_q],
                                         in_=srow[:, n_q:2 * n_q])
        nc.scalar.copy(ot_sb[hb_e, c, :], po_all[h_e][0:64, :n_q])
        nc.scalar.copy(ot_sb[hb_o, c, :], po_all[h_o][0:64, :n_q])
        drp = cx.dram.tile([2, n_q], F32, tag="drp", name=f"drp{c}")
        nc.sync.dma_start(drp[0:1, :], rr[:, 0:n_q])
        nc.sync.dma_start(drp[1:2, :], rr[:, n_q:2 * n_q])
        bc = work.tile([P, n_q], F32, tag="attn_bc", name=f"bc{c}")
        nc.gpsimd.dma_start(bc[0:64, :], drp[0:1, :].to_broadcast([64, n_q]))
        nc.gpsimd.dma_start(bc[64:128, :],
                            drp[1:2, :].to_broadcast([64, n_q]))
        nc.vector.tensor_tensor(ot_sb[:, c, :], ot_sb[:, c, :], bc[:],
                                ALU.mult)


def _emit_attn(nc, cx, work, psum_s, psum_ot, qT_sb, kT_sb, v_sb, ot_sb,
               layer, n_q, n_kv, filler=None):
    """Cross-attention. Per unit: the heads of a pair (x kv-pair for layer 2)
    are emitted as ADJACENT K=64 matmuls into PE row-groups 0/64 (concurrent)
    writing one shared PSUM tile; ONE ACT exp call (wm scale folded in)
    covers the whole tile. O.T accumulates per unit with a ones-augmented V
    column so the softmax denominator lands in PSUM row 64. S for unit u+1 is
    emitted before O of unit u so the PE has independent work while ACT
    computes exp.

    filler: optional per-pair list of callables emitting independent
    full-array PE work (borrowing an s-tag PSUM slot). Attention's K=64 /
    65-row matmuls only half-use the PE array and never re-warm the HAM
    clock gate; dense filler matmuls keep it at 2.4 GHz."""
    J = n_kv // P
    units = _attn_units(layer, n_kv)
    heads = lambda c: ((slice(0, 64), 2 * c), (slice(64, 128), 2 * c + 1))

    def ekey(jl):
        return jl[0]

    def emit_S_unit(c, u, etiles):
        jl, a, b, sc = units[u]
        w = b - a
        nj = len(jl)
        if ekey(jl) not in etiles:
            etiles[ekey(jl)] = work.tile([P, nj * 2, n_q], BF, tag="exps",
                                         bufs=3, name=f"e{c}_{ekey(jl)}")
        E = etiles[ekey(jl)]
        # Bank-safe layout: all head-even S in bank 0 ([0:512]), head-odd in
        # bank 1 ([512:1024]) — the two heads' matmuls execute CONCURRENTLY
        # via PE row-groups 0/64, and concurrent drains into the SAME PSUM
        # bank are a hardware fault. u-order (all-he, then all-ho) keeps the
        # psum stride regular (w) so one strided ACT exp covers the tile.
        ps = psum_s.tile([P, 1024], F32, tag="s", name=f"s{c}_{u}")
        (hb_e, h_e), (hb_o, h_o) = heads(c)
        for ji, j in enumerate(jl):
            nc.tensor.matmul(ps[:, ji * w:(ji + 1) * w],
                             kT_sb[hb_e, c, j * P:(j + 1) * P],
                             qT_sb[hb_e, c, a:b], start=True, stop=True)
            nc.tensor.matmul(ps[:, 512 + ji * w:512 + (ji + 1) * w],
                             kT_sb[hb_o, c, j * P:(j + 1) * P],
                             qT_sb[hb_o, c, a:b], start=True, stop=True)
        ps_g = ps[:, 0:1024].rearrange("p (g q) -> p g q", g=2)
        if nj == 1:
            nc.scalar.activation(E[:, :, a:b], ps_g[:, :, 0:w],
                                 AF.Exp, bias=0.0, scale=sc)
        else:
            nc.scalar.activation(
                E[:, :, a:b].rearrange("p (g j) q -> p g j q", g=2),
                ps_g[:, :, 0:nj * w].rearrange("p g (j q) -> p g j q", j=nj),
                AF.Exp, bias=0.0, scale=sc)

    def emit_O_unit(c, u, etiles, po_all):
        jl, a, b, sc = units[u]
        nj = len(jl)
        E = etiles[ekey(jl)]
        # start/stop must be unique per PSUM BANK (start=True clears the
        # whole bank's has_written bits): only the first/last matmul touching
        # a bank carries the flag; sibling regions in the same bank rely on
        # per-element overwrite-then-accumulate semantics.
        first_in_bank = a % 512 == 0
        last_in_bank = (b % 512 == 0) or (b == n_q)
        for ji, j in enumerate(jl):
            for hi, (hb, h) in enumerate(heads(c)):
                nc.tensor.matmul(
                    po_all[h][0:65, a:b],
                    v_sb[:, j, h, 0:65],
                    E[:, hi * nj + ji, a:b],
                    start=(j == 0 and first_in_bank),
                    stop=(j == J - 1 and last_in_bank))

    fill_i = [0]

    def feed_filler(c):
        # filler entries are (gate, fn): fn may only be emitted once pair
        # `gate` has been normalized (so a stalled piece never parks on an
        # s-slot the attention pipeline needs)
        if filler is not None and fill_i[0] < len(filler):
            gate, fn = filler[fill_i[0]]
            if gate < c:
                fn()
                fill_i[0] += 1

    for c in range(H // 2):
        po_all = {}
        for hb, h in heads(c):
            po_all[h] = psum_ot.tile([65, 768], F32, tag="ot", name=f"po{h}")
        etiles = {}
        emit_S_unit(c, 0, etiles)
        for u in range(len(units)):
            if u + 1 < len(units):
                emit_S_unit(c, u + 1, etiles)
            emit_O_unit(c, u, etiles, po_all)
            if layer == 2 or u % 2 == 1:
                feed_filler(c)
        # normalizers: stage denominator rows at partition 0 (custom-DVE
        # reciprocal misreads PSUM at base partition 64 on hardware), recip,
        # DRAM-bounce broadcast. The copies are split across ACT and DVE so
        # the serial tail chain pipelines across both engines.
        srow = work.tile([1, 2 * n_q], F32, tag="srow", bufs=2,
                         name=f"sr{c}")
        rr = work.tile([1, 2 * n_q], F32, tag="rr", bufs=2, name=f"rr{c}")
        (hb_e, h_e), (hb_o, h_o) = heads(c)
        nc.scalar.copy(srow[:, 0:n_q], po_all[h_e][64:65, 0:n_q])
        nc.vector.tensor_copy(srow[:, n_q:2 * n_q],
                              po_all[h_o][64:65, 0:n_q])
        nc.vector.reciprocal_approx_fast(out=rr[:, 0:n_q],
                                         in_=srow[:, 0:n_q])
        nc.vector.reciprocal_approx_fast(out=rr[:, n_q:2 * n_q],
                                         in_=srow[:, n_q:2 * n_q])
        nc.scalar.copy(ot_sb[hb_e, c, :], po_all[h_e][0:64, :n_q])
        nc.vector.tensor_copy(ot_sb[hb_o, c, :], po_all[h_o][0:64, :n_q])
        drp = cx.dram.tile([2, n_q], F32, tag="drp", name=f"drp{c}")
        nc.sync.dma_start(drp[0:1, :], rr[:, 0:n_q])
        nc.sync.dma_start(drp[1:2, :], rr[:, n_q:2 * n_q])
        bc = work.tile([P, n_q], F32, tag="attn_bc", name=f"bc{c}")
        nc.gpsimd.dma_start(bc[0:64, :], drp[0:1, :].to_broadcast([64, n_q]))
        nc.gpsimd.dma_start(bc[64:128, :],
                            drp[1:2, :].to_broadcast([64, n_q]))
        nc.vector.tensor_tensor(ot_sb[:, c, :], ot_sb[:, c, :], bc[:],
                                ALU.mult)
        feed_filler(c + 1)
    # flush remaining filler pieces (their deps are all satisfied now)
    if filler is not None:
        while fill_i[0] < len(filler):
            filler[fill_i[0]][1]()
            fill_i[0] += 1


def _r3(ap):
    """DRAM [K*128, n] -> [128(part), K, n] view for DMA."""
    return ap.rearrange("(ko p) s -> p ko s", p=P)


def _build_program(flags):
    use_bo1, use_bo2, use_fb1, use_fb2, use_g1, use_g2, use_g3 = flags
    nc = _Bacc("TRN2", target_bir_lowering=False, debug=False)

    def din(name, shape, dt=BF):
        return nc.dram_tensor(name, shape, dt, kind="ExternalInput").ap()

    x1T = din("x1T", [D, S1])
    x2T = din("x2T", [D, S2])
    x3T = din("x3T", [D, S3])
    wts = {n: din(n, [D, D]) for n in
           ("wq1T", "wk1T", "wv1T", "wo1T", "wq2T", "wk2T", "wv2T", "wo2T")}
    fw1T = din("fw1T", [D, DF])
    fw2T = din("fw2T", [DF, D])
    bo1 = din("bo1", [P, NCH]) if use_bo1 else None
    bo2 = din("bo2", [P, NCH]) if use_bo2 else None
    fb1 = din("fb1", [P, DF // P]) if use_fb1 else None
    fb2 = din("fb2", [P, NCH]) if use_fb2 else None
    gbd = {}
    for i, use in ((1, use_g1), (2, use_g2), (3, use_g3)):
        gbd[i] = (din(f"g{i}", [P, NCH]),
                  din(f"b{i}", [P, NCH])) if use else None
    yT = nc.dram_tensor("yT", [D, S1], F32, kind="ExternalOutput").ap()
    taps = {}
    if _DEBUG_TAPS:
        for tn in ("t_q1", "t_ot1", "t_y1", "t_ot2", "t_y2"):
            taps[tn] = nc.dram_tensor(tn, [D, S1], BF,
                                      kind="ExternalOutput").ap()

    def tap(name, src):
        if _DEBUG_TAPS:
            nc.sync.dma_start(_r3(taps[name]), src[:])

    with tile.TileContext(nc, pool_alloc_mode="queue") as tc:
        cx = _Ctx()
        cx.tc = tc
        with tc.tile_pool(name="sb", bufs=1) as sb, \
             tc.tile_pool(name="zsq", bufs=1) as zsq_pool, \
             tc.tile_pool(name="wpre", bufs=1) as wpre, \
             tc.tile_pool(name="dram", bufs=2, space="DRAM") as dram:
            cx.sb, cx.zsq_pool, cx.dram = sb, zsq_pool, dram

            ones_bf = sb.tile([P, 1], BF, tag="ones_bf")
            nc.vector.memset(ones_bf[:], 1.0)
            cx.ones_bf = ones_bf
            ones128 = sb.tile([1, P], BF, tag="ones128")
            nc.vector.memset(ones128[:], 1.0)
            cx.ones128 = ones128
            eps_sb = sb.tile([P, 1], F32, tag="eps")
            nc.vector.memset(eps_sb[:], EPS)
            cx.eps_sb = eps_sb

            def load_pm(ap, cols, tag):
                if ap is None:
                    return None
                t = sb.tile([P, cols], BF, tag=tag)
                nc.sync.dma_start(t[:], ap)
                return t

            bo1_sb = load_pm(bo1, NCH, "bo1")
            bo2_sb = load_pm(bo2, NCH, "bo2")
            fb1_sb = load_pm(fb1, DF // P, "fb1")
            fb2_sb = load_pm(fb2, NCH, "fb2")
            gb_sb = {}
            for i in (1, 2, 3):
                gb_sb[i] = None if gbd[i] is None else (
                    load_pm(gbd[i][0], NCH, f"g{i}"),
                    load_pm(gbd[i][1], NCH, f"b{i}"))

            y1_sb = sb.tile([P, NCH, S1], BF, tag="y1")
            y2_sb = sb.tile([P, NCH, S1], BF, tag="y2")

            def copy_cb(dst, eng):
                return lambda m, ps: eng(dst[:, m, :], ps)

            dve_copy = lambda out, ps: nc.vector.tensor_copy(out, ps)
            act_copy = (lambda out, ps: nc.scalar.copy(out, ps)) \
                if _USE_ACT_COPY else dve_copy

            def emit_v_proj(psum_pool, x_sb, wv_sb, v_sb, Jkv):
                nc.vector.memset(v_sb[:, :, :, 64:65], 1.0)
                for j in range(Jkv):
                    ps = psum_pool.tile([P, 1024], F32, tag="proj",
                                        name=f"vps{j}")
                    for ko in range(NCH):
                        nc.tensor.matmul(
                            ps[:, 0:D],
                            x_sb[:, ko, j * P:(j + 1) * P],
                            wv_sb[:, ko, :],
                            start=(ko == 0), stop=(ko == NCH - 1))
                    nc.vector.tensor_copy(
                        v_sb[:, j, :, 0:64],
                        ps[:, 0:D].rearrange("p (h v) -> p h v", h=H))

            # open order is reverse of close order (pool stack is LIFO)
            kv2 = tc.tile_pool(name="kv2", bufs=1)
            with kv2 as kv2p:
                x3_sb = kv2p.tile([P, NCH, S3], BF, tag="xkv")
                wk2_sb = kv2p.tile([P, NCH, D], BF, tag="wk")
                wv2_sb = kv2p.tile([P, NCH, D], BF, tag="wv")
                k2_sb = kv2p.tile([P, NCH, S3], BF, tag="k")
                v2_sb = kv2p.tile([P, S3 // P, H, 65], BF, tag="v")

                otp1 = contextlib.ExitStack()
                otp1p = otp1.enter_context(tc.tile_pool(name="otp1", bufs=1))
                x1_sb = otp1p.tile([P, NCH, S1], BF, tag="x1")
                ot_sb = otp1p.tile([P, NCH, S1], BF, tag="ot1")

                at1_ctx = contextlib.ExitStack()
                at1p = at1_ctx.enter_context(tc.tile_pool(name="at1", bufs=1))
                q_sb = at1p.tile([P, NCH, S1], BF, tag="q")
                k_sb = at1p.tile([P, NCH, S2], BF, tag="k")
                v_sb = at1p.tile([P, S2 // P, H, 65], BF, tag="v")

                kv1_ctx = contextlib.ExitStack()
                kv1p = kv1_ctx.enter_context(tc.tile_pool(name="kv1", bufs=1))
                wq_sb = kv1p.tile([P, NCH, D], BF, tag="wq")
                x2_sb = kv1p.tile([P, NCH, S2], BF, tag="xkv")
                wk_sb = kv1p.tile([P, NCH, D], BF, tag="wk")
                wv_sb = kv1p.tile([P, NCH, D], BF, tag="wv")
                # per-chunk DMAs in consumption order, issued across FOUR
                # engine DGE queues in parallel (descriptor generation is
                # ~1us serial per engine): the ko-major Q1/K1 projections
                # start as soon as their first chunks land
                for ko in range(NCH):
                    nc.sync.dma_start(wq_sb[:, ko, :],
                                      _r3(wts["wq1T"])[:, ko, :])
                    nc.gpsimd.dma_start(x1_sb[:, ko, :], _r3(x1T)[:, ko, :])
                for ko in range(NCH):
                    nc.scalar.dma_start(wk_sb[:, ko, :],
                                        _r3(wts["wk1T"])[:, ko, :])
                    nc.scalar.dma_start(x2_sb[:, ko, :], _r3(x2T)[:, ko, :])
                nc.sync.dma_start(wv_sb[:], _r3(wts["wv1T"]))
                nc.gpsimd.dma_start(x3_sb[:], _r3(x3T))
                nc.sync.dma_start(wk2_sb[:], _r3(wts["wk2T"]))
                nc.sync.dma_start(wv2_sb[:], _r3(wts["wv2T"]))

                # prefetch every later-phase weight now (bf16 fits in SBUF)
                wo1_sb = wpre.tile([P, NCH, D], BF, tag="wo1")
                wq2_sb = wpre.tile([P, NCH, D], BF, tag="wq2")
                wo2_sb = wpre.tile([P, NCH, D], BF, tag="wo2")
                fw1_sb = wpre.tile([P, NCH, DF], BF, tag="fw1")
                fw2_sb = wpre.tile([P, DF // P, D], BF, tag="fw2")
                nc.sync.dma_start(wo1_sb[:], _r3(wts["wo1T"]))
                nc.sync.dma_start(wq2_sb[:], _r3(wts["wq2T"]))
                nc.sync.dma_start(wo2_sb[:], _r3(wts["wo2T"]))
                nc.sync.dma_start(fw1_sb[:], _r3(fw1T))
                nc.sync.dma_start(fw2_sb[:], _r3(fw2T))

                # dense warm-up block: q1,k1,v1 (q1/k1 ko-major so the PE
                # starts on the first DMA'd chunks); k2/v2 are deferred into
                # the WO1/LN1 and WO2/LN2 phases as independent PE filler
                with tc.tile_pool(name="psA", bufs=4, space="PSUM") as psA:
                    _emit_proj(nc, psA, wq_sb, x1_sb, D, S1, NCH,
                               copy_cb(q_sb, act_copy), ko_major=True)
                    _emit_proj(nc, psA, wk_sb, x2_sb, D, S2, NCH,
                               copy_cb(k_sb, act_copy), ko_major=True)
                    emit_v_proj(psA, x2_sb, wv_sb, v_sb, S2 // P)
                kv1_ctx.close()  # frees x2 + wq1/wk1/wv1 SBUF

                # attention 1, with K2/V2 projection pieces as full-array
                # PE filler woven between j-groups (keeps the HAM clock warm
                # through the half-array attention matmuls)
                attn_fn = _emit_attn_base if _USE_BASE_ATTN else _emit_attn
                s_bufs = 4 if _USE_BASE_ATTN else 2
                nc.vector.memset(v2_sb[:, :, :, 64:65], 1.0)
                with tc.tile_pool(name="wk1w", bufs=3) as work, \
                     tc.tile_pool(name="ps_s1", bufs=s_bufs,
                                  space="PSUM") as pss, \
                     tc.tile_pool(name="ps_ot1", bufs=2,
                                  space="PSUM") as psot:
                    def mk_k2(m, a, b):
                        def f():
                            ps = pss.tile([P, 1024], F32, tag="s",
                                          name=f"fk2_{m}_{a}")
                            for ko in range(NCH):
                                nc.tensor.matmul(
                                    ps[:, 0:b - a],
                                    wk2_sb[:, ko, m * P:(m + 1) * P],
                                    x3_sb[:, ko, a:b],
                                    start=(ko == 0), stop=(ko == NCH - 1))
                            nc.vector.tensor_copy(k2_sb[:, m, a:b],
                                                  ps[:, 0:b - a])
                        return f

                    def mk_v2(j, half):
                        def f():
                            ps = pss.tile([P, 1024], F32, tag="s",
                                          name=f"fv2_{j}_{half}")
                            a = half * 256
                            for ko in range(NCH):
                                nc.tensor.matmul(
                                    ps[:, 0:256],
                                    x3_sb[:, ko, j * P:(j + 1) * P],
                                    wv2_sb[:, ko, a:a + 256],
                                    start=(ko == 0), stop=(ko == NCH - 1))
                            nc.vector.tensor_copy(
                                v2_sb[:, j, 4 * half:4 * half + 4, 0:64],
                                ps[:, 0:256].rearrange("p (h v) -> p h v",
                                                       h=4))
                        return f

                    fillers = []
                    for m in range(NCH):
                        fillers.append((-1, mk_k2(m, 0, 256)))
                        fillers.append((-1, mk_k2(m, 256, 512)))
                        fillers.append((-1, mk_k2(m, 512, 768)))
                    for j in range(S3 // P):
                        fillers.append((-1, mk_v2(j, 0)))
                        fillers.append((-1, mk_v2(j, 1)))
                    if _USE_BASE_ATTN:
                        attn_fn(nc, cx, work, pss, psot, q_sb, k_sb, v_sb,
                                ot_sb, 1, S1, S2)
                    else:
                        attn_fn(nc, cx, work, pss, psot, q_sb, k_sb, v_sb,
                                ot_sb, 1, S1, S2, filler=fillers)
                tap("t_q1", q_sb)
                tap("t_ot1", ot_sb)
                at1_ctx.close()  # frees q1/k1/v1 SBUF

                # wo1 + LN1 (wo1 ko-major in 2-m groups: the first WO matmuls
                # need only attention pair 0's normalized output)
                with tc.tile_pool(name="psB1", bufs=2, space="PSUM") as psB:
                    wo_ps = {}
                    _emit_proj(nc, psB, wo1_sb, ot_sb, D, S1, NCH,
                               lambda m, ps: wo_ps.__setitem__(m, ps),
                               ko_major=True, m_group=2)
                    _emit_ln(nc, cx, psB, lambda m: wo_ps[m], x1_sb,
                             bo1_sb, y1_sb, y1_sb, S1, gb_sb[1])
                tap("t_y1", y1_sb)
                otp1.close()

                # q2 projection (ko-major: starts as soon as y1 chunk 0 is
                # normalized)
                otp2 = contextlib.ExitStack()
                otp2p = otp2.enter_context(tc.tile_pool(name="otp2", bufs=1))
                ot2_sb = otp2p.tile([P, NCH, S1], BF, tag="ot2")
                q2_sb = otp2p.tile([P, NCH, S1], BF, tag="q2")
                with tc.tile_pool(name="psC", bufs=4, space="PSUM") as psC:
                    _emit_proj(nc, psC, wq2_sb, y1_sb, D, S1, NCH,
                               copy_cb(q2_sb, act_copy), ko_major=_KO_MAJOR)

                # attention 2
                with tc.tile_pool(name="wk2w", bufs=3) as work2, \
                     tc.tile_pool(name="ps_s2", bufs=s_bufs,
                                  space="PSUM") as pss2, \
                     tc.tile_pool(name="ps_ot2", bufs=2,
                                  space="PSUM") as psot2:
                    attn_fn(nc, cx, work2, pss2, psot2, q2_sb, k2_sb,
                            v2_sb, ot2_sb, 2, S1, S3)
                tap("t_ot2", ot2_sb)

                # wo2 + LN2
                with tc.tile_pool(name="psD", bufs=2, space="PSUM") as psD:
                    wo2_ps = {}
                    _emit_proj(nc, psD, wo2_sb, ot2_sb, D, S1, NCH,
                               lambda m, ps: wo2_ps.__setitem__(m, ps),
                               ko_major=True, m_group=2)
                    _emit_ln(nc, cx, psD, lambda m: wo2_ps[m], y1_sb,
                             bo2_sb, y2_sb, y2_sb, S1, gb_sb[2])
                tap("t_y2", y2_sb)
                otp2.close()

            # FFN + LN3
            zbuf = sb.tile([P, NCH, S1], BF, tag="y1")  # reuse y1 slot
            yT_sb = sb.tile([P, NCH, S1], F32, tag="yT")
            with tc.tile_pool(name="ffn1", bufs=1) as f1p:
                h_sb = f1p.tile([P, DF // P, S1], BF, tag="hT")
                with tc.tile_pool(name="psE", bufs=3, space="PSUM") as psE:
                    def gelu_consume(m, ps):
                        nc.scalar.activation(
                            h_sb[:, m, :], ps[:, 0:S1], AF.Gelu,
                            bias=(fb1_sb[:, m:m + 1]
                                  if fb1_sb is not None else 0.0),
                            scale=1.0)
                    # ko-major pairs: the first FFN1 matmuls need only y2
                    # chunk 0, starting inside LN2's normalize window
                    _emit_proj(nc, psE, fw1_sb, y2_sb, DF, S1, NCH,
                               gelu_consume, ko_major=True, m_group=2)

                with tc.tile_pool(name="psF", bufs=2, space="PSUM") as psF:
                    f2_ps = {}
                    _emit_proj(nc, psF, fw2_sb, h_sb, D, S1, DF // P,
                               lambda m, ps: f2_ps.__setitem__(m, ps))

                    out_eng = [nc.sync, nc.scalar, nc.gpsimd, nc.sync]

                    def out_dma(m):
                        out_eng[m].dma_start(_r3(yT)[:, m, :],
                                             yT_sb[:, m, :])

                    _emit_ln(nc, cx, psF, lambda m: f2_ps[m], y2_sb, fb2_sb,
                             zbuf, yT_sb, S1, gb_sb[3], out_cb=out_dma)

    nc.finalize()
    return nc


def _to_pm(vec, cols):
    """[cols*128] vector -> [128, cols] partition-major layout (bf16)."""
    return np.ascontiguousarray(vec.reshape(cols, P).T).astype(
        ml_dtypes.bfloat16)


def _bf(a):
    return np.ascontiguousarray(a).astype(ml_dtypes.bfloat16)


def kernel(**inputs):
    cords = np.asarray(inputs["cords_features"], np.float32)
    spatial = np.asarray(inputs["spatial_features"], np.float32)
    speed = np.asarray(inputs["speed_features"], np.float32)
    B = cords.shape[0]
    assert B == 8

    def g(name):
        return np.asarray(inputs[name], np.float32)

    flags = (
        not np.allclose(g("bo1"), 0), not np.allclose(g("bo2"), 0),
        not np.allclose(g("ffn_b1"), 0), not np.allclose(g("ffn_b2"), 0),
        not (np.allclose(g("ln1_g"), 1) and np.allclose(g("ln1_b"), 0)),
        not (np.allclose(g("ln2_g"), 1) and np.allclose(g("ln2_b"), 0)),
        not (np.allclose(g("ln3_g"), 1) and np.allclose(g("ln3_b"), 0)),
    )
    if flags not in _PROGRAM_CACHE:
        _PROGRAM_CACHE[flags] = _build_program(flags)
    nc = _PROGRAM_CACHE[flags]

    shared = {
        "wq1T": _bf(g("wq1").T), "wk1T": _bf(g("wk1").T),
        "wv1T": _bf(g("wv1").T), "wo1T": _bf(g("wo1").T),
        "wq2T": _bf(g("wq2").T), "wk2T": _bf(g("wk2").T),
        "wv2T": _bf(g("wv2").T), "wo2T": _bf(g("wo2").T),
        "fw1T": _bf(g("ffn_w1").T), "fw2T": _bf(g("ffn_w2").T),
    }
    use_bo1, use_bo2, use_fb1, use_fb2, use_g1, use_g2, use_g3 = flags
    if use_bo1:
        shared["bo1"] = _to_pm(g("bo1"), NCH)
    if use_bo2:
        shared["bo2"] = _to_pm(g("bo2"), NCH)
    if use_fb1:
        shared["fb1"] = _to_pm(g("ffn_b1"), DF // P)
    if use_fb2:
        shared["fb2"] = _to_pm(g("ffn_b2"), NCH)
    for i, use in ((1, use_g1), (2, use_g2), (3, use_g3)):
        if use:
            shared[f"g{i}"] = _to_pm(g(f"ln{i}_g"), NCH)
            shared[f"b{i}"] = _to_pm(g(f"ln{i}_b"), NCH)

    in_maps = []
    for b in range(B):
        m = dict(shared)
        m["x1T"] = _bf(cords[b].T)
        m["x2T"] = _bf(spatial[b].T)
        m["x3T"] = _bf(speed[b].T)
        in_maps.append(m)

    global _LAST_IN_MAPS
    _LAST_IN_MAPS = in_maps
    res = run_bass_kernel_spmd(nc, in_maps, core_ids=list(range(B)))
    out = np.stack([res.results[b]["yT"].T for b in range(B)], axis=0)
    return np.ascontiguousarray(out.astype(np.float32))


# revision 74
# speedup vs baseline: 1.2931x; 1.0071x over previous
"""DualCrossAttention Trainium2 kernel (bf16).

Data-parallel: batch=8 across 8 NeuronCores, one batch element per core.
Per core: two cross-attentions + FFN + 3 LayerNorms on [768, 512] activations.

Layout: feature-major activations (x.T: [feature(part), seq(free)]); weights
host-pre-transposed and cast to bf16 so every projection is a full-rate PE
matmul (bf16 streams 1 cycle/row vs 1.5 for fp32-HIGH, and enables FWL).
Attention: S.T = k_h @ q_h.T with the two heads of a pair emitted as adjacent
matmuls into PE row-groups 0/64 (concurrent execution) writing one shared
PSUM tile, so a single wide ACT exp call covers both heads; the wm scale is
folded into the exp affine. O.T accumulates with a ones-augmented V column so
the softmax denominator lands in PSUM row 64. All per-position normalizers
(attn 1/denominator, LN rstd/mean*rstd) are broadcast across partitions with
tiny K=1 PE matmuls into PSUM — no DRAM bounce. All weights are prefetched
at kernel start (bf16 halves the SBUF/DMA footprint).
"""
import contextlib

import numpy as np
import ml_dtypes

import concourse.bacc as bacc
import concourse.bass as bass
import concourse.tile as tile
from concourse import mybir
from concourse.bass_utils import run_bass_kernel_spmd
class _Bacc(bacc.Bacc):
    """Bacc with Exp/Ln pinned to the natural_log_exp_and_others ACT table
    set: the default chooser alternates between exp_and_others (attention
    softmax) and natural_log (LayerNorm rstd), paying a ~2.7us table load at
    every switch. Removing Exp/Ln from the single-function sets makes both
    resolve to the combined set, leaving only the Gelu switches."""

    def insert_act_table_loads(self):
        from concourse.hw_specs import get_activation_tables
        import bass_rust as _bass_rust
        has_activation = any(
            isinstance(i, mybir.InstActivation)
            for b in self.main_func.blocks
            for i in b.instructions
        )
        if not has_activation:
            return
        AFt = mybir.ActivationFunctionType
        tables = []
        for name, fns in get_activation_tables(self.m.arch).items():
            if name == "exp_and_others":
                fns = fns - {AFt.Exp}
            elif name == "natural_log":
                fns = fns - {AFt.Ln}
            tables.append((name, fns))
        _bass_rust.insert_act_table_loads(self, tables)


F32 = mybir.dt.float32
F32R = mybir.dt.float32r
BF = mybir.dt.bfloat16
AF = mybir.ActivationFunctionType
ALU = mybir.AluOpType

H, KD, VD = 8, 64, 64
D, DF = 512, 2048
S1, S2, S3 = 768, 1024, 768
P = 128
NCH = D // P            # 4 feature chunks of the 512-dim residual stream
W = 1.25                # wm weight scale
INV_SQRT = 0.125        # 1/sqrt(64)
EPS = 1e-5

_PROGRAM_CACHE = {}
_USE_BASE_ATTN = False
_USE_ACT_COPY = True
_KO_MAJOR = True
_DEBUG_TAPS = False


def _regions(n):
    """Split free dim n into <=512 column regions (PSUM-bank aligned)."""
    out = []
    s = 0
    while s < n:
        e = min(s + 512, n)
        out.append((s, e))
        s = e
    return out


class _Ctx:
    """Shared handles for the emit helpers."""
    pass


def _emit_proj(nc, psum_pool, wT_sb, xT_sb, n_out, n_seq, k_chunks, consume,
               ko_major=False, m_group=None, tag="proj"):
    """out.T[o, i] = sum_d wT[d, o] * xT[d, i]; calls consume(m, psum_ap).

    ko_major=True holds m-tiles live and loops ko outer / m inner, so the
    first matmuls only need xT chunk 0 (pipelines into a producer of xT).
    m_group limits how many m-tiles are live at once (PSUM pressure): the
    m-range is processed in groups, each group ko-major.
    """
    if ko_major:
        n_m = n_out // P
        if m_group is None:
            m_group = n_m
        for m0 in range(0, n_m, m_group):
            ms = range(m0, min(m0 + m_group, n_m))
            tiles = {m: psum_pool.tile([P, 1024], F32, tag=tag,
                                       name=f"pp{tag}{m}") for m in ms}
            for ko in range(k_chunks):
                for m in ms:
                    for (a, b) in _regions(n_seq):
                        nc.tensor.matmul(
                            tiles[m][:, a:b],
                            wT_sb[:, ko, m * P:(m + 1) * P],
                            xT_sb[:, ko, a:b],
                            start=(ko == 0), stop=(ko == k_chunks - 1),
                        )
            for m in ms:
                consume(m, tiles[m][:, :n_seq])
        return
    for m in range(n_out // P):
        ps = psum_pool.tile([P, 1024], F32, tag="proj", name=f"pp{m}")
        for ko in range(k_chunks):
            for (a, b) in _regions(n_seq):
                nc.tensor.matmul(
                    ps[:, a:b],
                    wT_sb[:, ko, m * P:(m + 1) * P],
                    xT_sb[:, ko, a:b],
                    start=(ko == 0), stop=(ko == k_chunks - 1),
                )
        consume(m, ps[:, :n_seq])


def _emit_ln(nc, cx, psum_pool, z_src, resid, bias_pm, z_sb, y_sb, n_seq,
             scale_gb, out_cb=None, bc_tags=("proj", "proj")):
    """LayerNorm over the feature axis (partitions x NCH chunks).

    z_src(m) -> psum AP [P, n_seq] (projection output chunk m);
    z = psum + bias + resid is built in z_sb (bf16); stats via ones-matmuls
    (partition reduction on PE); rstd via exp(-0.5 ln(var+eps)); rstd and
    mean*rstd are broadcast to [P, n_seq] with K=1 PE matmuls into recycled
    proj-tag PSUM slots; y_sb = z*rstd_bc - mr_bc (may alias z_sb).
    """
    sb = cx.sb
    stat_z = psum_pool.tile([1, 768], F32, tag="ln_stat_z", bufs=1)
    stat_zsq = psum_pool.tile([1, 768], F32, tag="ln_stat_zsq", bufs=1)
    for m in range(NCH):
        ps = z_src(m)
        if bias_pm is not None:
            nc.vector.scalar_tensor_tensor(
                z_sb[:, m, :], ps, bias_pm[:, m:m + 1], resid[:, m, :],
                op0=ALU.add, op1=ALU.add)
        else:
            nc.vector.tensor_tensor(z_sb[:, m, :], ps, resid[:, m, :], ALU.add)
        zsq = cx.zsq_pool.tile([P, 768], BF, tag="ln_zsq", name=f"zsq{m}")
        # last chunk's square on DVE (shorter critical path); others on the
        # otherwise-idle gpsimd
        sq_eng = nc.vector if m == NCH - 1 else nc.gpsimd
        sq_eng.tensor_tensor(zsq[:, :n_seq], z_sb[:, m, :], z_sb[:, m, :],
                             ALU.mult)
        for (a, b) in _regions(n_seq):
            nc.tensor.matmul(stat_z[0:1, a:b], cx.ones_bf[:, 0:1],
                             z_sb[:, m, a:b],
                             start=(m == 0), stop=(m == NCH - 1))
            nc.tensor.matmul(stat_zsq[0:1, a:b], cx.ones_bf[:, 0:1],
                             zsq[:, a:b],
                             start=(m == 0), stop=(m == NCH - 1))
    # y = (z - mean) * rstd, ordered so only the rstd-multiply trails the
    # serial Square->var->Ln->Exp chain: the mean broadcast and the
    # mean-subtract of every chunk run as soon as the stats land.
    lnv = sb.tile([1, 2 * n_seq], BF, tag="lnv")
    nc.vector.tensor_scalar_mul(lnv[:, n_seq:2 * n_seq],
                                stat_z[0:1, :n_seq], 1.0 / D)
    mean_bc = psum_pool.tile([P, 1024], F32, tag=bc_tags[1], name="meanbc")
    for (a, b) in _regions(n_seq):
        nc.tensor.matmul(mean_bc[:, a:b], cx.ones128[:, :],
                         lnv[0:1, n_seq + a:n_seq + b], start=True, stop=True)
    for m in range(NCH):
        nc.vector.tensor_tensor(y_sb[:, m, :], z_sb[:, m, :],
                                mean_bc[:, :n_seq], ALU.subtract)
    msq = sb.tile([1, n_seq], F32, tag="ln_msq")
    nc.scalar.activation(msq[:], stat_z[0:1, :n_seq], AF.Square, bias=0.0,
                         scale=1.0 / D)
    rstd_t = sb.tile([1, n_seq], F32, tag="ln_rstd")
    nc.vector.scalar_tensor_tensor(rstd_t[:], stat_zsq[0:1, :n_seq], 1.0 / D,
                                   msq[:], op0=ALU.mult, op1=ALU.subtract)
    nc.scalar.activation(rstd_t[:], rstd_t[:], AF.Ln, bias=cx.eps_sb[0:1, :],
                         scale=1.0)
    nc.scalar.activation(lnv[:, 0:n_seq], rstd_t[:], AF.Exp, bias=0.0,
                         scale=-0.5)
    rstd_bc = psum_pool.tile([P, 1024], F32, tag=bc_tags[0], name="rstdbc")
    for (a, b) in _regions(n_seq):
        nc.tensor.matmul(rstd_bc[:, a:b], cx.ones128[:, :], lnv[0:1, a:b],
                         start=True, stop=True)
    for m in range(NCH):
        nc.vector.tensor_tensor(y_sb[:, m, :], y_sb[:, m, :],
                                rstd_bc[:, :n_seq], ALU.mult)
        if scale_gb is not None:
            g_sb, b_sb = scale_gb
            nc.vector.tensor_scalar(
                y_sb[:, m, :], y_sb[:, m, :],
                g_sb[:, m:m + 1], b_sb[:, m:m + 1], op0=ALU.mult, op1=ALU.add)
        if out_cb is not None:
            out_cb(m)


def _attn_units(layer, n_kv):
    """Unit list: (j_list, qa, qb, exp_scale). One exp call per unit covers
    both heads of the pair (and both j's of a pair for layer 2)."""
    J = n_kv // P
    units = []
    if layer == 1:
        # wm1: (q<512, kv<512) and (q>=512, kv>=512) get W. kv-pairs never
        # straddle the 512 boundary, so scale is uniform per (q-block, pair)
        # and each unit covers two kv chunks x both heads in one exp call.
        for jp in range(J // 2):
            for b in range(3):
                sc = (W * INV_SQRT if (b * 256 < 512) == (jp * 2 * P < 512)
                      else INV_SQRT)
                units.append(([2 * jp, 2 * jp + 1], 256 * b, 256 * (b + 1),
                              sc))
    else:
        # wm2: diagonal 256-blocks get W; kv pair jp covers block jp
        for jp in range(J // 2):
            for b in range(3):
                sc = W * INV_SQRT if b == jp else INV_SQRT
                units.append(([2 * jp, 2 * jp + 1], 256 * b, 256 * (b + 1),
                              sc))
    return units


def _exp_slices(layer, j, n_q):
    """Per (attention layer, key-chunk j): (col_lo, col_hi, exp scale)."""
    if layer == 1:
        jlo = j * P < 512
        s_lo = W * INV_SQRT if jlo else INV_SQRT
        s_hi = INV_SQRT if jlo else W * INV_SQRT
        return [(0, 512, s_lo), (512, n_q, s_hi)]
    blk = j // 2
    raw = [(b * 256, min((b + 1) * 256, n_q),
            W * INV_SQRT if b == blk else INV_SQRT) for b in range(3)]
    out = [raw[0]]
    for (lo, hi, sc) in raw[1:]:
        plo, phi, psc = out[-1]
        if sc == psc and lo == phi:
            out[-1] = (plo, hi, sc)
        else:
            out.append((lo, hi, sc))
    return out


def _emit_attn_base(nc, cx, work, psum_s, psum_ot, qT_sb, kT_sb, v_sb, ot_sb,
                    layer, n_q, n_kv):
    """Baseline-style attention: per-head S psum tiles + sliced exp."""
    J = n_kv // P
    heads = lambda c: ((slice(0, 64), 2 * c), (slice(64, 128), 2 * c + 1))
    units = [(j, a, b) for j in range(J) for (a, b) in _regions(n_q)]

    def emit_S_unit(c, u, etiles):
        (hb_e, h_e), (hb_o, h_o) = heads(c)
        j, a, b = units[u]
        if j not in etiles:
            etiles[j] = (
                work.tile([P, n_q], BF, tag="exps", bufs=4, name=f"ee{c}_{j}"),
                work.tile([P, n_q], BF, tag="exps", bufs=4, name=f"eo{c}_{j}"),
            )
        e_e, e_o = etiles[j]
        ps_e = psum_s.tile([P, 512], F32, tag="s", name=f"se{c}_{j}_{a}")
        ps_o = psum_s.tile([P, 512], F32, tag="s", name=f"so{c}_{j}_{a}")
        nc.tensor.matmul(ps_e[:, :b - a],
                         kT_sb[hb_e, c, j * P:(j + 1) * P],
                         qT_sb[hb_e, c, a:b], start=True, stop=True)
        nc.tensor.matmul(ps_o[:, :b - a],
                         kT_sb[hb_o, c, j * P:(j + 1) * P],
                         qT_sb[hb_o, c, a:b], start=True, stop=True)
        for e, ps in ((e_e, ps_e), (e_o, ps_o)):
            for (lo, hi, sc) in _exp_slices(layer, j, n_q):
                lo2, hi2 = max(lo, a), min(hi, b)
                if lo2 < hi2:
                    nc.scalar.activation(
                        e[:, lo2:hi2], ps[:, lo2 - a:hi2 - a],
                        AF.Exp, bias=0.0, scale=sc)

    def emit_O_unit(c, u, etiles, po_all):
        j, a, b = units[u]
        e_e, e_o = etiles[j]
        for (hb, h), e in zip(heads(c), (e_e, e_o)):
            nc.tensor.matmul(
                po_all[h][0:65, a:b],
                v_sb[:, j, h, 0:65],
                e[:, a:b],
                start=(j == 0), stop=(j == J - 1))

    for c in range(H // 2):
        po_all = {}
        for hb, h in heads(c):
            po_all[h] = psum_ot.tile([65, 768], F32, tag="ot", name=f"po{h}")
        etiles = {}
        emit_S_unit(c, 0, etiles)
        for u in range(len(units)):
            if u + 1 < len(units):
                emit_S_unit(c, u + 1, etiles)
            emit_O_unit(c, u, etiles, po_all)
        srow = work.tile([1, 2 * n_q], F32, tag="srow", bufs=2,
                         name=f"sr{c}")
        rr = work.tile([1, 2 * n_q], F32, tag="rr", bufs=2, name=f"rr{c}")
        (hb_e, h_e), (hb_o, h_o) = heads(c)
        # stage denominator rows at partition 0 in SBUF: the custom-DVE
        # reciprocal misreads PSUM at base partition 64 on hardware
        nc.vector.tensor_copy(srow[:, 0:n_q], po_all[h_e][64:65, 0:n_q])
        nc.vector.tensor_copy(srow[:, n_q:2 * n_q],
                              po_all[h_o][64:65, 0:n_q])
        nc.vector.reciprocal_approx_fast(out=rr[:, 0:n_q],
                                         in_=srow[:, 0:n_q])
        nc.vector.reciprocal_approx_fast(out=rr[:, n_q:2 * n_q],
                                         in_=srow[:, n_q:2 * n_q])
        nc.scalar.copy(ot_sb[hb_e, c, :], po_all[h_e][0:64, :n_q])
        nc.scalar.copy(ot_sb[hb_o, c, :], po_all[h_o][0:64, :n_q])
        drp = cx.dram.tile([2, n_q], F32, tag="drp", name=f"drp{c}")
        nc.sync.dma_start(drp[0:1, :], rr[:, 0:n_q])
        nc.sync.dma_start(drp[1:2, :], rr[:, n_q:2 * n_q])
        bc = work.tile([P, n_q], F32, tag="attn_bc", name=f"bc{c}")
        nc.gpsimd.dma_start(bc[0:64, :], drp[0:1, :].to_broadcast([64, n_q]))
        nc.gpsimd.dma_start(bc[64:128, :],
                            drp[1:2, :].to_broadcast([64, n_q]))
        nc.vector.tensor_tensor(ot_sb[:, c, :], ot_sb[:, c, :], bc[:],
                                ALU.mult)


def _emit_attn(nc, cx, work, psum_s, psum_ot, qT_sb, kT_sb, v_sb, ot_sb,
               layer, n_q, n_kv, filler=None):
    """Cross-attention. Per unit: the heads of a pair (x kv-pair for layer 2)
    are emitted as ADJACENT K=64 matmuls into PE row-groups 0/64 (concurrent)
    writing one shared PSUM tile; ONE ACT exp call (wm scale folded in)
    covers the whole tile. O.T accumulates per unit with a ones-augmented V
    column so the softmax denominator lands in PSUM row 64. S for unit u+1 is
    emitted before O of unit u so the PE has independent work while ACT
    computes exp.

    filler: optional per-pair list of callables emitting independent
    full-array PE work (borrowing an s-tag PSUM slot). Attention's K=64 /
    65-row matmuls only half-use the PE array and never re-warm the HAM
    clock gate; dense filler matmuls keep it at 2.4 GHz."""
    J = n_kv // P
    units = _attn_units(layer, n_kv)
    heads = lambda c: ((slice(0, 64), 2 * c), (slice(64, 128), 2 * c + 1))

    def ekey(jl):
        return jl[0]

    def emit_S_unit(c, u, etiles):
        jl, a, b, sc = units[u]
        w = b - a
        nj = len(jl)
        if ekey(jl) not in etiles:
            etiles[ekey(jl)] = work.tile([P, nj * 2, n_q], BF, tag="exps",
                                         bufs=3, name=f"e{c}_{ekey(jl)}")
        E = etiles[ekey(jl)]
        # Bank-safe layout: all head-even S in bank 0 ([0:512]), head-odd in
        # bank 1 ([512:1024]) — the two heads' matmuls execute CONCURRENTLY
        # via PE row-groups 0/64, and concurrent drains into the SAME PSUM
        # bank are a hardware fault. u-order (all-he, then all-ho) keeps the
        # psum stride regular (w) so one strided ACT exp covers the tile.
        ps = psum_s.tile([P, 1024], F32, tag="s", name=f"s{c}_{u}")
        (hb_e, h_e), (hb_o, h_o) = heads(c)
        for ji, j in enumerate(jl):
            nc.tensor.matmul(ps[:, ji * w:(ji + 1) * w],
                             kT_sb[hb_e, c, j * P:(j + 1) * P],
                             qT_sb[hb_e, c, a:b], start=True, stop=True)
            nc.tensor.matmul(ps[:, 512 + ji * w:512 + (ji + 1) * w],
                             kT_sb[hb_o, c, j * P:(j + 1) * P],
                             qT_sb[hb_o, c, a:b], start=True, stop=True)
        ps_g = ps[:, 0:1024].rearrange("p (g q) -> p g q", g=2)
        if nj == 1:
            nc.scalar.activation(E[:, :, a:b], ps_g[:, :, 0:w],
                                 AF.Exp, bias=0.0, scale=sc)
        else:
            nc.scalar.activation(
                E[:, :, a:b].rearrange("p (g j) q -> p g j q", g=2),
                ps_g[:, :, 0:nj * w].rearrange("p g (j q) -> p g j q", j=nj),
                AF.Exp, bias=0.0, scale=sc)

    def emit_O_unit(c, u, etiles, po_all):
        jl, a, b, sc = units[u]
        nj = len(jl)
        E = etiles[ekey(jl)]
        # start/stop must be unique per PSUM BANK (start=True clears the
        # whole bank's has_written bits): only the first/last matmul touching
        # a bank carries the flag; sibling regions in the same bank rely on
        # per-element overwrite-then-accumulate semantics.
        first_in_bank = a % 512 == 0
        last_in_bank = (b % 512 == 0) or (b == n_q)
        for ji, j in enumerate(jl):
            for hi, (hb, h) in enumerate(heads(c)):
                nc.tensor.matmul(
                    po_all[h][0:65, a:b],
                    v_sb[:, j, h, 0:65],
                    E[:, hi * nj + ji, a:b],
                    start=(j == 0 and first_in_bank),
                    stop=(j == J - 1 and last_in_bank))

    fill_i = [0]

    def feed_filler(c):
        # filler entries are (gate, fn): fn may only be emitted once pair
        # `gate` has been normalized (so a stalled piece never parks on an
        # s-slot the attention pipeline needs)
        if filler is not None and fill_i[0] < len(filler):
            gate, fn = filler[fill_i[0]]
            if gate < c:
                fn()
                fill_i[0] += 1

    for c in range(H // 2):
        po_all = {}
        for hb, h in heads(c):
            po_all[h] = psum_ot.tile([65, 768], F32, tag="ot", name=f"po{h}")
        etiles = {}
        emit_S_unit(c, 0, etiles)
        for u in range(len(units)):
            if u + 1 < len(units):
                emit_S_unit(c, u + 1, etiles)
            emit_O_unit(c, u, etiles, po_all)
            feed_filler(c)
        # normalizers: stage denominator rows at partition 0 (custom-DVE
        # reciprocal misreads PSUM at base partition 64 on hardware), recip,
        # DRAM-bounce broadcast. The copies are split across ACT and DVE so
        # the serial tail chain pipelines across both engines.
        srow = work.tile([1, 2 * n_q], F32, tag="srow", bufs=2,
                         name=f"sr{c}")
        rr = work.tile([1, 2 * n_q], F32, tag="rr", bufs=2, name=f"rr{c}")
        (hb_e, h_e), (hb_o, h_o) = heads(c)
        nc.scalar.copy(srow[:, 0:n_q], po_all[h_e][64:65, 0:n_q])
        nc.vector.tensor_copy(srow[:, n_q:2 * n_q],
                              po_all[h_o][64:65, 0:n_q])
        nc.vector.reciprocal_approx_fast(out=rr[:, 0:n_q],
                                         in_=srow[:, 0:n_q])
        nc.vector.reciprocal_approx_fast(out=rr[:, n_q:2 * n_q],
                                         in_=srow[:, n_q:2 * n_q])
        nc.scalar.copy(ot_sb[hb_e, c, :], po_all[h_e][0:64, :n_q])
        nc.vector.tensor_copy(ot_sb[hb_o, c, :], po_all[h_o][0:64, :n_q])
        drp = cx.dram.tile([2, n_q], F32, tag="drp", name=f"drp{c}")
        nc.sync.dma_start(drp[0:1, :], rr[:, 0:n_q])
        nc.sync.dma_start(drp[1:2, :], rr[:, n_q:2 * n_q])
        bc = work.tile([P, n_q], F32, tag="attn_bc", name=f"bc{c}")
        nc.gpsimd.dma_start(bc[0:64, :], drp[0:1, :].to_broadcast([64, n_q]))
        nc.gpsimd.dma_start(bc[64:128, :],
                            drp[1:2, :].to_broadcast([64, n_q]))
        nc.vector.tensor_tensor(ot_sb[:, c, :], ot_sb[:, c, :], bc[:],
                                ALU.mult)
        feed_filler(c + 1)
    # flush remaining filler pieces (their deps are all satisfied now)
    if filler is not None:
        while fill_i[0] < len(filler):
            filler[fill_i[0]][1]()
            fill_i[0] += 1


def _r3(ap):
    """DRAM [K*128, n] -> [128(part), K, n] view for DMA."""
    return ap.rearrange("(ko p) s -> p ko s", p=P)


def _build_program(flags):
    use_bo1, use_bo2, use_fb1, use_fb2, use_g1, use_g2, use_g3 = flags
    nc = _Bacc("TRN2", target_bir_lowering=False, debug=False)

    def din(name, shape, dt=BF):
        return nc.dram_tensor(name, shape, dt, kind="ExternalInput").ap()

    x1T = din("x1T", [D, S1])
    x2T = din("x2T", [D, S2])
    x3T = din("x3T", [D, S3])
    wts = {n: din(n, [D, D]) for n in
           ("wq1T", "wk1T", "wv1T", "wo1T", "wq2T", "wk2T", "wv2T", "wo2T")}
    fw1T = din("fw1T", [D, DF])
    fw2T = din("fw2T", [DF, D])
    bo1 = din("bo1", [P, NCH]) if use_bo1 else None
    bo2 = din("bo2", [P, NCH]) if use_bo2 else None
    fb1 = din("fb1", [P, DF // P]) if use_fb1 else None
    fb2 = din("fb2", [P, NCH]) if use_fb2 else None
    gbd = {}
    for i, use in ((1, use_g1), (2, use_g2), (3, use_g3)):
        gbd[i] = (din(f"g{i}", [P, NCH]),
                  din(f"b{i}", [P, NCH])) if use else None
    yT = nc.dram_tensor("yT", [D, S1], F32, kind="ExternalOutput").ap()
    taps = {}
    if _DEBUG_TAPS:
        for tn in ("t_q1", "t_ot1", "t_y1", "t_ot2", "t_y2"):
            taps[tn] = nc.dram_tensor(tn, [D, S1], BF,
                                      kind="ExternalOutput").ap()

    def tap(name, src):
        if _DEBUG_TAPS:
            nc.sync.dma_start(_r3(taps[name]), src[:])

    with tile.TileContext(nc, pool_alloc_mode="queue") as tc:
        cx = _Ctx()
        cx.tc = tc
        with tc.tile_pool(name="sb", bufs=1) as sb, \
             tc.tile_pool(name="zsq", bufs=1) as zsq_pool, \
             tc.tile_pool(name="wpre", bufs=1) as wpre, \
             tc.tile_pool(name="dram", bufs=2, space="DRAM") as dram:
            cx.sb, cx.zsq_pool, cx.dram = sb, zsq_pool, dram

            ones_bf = sb.tile([P, 1], BF, tag="ones_bf")
            nc.vector.memset(ones_bf[:], 1.0)
            cx.ones_bf = ones_bf
            ones128 = sb.tile([1, P], BF, tag="ones128")
            nc.vector.memset(ones128[:], 1.0)
            cx.ones128 = ones128
            eps_sb = sb.tile([P, 1], F32, tag="eps")
            nc.vector.memset(eps_sb[:], EPS)
            cx.eps_sb = eps_sb

            def load_pm(ap, cols, tag):
                if ap is None:
                    return None
                t = sb.tile([P, cols], BF, tag=tag)
                nc.sync.dma_start(t[:], ap)
                return t

            bo1_sb = load_pm(bo1, NCH, "bo1")
            bo2_sb = load_pm(bo2, NCH, "bo2")
            fb1_sb = load_pm(fb1, DF // P, "fb1")
            fb2_sb = load_pm(fb2, NCH, "fb2")
            gb_sb = {}
            for i in (1, 2, 3):
                gb_sb[i] = None if gbd[i] is None else (
                    load_pm(gbd[i][0], NCH, f"g{i}"),
                    load_pm(gbd[i][1], NCH, f"b{i}"))

            y1_sb = sb.tile([P, NCH, S1], BF, tag="y1")
            y2_sb = sb.tile([P, NCH, S1], BF, tag="y2")

            def copy_cb(dst, eng):
                return lambda m, ps: eng(dst[:, m, :], ps)

            dve_copy = lambda out, ps: nc.vector.tensor_copy(out, ps)
            act_copy = (lambda out, ps: nc.scalar.copy(out, ps)) \
                if _USE_ACT_COPY else dve_copy

            def emit_v_proj(psum_pool, x_sb, wv_sb, v_sb, Jkv):
                nc.vector.memset(v_sb[:, :, :, 64:65], 1.0)
                for j in range(Jkv):
                    ps = psum_pool.tile([P, 1024], F32, tag="proj",
                                        name=f"vps{j}")
                    for ko in range(NCH):
                        nc.tensor.matmul(
                            ps[:, 0:D],
                            x_sb[:, ko, j * P:(j + 1) * P],
                            wv_sb[:, ko, :],
                            start=(ko == 0), stop=(ko == NCH - 1))
                    nc.vector.tensor_copy(
                        v_sb[:, j, :, 0:64],
                        ps[:, 0:D].rearrange("p (h v) -> p h v", h=H))

            # open order is reverse of close order (pool stack is LIFO)
            kv2 = tc.tile_pool(name="kv2", bufs=1)
            with kv2 as kv2p:
                x3_sb = kv2p.tile([P, NCH, S3], BF, tag="xkv")
                wk2_sb = kv2p.tile([P, NCH, D], BF, tag="wk")
                wv2_sb = kv2p.tile([P, NCH, D], BF, tag="wv")
                k2_sb = kv2p.tile([P, NCH, S3], BF, tag="k")
                v2_sb = kv2p.tile([P, S3 // P, H, 65], BF, tag="v")

                otp1 = contextlib.ExitStack()
                otp1p = otp1.enter_context(tc.tile_pool(name="otp1", bufs=1))
                x1_sb = otp1p.tile([P, NCH, S1], BF, tag="x1")
                ot_sb = otp1p.tile([P, NCH, S1], BF, tag="ot1")

                at1_ctx = contextlib.ExitStack()
                at1p = at1_ctx.enter_context(tc.tile_pool(name="at1", bufs=1))
                q_sb = at1p.tile([P, NCH, S1], BF, tag="q")
                k_sb = at1p.tile([P, NCH, S2], BF, tag="k")
                v_sb = at1p.tile([P, S2 // P, H, 65], BF, tag="v")

                kv1_ctx = contextlib.ExitStack()
                kv1p = kv1_ctx.enter_context(tc.tile_pool(name="kv1", bufs=1))
                wq_sb = kv1p.tile([P, NCH, D], BF, tag="wq")
                x2_sb = kv1p.tile([P, NCH, S2], BF, tag="xkv")
                wk_sb = kv1p.tile([P, NCH, D], BF, tag="wk")
                wv_sb = kv1p.tile([P, NCH, D], BF, tag="wv")
                # per-chunk DMAs in consumption order, issued across FOUR
                # engine DGE queues in parallel (descriptor generation is
                # ~1us serial per engine): the ko-major Q1/K1 projections
                # start as soon as their first chunks land
                for ko in range(NCH):
                    nc.sync.dma_start(wq_sb[:, ko, :],
                                      _r3(wts["wq1T"])[:, ko, :])
                    nc.gpsimd.dma_start(x1_sb[:, ko, :], _r3(x1T)[:, ko, :])
                for ko in range(NCH):
                    nc.scalar.dma_start(wk_sb[:, ko, :],
                                        _r3(wts["wk1T"])[:, ko, :])
                    nc.scalar.dma_start(x2_sb[:, ko, :], _r3(x2T)[:, ko, :])
                nc.sync.dma_start(wv_sb[:], _r3(wts["wv1T"]))
                nc.gpsimd.dma_start(x3_sb[:], _r3(x3T))
                nc.sync.dma_start(wk2_sb[:], _r3(wts["wk2T"]))
                nc.sync.dma_start(wv2_sb[:], _r3(wts["wv2T"]))

                # prefetch every later-phase weight now (bf16 fits in SBUF)
                wo1_sb = wpre.tile([P, NCH, D], BF, tag="wo1")
                wq2_sb = wpre.tile([P, NCH, D], BF, tag="wq2")
                wo2_sb = wpre.tile([P, NCH, D], BF, tag="wo2")
                fw1_sb = wpre.tile([P, NCH, DF], BF, tag="fw1")
                fw2_sb = wpre.tile([P, DF // P, D], BF, tag="fw2")
                nc.sync.dma_start(wo1_sb[:], _r3(wts["wo1T"]))
                nc.sync.dma_start(wq2_sb[:], _r3(wts["wq2T"]))
                nc.sync.dma_start(wo2_sb[:], _r3(wts["wo2T"]))
                nc.sync.dma_start(fw1_sb[:], _r3(fw1T))
                nc.sync.dma_start(fw2_sb[:], _r3(fw2T))

                # dense warm-up block: q1,k1,v1 (q1/k1 ko-major so the PE
                # starts on the first DMA'd chunks); k2/v2 are deferred into
                # the WO1/LN1 and WO2/LN2 phases as independent PE filler
                with tc.tile_pool(name="psA", bufs=4, space="PSUM") as psA:
                    _emit_proj(nc, psA, wq_sb, x1_sb, D, S1, NCH,
                               copy_cb(q_sb, act_copy), ko_major=True)
                    _emit_proj(nc, psA, wk_sb, x2_sb, D, S2, NCH,
                               copy_cb(k_sb, act_copy), ko_major=True)
                    emit_v_proj(psA, x2_sb, wv_sb, v_sb, S2 // P)
                kv1_ctx.close()  # frees x2 + wq1/wk1/wv1 SBUF

                # attention 1, with K2/V2 projection pieces as full-array
                # PE filler woven between j-groups (keeps the HAM clock warm
                # through the half-array attention matmuls)
                attn_fn = _emit_attn_base if _USE_BASE_ATTN else _emit_attn
                s_bufs = 4 if _USE_BASE_ATTN else 2
                nc.vector.memset(v2_sb[:, :, :, 64:65], 1.0)
                with tc.tile_pool(name="wk1w", bufs=3) as work, \
                     tc.tile_pool(name="ps_s1", bufs=s_bufs,
                                  space="PSUM") as pss, \
                     tc.tile_pool(name="ps_ot1", bufs=2,
                                  space="PSUM") as psot:
                    def mk_k2(m, a, b):
                        def f():
                            ps = pss.tile([P, 1024], F32, tag="s",
                                          name=f"fk2_{m}_{a}")
                            for ko in range(NCH):
                                nc.tensor.matmul(
                                    ps[:, 0:b - a],
                                    wk2_sb[:, ko, m * P:(m + 1) * P],
                                    x3_sb[:, ko, a:b],
                                    start=(ko == 0), stop=(ko == NCH - 1))
                            nc.vector.tensor_copy(k2_sb[:, m, a:b],
                                                  ps[:, 0:b - a])
                        return f

                    def mk_v2(j, half):
                        def f():
                            ps = pss.tile([P, 1024], F32, tag="s",
                                          name=f"fv2_{j}_{half}")
                            a = half * 256
                            for ko in range(NCH):
                                nc.tensor.matmul(
                                    ps[:, 0:256],
                                    x3_sb[:, ko, j * P:(j + 1) * P],
                                    wv2_sb[:, ko, a:a + 256],
                                    start=(ko == 0), stop=(ko == NCH - 1))
                            nc.vector.tensor_copy(
                                v2_sb[:, j, 4 * half:4 * half + 4, 0:64],
                                ps[:, 0:256].rearrange("p (h v) -> p h v",
                                                       h=4))
                        return f

                    fillers = []
                    for m in range(NCH):
                        fillers.append((-1, mk_k2(m, 0, 256)))
                        fillers.append((-1, mk_k2(m, 256, 512)))
                        fillers.append((-1, mk_k2(m, 512, 768)))
                    for j in range(S3 // P):
                        fillers.append((-1, mk_v2(j, 0)))
                        fillers.append((-1, mk_v2(j, 1)))
                    if _USE_BASE_ATTN:
                        attn_fn(nc, cx, work, pss, psot, q_sb, k_sb, v_sb,
                                ot_sb, 1, S1, S2)
                    else:
                        attn_fn(nc, cx, work, pss, psot, q_sb, k_sb, v_sb,
                                ot_sb, 1, S1, S2, filler=fillers)
                tap("t_q1", q_sb)
                tap("t_ot1", ot_sb)
                at1_ctx.close()  # frees q1/k1/v1 SBUF

                # wo1 + LN1 (wo1 ko-major in 2-m groups: the first WO matmuls
                # need only attention pair 0's normalized output)
                with tc.tile_pool(name="psB1", bufs=2, space="PSUM") as psB:
                    wo_ps = {}
                    _emit_proj(nc, psB, wo1_sb, ot_sb, D, S1, NCH,
                               lambda m, ps: wo_ps.__setitem__(m, ps),
                               ko_major=True, m_group=2)
                    _emit_ln(nc, cx, psB, lambda m: wo_ps[m], x1_sb,
                             bo1_sb, y1_sb, y1_sb, S1, gb_sb[1])
                tap("t_y1", y1_sb)
                otp1.close()

                # q2 projection (ko-major: starts as soon as y1 chunk 0 is
                # normalized)
                otp2 = contextlib.ExitStack()
                otp2p = otp2.enter_context(tc.tile_pool(name="otp2", bufs=1))
                ot2_sb = otp2p.tile([P, NCH, S1], BF, tag="ot2")
                q2_sb = otp2p.tile([P, NCH, S1], BF, tag="q2")
                with tc.tile_pool(name="psC", bufs=4, space="PSUM") as psC:
                    _emit_proj(nc, psC, wq2_sb, y1_sb, D, S1, NCH,
                               copy_cb(q2_sb, act_copy), ko_major=_KO_MAJOR)

                # attention 2
                with tc.tile_pool(name="wk2w", bufs=3) as work2, \
                     tc.tile_pool(name="ps_s2", bufs=s_bufs,
                                  space="PSUM") as pss2, \
                     tc.tile_pool(name="ps_ot2", bufs=2,
                                  space="PSUM") as psot2:
                    attn_fn(nc, cx, work2, pss2, psot2, q2_sb, k2_sb,
                            v2_sb, ot2_sb, 2, S1, S3)
                tap("t_ot2", ot2_sb)

                # wo2 + LN2
                with tc.tile_pool(name="psD", bufs=2, space="PSUM") as psD:
                    wo2_ps = {}
                    _emit_proj(nc, psD, wo2_sb, ot2_sb, D, S1, NCH,
                               lambda m, ps: wo2_ps.__setitem__(m, ps),
                               ko_major=True, m_group=2)
                    _emit_ln(nc, cx, psD, lambda m: wo2_ps[m], y1_sb,
                             bo2_sb, y2_sb, y2_sb, S1, gb_sb[2])
                tap("t_y2", y2_sb)
                otp2.close()

            # FFN + LN3
            zbuf = sb.tile([P, NCH, S1], BF, tag="y1")  # reuse y1 slot
            yT_sb = sb.tile([P, NCH, S1], F32, tag="yT")
            with tc.tile_pool(name="ffn1", bufs=1) as f1p:
                h_sb = f1p.tile([P, DF // P, S1], BF, tag="hT")
                with tc.tile_pool(name="psE", bufs=3, space="PSUM") as psE:
                    def gelu_consume(m, ps):
                        nc.scalar.activation(
                            h_sb[:, m, :], ps[:, 0:S1], AF.Gelu,
                            bias=(fb1_sb[:, m:m + 1]
                                  if fb1_sb is not None else 0.0),
                            scale=1.0)
                    # ko-major pairs: the first FFN1 matmuls need only y2
                    # chunk 0, starting inside LN2's normalize window
                    _emit_proj(nc, psE, fw1_sb, y2_sb, DF, S1, NCH,
                               gelu_consume, ko_major=True, m_group=2)

                with tc.tile_pool(name="psF", bufs=2, space="PSUM") as psF:
                    f2_ps = {}
                    _emit_proj(nc, psF, fw2_sb, h_sb, D, S1, DF // P,
                               lambda m, ps: f2_ps.__setitem__(m, ps))

                    out_eng = [nc.sync, nc.scalar, nc.gpsimd, nc.sync]

                    def out_dma(m):
                        out_eng[m].dma_start(_r3(yT)[:, m, :],
                                             yT_sb[:, m, :])

                    _emit_ln(nc, cx, psF, lambda m: f2_ps[m], y2_sb, fb2_sb,
                             zbuf, yT_sb, S1, gb_sb[3], out_cb=out_dma)

    nc.finalize()
    return nc


def _to_pm(vec, cols):
    """[cols*128] vector -> [128, cols] partition-major layout (bf16)."""
    return np.ascontiguousarray(vec.reshape(cols, P).T).astype(
        ml_dtypes.bfloat16)


def _bf(a):
    return np.ascontiguousarray(a).astype(ml_dtypes.bfloat16)


def kernel(**inputs):
    cords = np.asarray(inputs["cords_features"], np.float32)
    spatial = np.asarray(inputs["spatial_features"], np.float32)
    speed = np.asarray(inputs["speed_features"], np.float32)
    B = cords.shape[0]
    assert B == 8

    def g(name):
        return np.asarray(inputs[name], np.float32)

    flags = (
        not np.allclose(g("bo1"), 0), not np.allclose(g("bo2"), 0),
        not np.allclose(g("ffn_b1"), 0), not np.allclose(g("ffn_b2"), 0),
        not (np.allclose(g("ln1_g"), 1) and np.allclose(g("ln1_b"), 0)),
        not (np.allclose(g("ln2_g"), 1) and np.allclose(g("ln2_b"), 0)),
        not (np.allclose(g("ln3_g"), 1) and np.allclose(g("ln3_b"), 0)),
    )
    if flags not in _PROGRAM_CACHE:
        _PROGRAM_CACHE[flags] = _build_program(flags)
    nc = _PROGRAM_CACHE[flags]

    shared = {
        "wq1T": _bf(g("wq1").T), "wk1T": _bf(g("wk1").T),
        "wv1T": _bf(g("wv1").T), "wo1T": _bf(g("wo1").T),
        "wq2T": _bf(g("wq2").T), "wk2T": _bf(g("wk2").T),
        "wv2T": _bf(g("wv2").T), "wo2T": _bf(g("wo2").T),
        "fw1T": _bf(g("ffn_w1").T), "fw2T": _bf(g("ffn_w2").T),
    }
    use_bo1, use_bo2, use_fb1, use_fb2, use_g1, use_g2, use_g3 = flags
    if use_bo1:
        shared["bo1"] = _to_pm(g("bo1"), NCH)
    if use_bo2:
        shared["bo2"] = _to_pm(g("bo2"), NCH)
    if use_fb1:
        shared["fb1"] = _to_pm(g("ffn_b1"), DF // P)
    if use_fb2:
        shared["fb2"] = _to_pm(g("ffn_b2"), NCH)
    for i, use in ((1, use_g1), (2, use_g2), (3, use_g3)):
        if use:
            shared[f"g{i}"] = _to_pm(g(f"ln{i}_g"), NCH)
            shared[f"b{i}"] = _to_pm(g(f"ln{i}_b"), NCH)

    in_maps = []
    for b in range(B):
        m = dict(shared)
        m["x1T"] = _bf(cords[b].T)
        m["x2T"] = _bf(spatial[b].T)
        m["x3T"] = _bf(speed[b].T)
        in_maps.append(m)

    global _LAST_IN_MAPS
    _LAST_IN_MAPS = in_maps
    res = run_bass_kernel_spmd(nc, in_maps, core_ids=list(range(B)))
    out = np.stack([res.results[b]["yT"].T for b in range(B)], axis=0)
    return np.ascontiguousarray(out.astype(np.float32))
